# revision 44
# baseline (speedup 1.0000x reference)
"""Trainium2 Bass kernel for nn_ConicaLayer (transformer decoder layer:
self-attn (causal) + cross-attn + FFN, post-LN residuals).

Sharding: rows (B x L) split across 8 cores; core c -> batch b=c//4, and 4
interleaved 128-row blocks {i, 7-i, 8+i, 15-i} of the 16 blocks of that batch
(balances causal attention work). Each core computes full K/V for its batch.

All transposes/packing are done host-side; on-device activations stay d-major
([D, tokens]) end-to-end. Softmax uses exp(s-20) without max-subtraction
(scores are bounded; masked entries handled by multiplying exp(mask), with
fully-masked blocks skipped in causal mode). The V matrix carries an appended
ones column so PV matmuls also produce softmax denominators. V-bias folds into
the out-proj bias host-side (bo_eff = bo + wo @ bv, exact since sum(p) = 1).

Dtypes: bf16 for kv-path (xT/encT, wk/wv, k/v/q/e), attention out + wo, f1 +
w2. f32 for residual stream, wq/w1, layernorms, all PSUM accumulation.
"""

import sys
import numpy as np

try:
    import concourse.bass as bass  # noqa: F401
except ImportError:
    sys.path.insert(0, "/opt/trn_rl_repo")

import ml_dtypes
import concourse.bass as bass
import concourse.bacc as bacc
import concourse.tile as tile
from concourse import mybir
from concourse.bass import ts

BF16 = ml_dtypes.bfloat16

P = 128
B, L, S, D, H, DFF = 2, 2048, 1024, 1024, 16, 4096
DH = D // H           # 64
KC = D // P           # 8
KC2 = DFF // P        # 32
NL = 4                # l-blocks per core
LW = 128              # l width per block
LTOT = NL * LW        # 512 rows per core
TC_SA = L // P        # 16 t-chunks (self attn)
TC_CA = S // P        # 8 t-chunks (cross attn)
MREG = 4              # masked tail chunks per block (causal mode)
EXT_CAUSAL = [4, 8, 12, 16]
EXP_SHIFT = -20.0
EPS = 1e-5

f32 = mybir.dt.float32
bf = mybir.dt.bfloat16
f8 = mybir.dt.float8e4
FP8 = mybir.dt.np(f8)
DR = mybir.MatmulPerfMode.DoubleRow
AF = mybir.ActivationFunctionType
OP = mybir.AluOpType

# fp8 scaling: activations x16, weights x512. q/k carry the combined factor
# A into SBUF (descaled for free in the exp's scale arg); v carries A into
# the PV output (descaled by dividing wo by A host-side).
S_X = 16.0
S_W = 512.0
A_SC = S_X * S_W
INV_A2 = 1.0 / (A_SC * A_SC)
INV_A = 1.0 / A_SC
INV_W = 1.0 / S_W
S_O = 32.0
INV_OW = 1.0 / (S_O * S_W)


def core_blocks(i):
    return [i, 7 - i, 8 + i, 15 - i]


# ---------------------------------------------------------------------------
# Bass kernel builder
# ---------------------------------------------------------------------------

def build_nc(sa_mode, ca_mode):
    """sa_mode: 'causal' | 'zeros' | 'generic'; ca_mode: 'zeros' | 'generic'."""
    nc = bacc.Bacc("TRN2", target_bir_lowering=False, debug=False, num_devices=8)

    def din(name, shape, dtype=f32):
        return nc.dram_tensor(name, list(shape), dtype, kind="ExternalInput").ap()

    env = {}
    env["sa_mode"], env["ca_mode"] = sa_mode, ca_mode
    env["exts"] = EXT_CAUSAL if sa_mode == "causal" else [TC_SA] * NL
    env["xT_d"] = din("xT", [P, KC, L], f8)
    env["xTown_d"] = din("xTown", [P, KC, LTOT])
    env["xTownb_d"] = din("xTownb", [P, KC, LTOT], f8)
    env["encT_d"] = din("encT", [P, KC, S], f8)
    wdt = {"wq_sa": f8, "wk_sa": f8, "wv_sa": f8, "wo_sa": f8,
           "wq_ca": f8, "wk_ca": f8, "wv_ca": f8, "wo_ca": f8}
    env["wd"] = {n: din(n, [P, KC, D], dt) for n, dt in wdt.items()}
    env["w1_d"] = din("w1", [P, KC, DFF], bf)
    env["w2_d"] = din("w2", [P, KC2, D], bf)
    env["biases_d"] = din("biases", [P, 10 * KC + KC2])
    env["gd"] = {n: din(n, [1, D], bf) for n in ["g1", "g2", "g3"]}
    env["expm_d"] = None
    if sa_mode == "causal":
        env["expm_d"] = din("expm", [P, NL, MREG, LW], bf)
    elif sa_mode == "generic":
        env["expm_d"] = din("expm", [P, TC_SA * NL, LW], bf)
    env["expmc_d"] = din("expmc", [P, TC_CA, LTOT], bf) if ca_mode == "generic" else None
    env["out_d"] = nc.dram_tensor("out", [P, KC, LTOT], f32, kind="ExternalOutput").ap()

    with tile.TileContext(nc) as tc:
        _build_body(nc, tc, env)
    nc.compile()
    return nc


def _build_body(nc, tc, env):
    from contextlib import ExitStack

    xT_d, xTown_d, encT_d = env["xT_d"], env["xTown_d"], env["encT_d"]
    xTownb_d = env["xTownb_d"]
    wd, w1_d, w2_d, biases_d, gd = env["wd"], env["w1_d"], env["w2_d"], env["biases_d"], env["gd"]
    expm_d, expmc_d, out_d = env["expm_d"], env["expmc_d"], env["out_d"]
    sa_mode, ca_mode, exts = env["sa_mode"], env["ca_mode"], env["exts"]

    with ExitStack() as ctx:
        consts = ctx.enter_context(tc.tile_pool(name="consts", bufs=1))

        # ---- constants (tiles allocated now; DMAs deferred via load_consts
        # so the startup DMA queue serves the k-proj inputs first) ----
        biases_sb = consts.tile([P, 10 * KC + KC2], f32, tag="c_bias")
        bias_names = ["bq_sa", "bk_sa", "bo_sa", "bq_ca", "bk_ca", "bo_ca",
                      "b2", "lb1", "lb2", "lb3"]
        bias_sb = {n: biases_sb[:, i * KC:(i + 1) * KC]
                   for i, n in enumerate(bias_names)}
        b1_sb = biases_sb[:, 10 * KC:10 * KC + KC2]
        ones128b = consts.tile([P, 1], bf, tag="ones128b")
        nc.vector.memset(ones128b, 1.0)
        ones64b = consts.tile([1, DH], bf, tag="ones64b")
        nc.vector.memset(ones64b, 1.0)
        eps_sb = consts.tile([1, 1], f32, tag="eps")
        nc.vector.memset(eps_sb, EPS)
        zero128 = consts.tile([P, 1], f32, tag="zero128")
        nc.vector.memset(zero128, 0.0)
        shift128 = consts.tile([P, 1], f32, tag="shift128")
        nc.vector.memset(shift128, EXP_SHIFT)
        expm_sb = None
        if expm_d is not None:
            shape = [P, NL, MREG, LW] if sa_mode == "causal" else [P, TC_SA * NL, LW]
            expm_sb = consts.tile(shape, bf, tag="expm")
        expmc_sb = None
        if expmc_d is not None:
            expmc_sb = consts.tile([P, TC_CA, LTOT], bf, tag="expmc")

        def load_consts():
            nc.sync.dma_start(biases_sb, biases_d)
            if expm_sb is not None:
                nc.sync.dma_start(expm_sb, expm_d)
            if expmc_sb is not None:
                nc.sync.dma_start(expmc_sb, expmc_d)

        # ------------- helpers -------------
        def proj_to(wpool, wtag, wdt, src_sb, w_dram, n_oc, evict, psum_proj, n_tt=1,
                    tt_width=LTOT, n_kc=KC, dr=False, wt0=None):
            for oc in range(n_oc):
                if oc == 0 and wt0 is not None:
                    wt = wt0
                else:
                    wt = wpool.tile([P, n_kc, P], wdt, tag=wtag)
                    nc.sync.dma_start(wt, w_dram[:, :, ts(oc, P)])
                for tt in range(n_tt):
                    ps = psum_proj.tile([P, tt_width], f32, tag="psproj")
                    if dr:
                        for k2 in range(n_kc // 2):
                            nc.tensor.matmul(
                                ps, wt[:, 2 * k2:2 * k2 + 2, :],
                                src_sb[:, 2 * k2:2 * k2 + 2, ts(tt, tt_width)],
                                start=(k2 == 0), stop=(k2 == n_kc // 2 - 1),
                                perf_mode=DR)
                    else:
                        for kc in range(n_kc):
                            nc.tensor.matmul(
                                ps, wt[:, kc, :], src_sb[:, kc, ts(tt, tt_width)],
                                start=(kc == 0), stop=(kc == n_kc - 1))
                    evict(oc, tt, ps)

        def layer_norm(tag, x_sb, g_dram, lb, out_sb, chunk_done=None, bf_out=None,
                       bf_scale=None):
            # Stats and per-token scale broadcasts run as bf16 matmuls (4x the
            # fp32 matmul rate); the residual/output path stays fp32.
            with ExitStack() as lctx:
                lnp = lctx.enter_context(tc.tile_pool(
                    name=f"lnp_{tag}", bufs=1 if sa_mode == "generic" else 2))
                lns = lctx.enter_context(tc.tile_pool(name=f"lns_{tag}", bufs=1))
                psum_st = lctx.enter_context(
                    tc.tile_pool(name=f"psum_lns_{tag}", bufs=1, space="PSUM"))
                psum_ln = lctx.enter_context(
                    tc.tile_pool(name=f"psum_ln_{tag}", bufs=3, space="PSUM"))
                g = lns.tile([1, D], bf, tag="ln_g")
                nc.sync.dma_start(g, g_dram)
                psx = psum_st.tile([1, LTOT], f32, tag="ln_sx")
                psx2 = psum_st.tile([1, LTOT], f32, tag="ln_sx2")
                for kc in range(KC):
                    xb = lnp.tile([P, LTOT], bf, tag="ln_xb")
                    nc.vector.tensor_copy(xb, x_sb[:, kc, :])
                    sq = lnp.tile([P, LTOT], bf, tag="ln_sq")
                    nc.scalar.activation(sq, x_sb[:, kc, :], AF.Square, bias=zero128)
                    nc.tensor.matmul(psx, ones128b, xb,
                                     start=(kc == 0), stop=(kc == KC - 1))
                    nc.tensor.matmul(psx2, ones128b, sq,
                                     start=(kc == 0), stop=(kc == KC - 1))
                mean = lns.tile([1, LTOT], f32, tag="ln_mean")
                nc.vector.tensor_scalar_mul(mean, psx, 1.0 / D)
                msq = lns.tile([1, LTOT], f32, tag="ln_msq")
                nc.vector.tensor_tensor(msq, mean, mean, OP.mult)
                var = lns.tile([1, LTOT], f32, tag="ln_var")
                nc.vector.scalar_tensor_tensor(var, psx2, 1.0 / D, msq, OP.mult, OP.subtract)
                std = lns.tile([1, LTOT], f32, tag="ln_std")
                nc.scalar.activation(std, var, AF.Sqrt, bias=eps_sb)
                rstd = lns.tile([1, LTOT], f32, tag="ln_rstd")
                nc.vector.reciprocal_approx_fast(rstd, std)
                rstd_b = lns.tile([1, LTOT], bf, tag="ln_rstdb")
                nc.vector.tensor_copy(rstd_b, rstd)
                mrstd_b = lns.tile([1, LTOT], bf, tag="ln_mrstdb")
                nc.vector.tensor_tensor(mrstd_b, mean, rstd, OP.mult)
                for kc in range(KC):
                    pg = psum_ln.tile([P, LTOT], f32, tag="ln_pg")
                    pm = psum_ln.tile([P, LTOT], f32, tag="ln_pm")
                    nc.tensor.matmul(pg, g[:, ts(kc, P)], rstd_b, start=True, stop=True)
                    nc.tensor.matmul(pm, g[:, ts(kc, P)], mrstd_b, start=True, stop=True)
                    t = lnp.tile([P, LTOT], f32, tag="ln_t")
                    nc.vector.tensor_tensor(t, x_sb[:, kc, :], pg, OP.mult)
                    nc.vector.scalar_tensor_tensor(out_sb[:, kc, :], t, lb[:, kc:kc + 1],
                                                   pm, OP.add, OP.subtract)
                    if bf_out is not None:
                        if bf_scale is not None:
                            nc.vector.tensor_scalar_mul(bf_out[:, kc, :],
                                                        out_sb[:, kc, :], bf_scale)
                        else:
                            nc.vector.tensor_copy(bf_out[:, kc, :], out_sb[:, kc, :])
                    if chunk_done is not None:
                        chunk_done(kc)

        def kv_proj_v(vpool_dst, src_sb, wv_dram, wrhs, wrhs_tag, n_tc, psum_proj):
            for half in range(2):
                wvh = wrhs.tile([P, KC, 512], f8, tag=wrhs_tag)
                nc.sync.dma_start(wvh, wv_dram[:, :, ts(half, 512)])
                for tci in range(n_tc):
                    ps = psum_proj.tile([P, 512], f32, tag="psproj")
                    for k2 in range(KC // 2):
                        nc.tensor.matmul(
                            ps, src_sb[:, 2 * k2:2 * k2 + 2, ts(tci, P)],
                            wvh[:, 2 * k2:2 * k2 + 2, :],
                            start=(k2 == 0), stop=(k2 == KC // 2 - 1),
                            perf_mode=DR)
                    nc.vector.tensor_copy(
                        vpool_dst[:, tci, half * 8:(half + 1) * 8, 0:DH],
                        ps.rearrange("p (h d) -> p h d", h=8))

        def den_recip(den_row, at_pool, inner):
            # den_row: [1, *inner] PSUM slice holding softmax denominators.
            # The PSUM->SBUF copy runs on ScalarE (Copy is in every table) so
            # only the reciprocal+cast sit on the vector queue.
            sums = at_pool.tile([1] + inner, f32, tag="at_sums")
            nc.scalar.activation(sums, den_row, AF.Copy, bias=0.0)
            recip = at_pool.tile([1] + inner, f32, tag="at_recip")
            nc.vector.reciprocal_approx_fast(recip, sums)
            recip_b = at_pool.tile([1] + inner, bf, tag="at_recipb")
            nc.vector.tensor_scalar_mul(recip_b, recip, S_O / A_SC)
            return recip_b

        def apply_norm(dst, pv_rows, recip_b, at_pool, psum_bc, inner):
            # dst = pv_rows * broadcast(recip)
            pb = psum_bc.tile([DH] + inner, f32, tag="bc")
            nc.tensor.matmul(pb, ones64b, recip_b, start=True, stop=True)
            bc_sb = at_pool.tile([DH] + inner, f32, tag="at_bc")
            nc.vector.tensor_copy(bc_sb, pb)
            if len(inner) == 2:
                dst = dst.rearrange("p (j l) -> p j l", j=inner[0])
            nc.vector.scalar_tensor_tensor(dst, pv_rows, 0.0, bc_sb,
                                           OP.bypass, OP.mult)

        # =================== SA ===================
        with ExitStack() as sctx:
            sa_pool = sctx.enter_context(tc.tile_pool(name="sa", bufs=1))
            kT_sb = sa_pool.tile([P, KC, L], bf, tag="kT")
            v_sb = sa_pool.tile([P, TC_SA, H, DH + 1], bf, tag="v")
            qT_sb = sa_pool.tile([P, KC, LTOT], bf, tag="qT")
            nc.gpsimd.memset(v_sb[:, :, :, DH:DH + 1], 1.0)

            with ExitStack() as xctx:
                xpool = xctx.enter_context(tc.tile_pool(name="xpool", bufs=1))
                wrhs = xctx.enter_context(tc.tile_pool(name="wrhs", bufs=2))
                wkp = xctx.enter_context(tc.tile_pool(name="wk_sa_p", bufs=3))
                psum_kv = xctx.enter_context(tc.tile_pool(name="psum_kv", bufs=4, space="PSUM"))
                # first weight tile and the xT chunks lead the DMA queue so
                # the k-proj can start ~4us in; const loads queue behind them.
                wt0k = wkp.tile([P, KC, P], f8, tag="wtb")
                nc.sync.dma_start(wt0k, wd["wk_sa"][:, :, ts(0, P)])
                xT_sb = xpool.tile([P, KC, L], f8, tag="xT")
                for kc in range(KC):
                    nc.sync.dma_start(xT_sb[:, kc, :], xT_d[:, kc, :])
                qsrc = xpool.tile([P, KC, LTOT], f8, tag="qsrc")
                nc.sync.dma_start(qsrc, xTownb_d)
                load_consts()

                def evk(oc, tt, ps):
                    nc.vector.tensor_scalar_add(kT_sb[:, oc, ts(tt, 512)], ps,
                                                bias_sb["bk_sa"][:, oc:oc + 1])
                proj_to(wkp, "wtb", f8, xT_sb, wd["wk_sa"], KC, evk, psum_kv,
                        n_tt=L // 512, tt_width=512, dr=True, wt0=wt0k)

                def evq(oc, tt, ps):
                    nc.vector.tensor_scalar_add(qT_sb[:, oc, :], ps,
                                                bias_sb["bq_sa"][:, oc:oc + 1])
                proj_to(wkp, "wtb", f8, qsrc, wd["wq_sa"], KC, evq, psum_kv, dr=True)

                kv_proj_v(v_sb, xT_sb, wd["wv_sa"], wrhs, "wrhs", TC_SA, psum_kv)

            ca_pool = ctx.enter_context(tc.tile_pool(name="ca", bufs=1, side="right"))
            kcT_sb = ca_pool.tile([P, KC, S], bf, tag="kcT")
            vc_sb = ca_pool.tile([P, TC_CA, H, DH + 1], bf, tag="vc")
            encT_sb = ca_pool.tile([P, KC, S], f8, tag="encT")
            for kc in range(KC):
                nc.sync.dma_start(encT_sb[:, kc, :], encT_d[:, kc, :])
            nc.gpsimd.memset(vc_sb[:, :, :, DH:DH + 1], 1.0)
            oT_sb = sctx.enter_context(tc.tile_pool(name="oT_sa", bufs=1)).tile(
                [P, KC, LTOT], f8, tag="oT")
            pre_pool = ctx.enter_context(tc.tile_pool(name="prep", bufs=1, side="right"))
            h1pre = pre_pool.tile([P, KC, LTOT], f32, tag="pre")
            nc.sync.dma_start(h1pre, xTown_d)

            with ExitStack() as actx:
                e_pool = actx.enter_context(tc.tile_pool(name="e_sa", bufs=2))
                at_pool = actx.enter_context(tc.tile_pool(name="at_sa", bufs=2))
                wkvc = actx.enter_context(tc.tile_pool(name="wkv_ca", bufs=2))
                wrhsc = actx.enter_context(tc.tile_pool(name="wrhs_ca", bufs=2))
                psum_s = actx.enter_context(tc.tile_pool(name="psum_s", bufs=2, space="PSUM"))
                psum_pv = actx.enter_context(tc.tile_pool(name="psum_pv", bufs=1, space="PSUM"))
                psum_bc = actx.enter_context(tc.tile_pool(name="psum_bc", bufs=1, space="PSUM"))
                psum_ckv = actx.enter_context(tc.tile_pool(name="psum_ckv", bufs=1, space="PSUM"))

                ca_state = {}

                def ca_kv_prefetch(hc):
                    # issue the CA weight DMAs early so the chunk's matmuls
                    # never wait on the queue
                    wt = wkvc.tile([P, KC, P], f8, tag="wt_ck")
                    nc.sync.dma_start(wt, wd["wk_ca"][:, :, ts(hc, P)])
                    ca_state[f"wt{hc}"] = wt
                    if hc % 4 == 0:
                        wvh_new = wrhsc.tile([P, KC, 512], f8, tag="wv_ca")
                        ca_state["wvh"] = wvh_new
                        nc.sync.dma_start(wvh_new,
                                          wd["wv_ca"][:, :, ts(hc // 4, 512)])

                def ca_kv_chunk(hc):
                    # 1/8 of CA k-proj and v-proj, emitted between SA head pairs
                    wt = ca_state.pop(f"wt{hc}")
                    for tt in range(S // 512):
                        ps = psum_ckv.tile([P, 512], f32, tag="ps_ckv")
                        for k2 in range(KC // 2):
                            nc.tensor.matmul(ps, wt[:, 2 * k2:2 * k2 + 2, :],
                                             encT_sb[:, 2 * k2:2 * k2 + 2, ts(tt, 512)],
                                             start=(k2 == 0), stop=(k2 == KC // 2 - 1),
                                             perf_mode=DR)
                        nc.vector.tensor_scalar_add(kcT_sb[:, hc, ts(tt, 512)], ps,
                                                    bias_sb["bk_ca"][:, hc:hc + 1])
                    half, tq = hc // 4, hc % 4
                    wvh = ca_state["wvh"]
                    for tci in (2 * tq, 2 * tq + 1):
                        ps = psum_ckv.tile([P, 512], f32, tag="ps_ckv")
                        for k2 in range(KC // 2):
                            nc.tensor.matmul(ps, encT_sb[:, 2 * k2:2 * k2 + 2, ts(tci, P)],
                                             wvh[:, 2 * k2:2 * k2 + 2, :],
                                             start=(k2 == 0), stop=(k2 == KC // 2 - 1),
                                             perf_mode=DR)
                        nc.vector.tensor_copy(
                            vc_sb[:, tci, half * 8:(half + 1) * 8, 0:DH],
                            ps.rearrange("p (h d) -> p h d", h=8))

                # compact slot layout over (tc, j>=jmin(tc)); causal skips j<tc//4
                jmin = [(tci // 4 if sa_mode == "causal" else 0) for tci in range(TC_SA)]
                bases = []
                nslot = 0
                for tci in range(TC_SA):
                    bases.append(nslot)
                    nslot += NL - jmin[tci]

                for hc in range(KC):  # head pair (2*hc, 2*hc+1)
                    ca_kv_prefetch(hc)
                    pv = psum_pv.tile([DH + 1, 2, NL, LW], f32, tag="pv")
                    e0 = e_pool.tile([P, nslot, LW], bf, tag="e_sa")
                    e1 = e_pool.tile([P, nslot, LW], bf, tag="e_sa")

                    def sa_scores(g0):
                        jm = jmin[g0]
                        N = (NL - jm) * LW
                        ps0 = psum_s.tile([P, 2, NL * LW], f32, tag="ps_sa")
                        ps1 = psum_s.tile([P, 2, NL * LW], f32, tag="ps_sa")
                        for u in range(2):
                            tci = g0 + u
                            loff = jm * LW
                            nc.tensor.matmul(
                                ps0[:, u, :N], kT_sb[0:DH, hc, ts(tci, P)],
                                qT_sb[0:DH, hc, loff:loff + N], start=True, stop=True)
                            nc.tensor.matmul(
                                ps1[:, u, :N], kT_sb[DH:P, hc, ts(tci, P)],
                                qT_sb[DH:P, hc, loff:loff + N], start=True, stop=True)
                        nsl = 2 * (NL - jm)
                        eo0 = e0[:, bases[g0]:bases[g0] + nsl, :].rearrange(
                            "p (u j) l -> p u j l", u=2)
                        eo1 = e1[:, bases[g0]:bases[g0] + nsl, :].rearrange(
                            "p (u j) l -> p u j l", u=2)
                        nc.scalar.activation(
                            eo0, ps0[:, :, :N].rearrange("p u (j l) -> p u j l", l=LW),
                            AF.Exp, bias=shift128, scale=INV_A2)
                        nc.scalar.activation(
                            eo1, ps1[:, :, :N].rearrange("p u (j l) -> p u j l", l=LW),
                            AF.Exp, bias=shift128, scale=INV_A2)

                    def sa_mask(j):
                        w = NL - j  # slot stride across the 4 diagonal chunks
                        for e_sb in (e0, e1):
                            view = e_sb[:, bases[4 * j]:bases[4 * j] + MREG * w, :]
                            view = view.rearrange("p (t w) l -> p t w l", w=w)[:, :, 0, :]
                            nc.vector.tensor_tensor(view, view,
                                                    expm_sb[:, j, :, :], OP.mult)

                    def sa_pv(tlo, thi):
                        for tci in range(tlo, thi):
                            jm = jmin[tci]
                            nc.tensor.matmul(
                                pv[:, 0, jm:, :], v_sb[:, tci, 2 * hc, :],
                                e0[:, bases[tci]:bases[tci] + NL - jm, :],
                                start=(tci == 0), stop=(tci == TC_SA - 1),
                                skip_group_check=True)
                            nc.tensor.matmul(
                                pv[:, 1, jm:, :], v_sb[:, tci, 2 * hc + 1, :],
                                e1[:, bases[tci]:bases[tci] + NL - jm, :],
                                start=(tci == 0), stop=(tci == TC_SA - 1),
                                skip_group_check=True)

                    if sa_mode == "causal":
                        # Interleave the PV chains one block-group behind the
                        # scores/exp so TensorE rides through the exp latency.
                        for j in range(NL):
                            sa_scores(4 * j)
                            if j > 0:
                                sa_pv(4 * (j - 1), 4 * j)
                            sa_scores(4 * j + 2)
                            sa_mask(j)
                        sa_pv(4 * (NL - 1), TC_SA)
                    else:
                        for g0 in range(0, TC_SA, 2):
                            sa_scores(g0)
                        if sa_mode == "generic":
                            nc.vector.tensor_tensor(e0, e0, expm_sb, OP.mult)
                            nc.vector.tensor_tensor(e1, e1, expm_sb, OP.mult)
                        sa_pv(0, TC_SA)
                    # reciprocal of the denominators runs on DVE while the
                    # interleaved CA kv-projection chunk keeps TensorE busy;
                    # the broadcast matmuls then find it ready.
                    recs = [den_recip(pv[DH:DH + 1, u], at_pool, [NL, LW])
                            for u in range(2)]
                    ca_kv_chunk(hc)
                    for u in range(2):
                        apply_norm(oT_sb[u * DH:(u + 1) * DH, hc, :],
                                   pv[0:DH, u], recs[u], at_pool, psum_bc,
                                   [NL, LW])

            with ExitStack() as octx:
                wop = octx.enter_context(tc.tile_pool(name="wo_sa_p", bufs=3))
                otp = octx.enter_context(tc.tile_pool(name="otmp_sa", bufs=2))
                psum_op = octx.enter_context(tc.tile_pool(name="psum_osa", bufs=4, space="PSUM"))

                def evo(oc, tt, ps):
                    # h1pre was pre-loaded with the residual (xTown); descale
                    # the fp8 o-proj on ScalarE, then accumulate in place
                    tmp = otp.tile([P, LTOT], f32, tag="otmp")
                    nc.scalar.activation(tmp, ps, AF.Identity,
                                         bias=bias_sb["bo_sa"][:, oc:oc + 1],
                                         scale=INV_OW)
                    nc.vector.tensor_tensor(h1pre[:, oc, :], tmp,
                                            h1pre[:, oc, :], OP.add)
                proj_to(wop, "wtb", f8, oT_sb, wd["wo_sa"], KC, evo, psum_op,
                        dr=True)

            h1_pool = ctx.enter_context(tc.tile_pool(name="h1p", bufs=1, side="right"))
            h1_sb = h1_pool.tile([P, KC, LTOT], f32, tag="h1")
            bfp = ctx.enter_context(tc.tile_pool(name="bfcast", bufs=1, side="right"))
            h1bf = bfp.tile([P, KC, LTOT], f8, tag="bfx")
            layer_norm("ln1", h1pre, gd["g1"], bias_sb["lb1"], h1_sb, bf_out=h1bf,
                       bf_scale=S_X)

        # =================== CA ===================
        with ExitStack() as cctx:
            qcT_sb = cctx.enter_context(tc.tile_pool(name="qc_ca", bufs=1)).tile(
                [P, KC, LTOT], bf, tag="qcT")

            with ExitStack() as xctx:
                wkp = xctx.enter_context(tc.tile_pool(name="wk_ca_p", bufs=3))
                psum_kv = xctx.enter_context(tc.tile_pool(name="psum_cq", bufs=4, space="PSUM"))

                def evqc(oc, tt, ps):
                    nc.vector.tensor_scalar_add(qcT_sb[:, oc, :], ps,
                                                bias_sb["bq_ca"][:, oc:oc + 1])
                proj_to(wkp, "wtb", f8, h1bf, wd["wq_ca"], KC, evqc, psum_kv,
                        dr=True)

            ocT_sb = cctx.enter_context(tc.tile_pool(name="oT_ca", bufs=1)).tile(
                [P, KC, LTOT], f8, tag="ocT")
            h2pre = pre_pool.tile([P, KC, LTOT], f32, tag="pre")

            with ExitStack() as actx:
                e_pool = actx.enter_context(tc.tile_pool(name="e_ca", bufs=4))
                at_pool = actx.enter_context(tc.tile_pool(name="at_ca", bufs=3))
                psum_s = actx.enter_context(tc.tile_pool(name="psum_cs", bufs=2, space="PSUM"))
                psum_pv = actx.enter_context(tc.tile_pool(name="psum_cpv", bufs=3, space="PSUM"))
                psum_bc = actx.enter_context(tc.tile_pool(name="psum_cbc", bufs=1, space="PSUM"))

                def flush_ca(st):
                    hc, pvu0, rec0, pvu1, rec1 = st
                    apply_norm(ocT_sb[0:DH, hc, :], pvu0[0:DH, :], rec0,
                               at_pool, psum_bc, [LTOT])
                    apply_norm(ocT_sb[DH:P, hc, :], pvu1[0:DH, :], rec1,
                               at_pool, psum_bc, [LTOT])

                # normalize for head pair hc-1 is deferred to between the two
                # PV chains of pair hc so its broadcast matmuls never stall
                # the tensor queue waiting on the DVE reciprocal.
                prev = None
                for hc in range(KC):  # head pair (2*hc, 2*hc+1)
                    ec0 = e_pool.tile([P, TC_CA, LTOT], bf, tag="ec")
                    ec1 = e_pool.tile([P, TC_CA, LTOT], bf, tag="ec")

                    def ca_scores(g0):
                        cs0 = psum_s.tile([P, 2, LTOT], f32, tag="cs")
                        cs1 = psum_s.tile([P, 2, LTOT], f32, tag="cs")
                        for u in range(2):
                            tci = g0 + u
                            nc.tensor.matmul(cs0[:, u, :],
                                             kcT_sb[0:DH, hc, ts(tci, P)],
                                             qcT_sb[0:DH, hc, :],
                                             start=True, stop=True)
                            nc.tensor.matmul(cs1[:, u, :],
                                             kcT_sb[DH:P, hc, ts(tci, P)],
                                             qcT_sb[DH:P, hc, :],
                                             start=True, stop=True)
                        nc.scalar.activation(ec0[:, g0:g0 + 2, :], cs0, AF.Exp,
                                             bias=shift128, scale=INV_A2)
                        nc.scalar.activation(ec1[:, g0:g0 + 2, :], cs1, AF.Exp,
                                             bias=shift128, scale=INV_A2)

                    def ca_pv(tlo, thi):
                        for tci in range(tlo, thi):
                            nc.tensor.matmul(pvc0, vc_sb[:, tci, 2 * hc, :],
                                             ec0[:, tci, :],
                                             start=(tci == 0), stop=(tci == TC_CA - 1),
                                             skip_group_check=True)
                            nc.tensor.matmul(pvc1, vc_sb[:, tci, 2 * hc + 1, :],
                                             ec1[:, tci, :],
                                             start=(tci == 0), stop=(tci == TC_CA - 1),
                                             skip_group_check=True)

                    if ca_mode == "generic":
                        # masks force a full-e barrier; keep the simple order
                        for g0 in range(0, TC_CA, 2):
                            ca_scores(g0)
                        nc.vector.tensor_tensor(ec0, ec0, expmc_sb, OP.mult)
                        nc.vector.tensor_tensor(ec1, ec1, expmc_sb, OP.mult)
                        pvc0 = psum_pv.tile([DH + 1, LTOT], f32, tag="pvc")
                        pvc1 = psum_pv.tile([DH + 1, LTOT], f32, tag="pvc")
                        if prev is not None:
                            flush_ca(prev)
                        ca_pv(0, TC_CA)
                    else:
                        # pipeline: pv lags scores by one pair-group; the
                        # deferred normalize of hc-1 fills the first slot.
                        ca_scores(0)
                        pvc0 = psum_pv.tile([DH + 1, LTOT], f32, tag="pvc")
                        pvc1 = psum_pv.tile([DH + 1, LTOT], f32, tag="pvc")
                        if prev is not None:
                            flush_ca(prev)
                        for g0 in range(2, TC_CA, 2):
                            ca_scores(g0)
                            ca_pv(g0 - 2, g0)
                        ca_pv(TC_CA - 2, TC_CA)
                    rec0 = den_recip(pvc0[DH:DH + 1, :], at_pool, [LTOT])
                    rec1 = den_recip(pvc1[DH:DH + 1, :], at_pool, [LTOT])
                    prev = (hc, pvc0, rec0, pvc1, rec1)
                flush_ca(prev)

            with ExitStack() as octx:
                wop = octx.enter_context(tc.tile_pool(name="wo_ca_p", bufs=3))
                otp = octx.enter_context(tc.tile_pool(name="otmp_ca", bufs=2))
                psum_op = octx.enter_context(tc.tile_pool(name="psum_oca", bufs=4, space="PSUM"))

                def evoc(oc, tt, ps):
                    tmp = otp.tile([P, LTOT], f32, tag="otmp")
                    nc.scalar.activation(tmp, ps, AF.Identity,
                                         bias=bias_sb["bo_ca"][:, oc:oc + 1],
                                         scale=INV_OW)
                    nc.vector.tensor_tensor(h2pre[:, oc, :], tmp,
                                            h1_sb[:, oc, :], OP.add)
                proj_to(wop, "wtb", f8, ocT_sb, wd["wo_ca"], KC, evoc, psum_op,
                        dr=True)

            h2_pool = ctx.enter_context(tc.tile_pool(name="h2p", bufs=1, side="right"))
            h2_sb = h2_pool.tile([P, KC, LTOT], f32, tag="h2")
            h2bf = bfp.tile([P, KC, LTOT], bf, tag="bfx")
            layer_norm("ln2", h2pre, gd["g2"], bias_sb["lb2"], h2_sb, bf_out=h2bf)

        # =================== FFN ===================
        with ExitStack() as fctx:
            ffn_pool = fctx.enter_context(tc.tile_pool(name="ffn", bufs=1))
            w2pool = fctx.enter_context(tc.tile_pool(name="wtile32", bufs=2))
            w1pool = fctx.enter_context(tc.tile_pool(name="w1p", bufs=3))
            psum_f = fctx.enter_context(tc.tile_pool(name="psum_f", bufs=4, space="PSUM"))
            f1_sb = ffn_pool.tile([P, KC2, LTOT], bf, tag="f1")
            h3pre = pre_pool.tile([P, KC, LTOT], f32, tag="pre")

            def evg(oc, tt, ps):
                nc.scalar.activation(f1_sb[:, oc, :], ps, AF.Gelu,
                                     bias=b1_sb[:, oc:oc + 1])
            proj_to(w1pool, "wtb", bf, h2bf, w1_d, KC2, evg, psum_f)

            for oc in range(KC):
                w2t = w2pool.tile([P, KC2, P], bf, tag="w2t")
                nc.sync.dma_start(w2t, w2_d[:, :, ts(oc, P)])
                ps = psum_f.tile([P, LTOT], f32, tag="psproj")
                for kc in range(KC2):
                    nc.tensor.matmul(ps, w2t[:, kc, :], f1_sb[:, kc, :],
                                     start=(kc == 0), stop=(kc == KC2 - 1))
                nc.vector.scalar_tensor_tensor(
                    h3pre[:, oc, :], ps, bias_sb["b2"][:, oc:oc + 1],
                    h2_sb[:, oc, :], OP.add, OP.add)

        out_sb = h1_pool.tile([P, KC, LTOT], f32, tag="h1")
        layer_norm("ln3", h3pre, gd["g3"], bias_sb["lb3"], out_sb,
                   chunk_done=lambda kc: nc.sync.dma_start(out_d[:, kc, :],
                                                           out_sb[:, kc, :]))


# ---------------------------------------------------------------------------
# Host-side packing
# ---------------------------------------------------------------------------

def _pack_wT(w, dtype=np.float32):
    # w: [dout, din] -> [P, din//P, dout] with wT[d, o] layout
    din = w.shape[1]
    return np.ascontiguousarray(
        w.T.reshape(din // P, P, w.shape[0]).transpose(1, 0, 2)).astype(dtype)


def _pack_xT(x, dtype=np.float32):
    # x: [T, D] -> [P, KC, T]
    t = x.shape[0]
    return np.ascontiguousarray(x.T.reshape(KC, P, t).transpose(1, 0, 2)).astype(dtype)


def _pack_bias(v):
    n = v.shape[0] // P
    return np.ascontiguousarray(v.reshape(n, P).T).astype(np.float32)


def detect_sa_mode(mask):
    if not np.isfinite(np.nan_to_num(mask, nan=np.inf)).all():
        return "generic"
    if (mask == 0).all():
        return "zeros"
    li, ti = np.tril_indices(L)
    if (mask[li, ti] == 0).all():
        ui, uj = np.triu_indices(L, k=1)
        if (mask[ui, uj] <= -1e8).all():
            return "causal"
    return "generic"


def make_in_maps(inputs):
    inputs = {k: np.asarray(v, dtype=np.float32) for k, v in inputs.items()}
    mask = inputs["attention_mask"]
    cmask = inputs["encoder_attention_mask"]
    sa_mode = detect_sa_mode(mask)
    ca_mode = "zeros" if (cmask == 0).all() else "generic"
    s = DH ** -0.5

    def fp8q(arr):
        return np.clip(arr, -240.0, 240.0).astype(FP8)

    A = S_X * S_W
    shared = {
        "wq_sa": fp8q(_pack_wT(inputs["sa_wq"] * (s * S_W))),
        "wk_sa": fp8q(_pack_wT(inputs["sa_wk"] * S_W)),
        "wv_sa": fp8q(_pack_wT(inputs["sa_wv"] * S_W)),
        "wo_sa": fp8q(_pack_wT(inputs["sa_wo"] * S_W)),
        "wq_ca": fp8q(_pack_wT(inputs["ca_wq"] * (s * S_W))),
        "wk_ca": fp8q(_pack_wT(inputs["ca_wk"] * S_W)),
        "wv_ca": fp8q(_pack_wT(inputs["ca_wv"] * S_W)),
        "wo_ca": fp8q(_pack_wT(inputs["ca_wo"] * S_W)),
        "w1": _pack_wT(inputs["ffn_w1"], BF16),
        "w2": _pack_wT(inputs["ffn_w2"], BF16),
        "biases": np.concatenate([
            _pack_bias(inputs["sa_bq"] * (s * A)),
            _pack_bias(inputs["sa_bk"] * A),
            _pack_bias(inputs["sa_bo"] + inputs["sa_wo"] @ inputs["sa_bv"]),
            _pack_bias(inputs["ca_bq"] * (s * A)),
            _pack_bias(inputs["ca_bk"] * A),
            _pack_bias(inputs["ca_bo"] + inputs["ca_wo"] @ inputs["ca_bv"]),
            _pack_bias(inputs["ffn_b2"]),
            _pack_bias(inputs["sa_ln_b"]),
            _pack_bias(inputs["ca_ln_b"]),
            _pack_bias(inputs["ffn_ln_b"]),
            _pack_bias(inputs["ffn_b1"]),
        ], axis=1),
        "g1": np.ascontiguousarray(inputs["sa_ln_g"].reshape(1, D)).astype(BF16),
        "g2": np.ascontiguousarray(inputs["ca_ln_g"].reshape(1, D)).astype(BF16),
        "g3": np.ascontiguousarray(inputs["ffn_ln_g"].reshape(1, D)).astype(BF16),
    }

    exts = EXT_CAUSAL if sa_mode == "causal" else [TC_SA] * NL
    in_maps = []
    for c in range(8):
        b, i = c // 4, c % 4
        blocks = core_blocks(i)
        own_rows = np.concatenate([np.arange(p * LW, (p + 1) * LW) for p in blocks])
        xTp32 = _pack_xT(inputs["hidden_states"][b])
        m = dict(shared)
        m["xT"] = fp8q(xTp32 * S_X)
        m["xTown"] = np.ascontiguousarray(xTp32[:, :, own_rows])
        m["xTownb"] = fp8q(m["xTown"] * S_X)
        m["encT"] = fp8q(_pack_xT(inputs["encoder_hidden_states"][b]) * S_X)
        if sa_mode == "causal":
            em = np.empty((P, NL, MREG, LW), dtype=BF16)
            for j, pblk in enumerate(blocks):
                rows = slice(pblk * LW, (pblk + 1) * LW)
                t0 = (exts[j] - MREG) * P
                blk = np.exp(np.minimum(mask[rows, t0:t0 + MREG * P], 60.0))
                em[:, j] = blk.reshape(LW, MREG, P).transpose(2, 1, 0)
            m["expm"] = em
        elif sa_mode == "generic":
            em = np.empty((P, TC_SA * NL, LW), dtype=BF16)
            for j, pblk in enumerate(blocks):
                rows = slice(pblk * LW, (pblk + 1) * LW)
                blk = np.exp(np.minimum(mask[rows, :], 60.0))
                em[:, j::NL, :] = blk.reshape(LW, TC_SA, P).transpose(2, 1, 0)
            m["expm"] = em
        if ca_mode == "generic":
            em = np.empty((P, TC_CA, LTOT), dtype=BF16)
            for j, pblk in enumerate(blocks):
                rows = slice(pblk * LW, (pblk + 1) * LW)
                blk = np.exp(np.minimum(cmask[rows, :], 60.0))
                em[:, :, j * LW:(j + 1) * LW] = blk.reshape(LW, TC_CA, P).transpose(2, 1, 0)
            m["expmc"] = em
        in_maps.append(m)
    return in_maps, sa_mode, ca_mode


def assemble_output(results):
    out = np.zeros((B, L, D), np.float32)
    for c in range(8):
        b, i = c // 4, c % 4
        arr = np.asarray(results[c]["out"])  # [P, KC, LTOT]
        for j, pblk in enumerate(core_blocks(i)):
            blk = arr[:, :, j * LW:(j + 1) * LW]          # [P, KC, LW]
            out[b, pblk * LW:(pblk + 1) * LW, :] = blk.transpose(2, 1, 0).reshape(LW, D)
    return out


# ---------------------------------------------------------------------------
# Entry point
# ---------------------------------------------------------------------------

_NC_CACHE = {}


def get_nc(sa_mode, ca_mode):
    key = (sa_mode, ca_mode)
    if key not in _NC_CACHE:
        _NC_CACHE[key] = build_nc(sa_mode, ca_mode)
    return _NC_CACHE[key]


def _install_ntff_hook():
    """bass_utils' trace path needs antenv.axon_hooks, absent in this image.
    Inject a shim and register the ctypes-based NTFF hook from trn_agent_boot."""
    import types
    if "antenv.axon_hooks" in sys.modules:
        return
    holder = {}
    mod = types.ModuleType("antenv.axon_hooks")
    mod.set_axon_ntff_profile_hook = lambda h: holder.__setitem__("h", h)
    mod.get_axon_ntff_profile_hook = lambda: holder.get("h")
    sys.modules["antenv.axon_hooks"] = mod
    try:
        import antenv
        antenv.axon_hooks = mod
    except ImportError:
        pass
    try:
        from trn_agent_boot.trn_boot import _ntff_profile_via_ctypes
        hook = _ntff_profile_via_ctypes("/opt/axon/libaxon_pjrt.so")
        if hook is not None:
            mod.set_axon_ntff_profile_hook(hook)
    except Exception as e:  # degrade to no tracing
        print(f"ntff hook install failed: {e}", file=sys.stderr)


def run(inputs, trace=False):
    _install_ntff_hook()
    from concourse.bass_utils import run_bass_kernel_spmd
    in_maps, sa_mode, ca_mode = make_in_maps(inputs)
    nc = get_nc(sa_mode, ca_mode)
    res = run_bass_kernel_spmd(nc, in_maps, core_ids=list(range(8)), trace=trace)
    return assemble_output(res.results), res


def kernel(**inputs):
    out, _ = run(inputs, trace=False)
    return out



# revision 47
# speedup vs baseline: 1.0189x; 1.0189x over previous
"""Trainium2 Bass kernel for nn_ConicaLayer (transformer decoder layer:
self-attn (causal) + cross-attn + FFN, post-LN residuals).

Sharding: rows (B x L) split across 8 cores; core c -> batch b=c//4, and 4
interleaved 128-row blocks {i, 7-i, 8+i, 15-i} of the 16 blocks of that batch
(balances causal attention work). Each core computes full K/V for its batch.

All transposes/packing are done host-side; on-device activations stay d-major
([D, tokens]) end-to-end. Softmax uses exp(s-20) without max-subtraction
(scores are bounded; masked entries handled by multiplying exp(mask), with
fully-masked blocks skipped in causal mode). The V matrix carries an appended
ones column so PV matmuls also produce softmax denominators. V-bias folds into
the out-proj bias host-side (bo_eff = bo + wo @ bv, exact since sum(p) = 1).

Dtypes: bf16 for kv-path (xT/encT, wk/wv, k/v/q/e), attention out + wo, f1 +
w2. f32 for residual stream, wq/w1, layernorms, all PSUM accumulation.
"""

import sys
import numpy as np

try:
    import concourse.bass as bass  # noqa: F401
except ImportError:
    sys.path.insert(0, "/opt/trn_rl_repo")

import ml_dtypes
import concourse.bass as bass
import concourse.bacc as bacc
import concourse.tile as tile
from concourse import mybir
from concourse.bass import ts

BF16 = ml_dtypes.bfloat16

P = 128
B, L, S, D, H, DFF = 2, 2048, 1024, 1024, 16, 4096
DH = D // H           # 64
KC = D // P           # 8
KC2 = DFF // P        # 32
NL = 4                # l-blocks per core
LW = 128              # l width per block
LTOT = NL * LW        # 512 rows per core
TC_SA = L // P        # 16 t-chunks (self attn)
TC_CA = S // P        # 8 t-chunks (cross attn)
MREG = 4              # masked tail chunks per block (causal mode)
EXT_CAUSAL = [4, 8, 12, 16]
EXP_SHIFT = -20.0
EPS = 1e-5

f32 = mybir.dt.float32
bf = mybir.dt.bfloat16
f8 = mybir.dt.float8e4
FP8 = mybir.dt.np(f8)
DR = mybir.MatmulPerfMode.DoubleRow
AF = mybir.ActivationFunctionType
OP = mybir.AluOpType

# fp8 scaling: activations x16, weights x512. q/k carry the combined factor
# A into SBUF (descaled for free in the exp's scale arg); v carries A into
# the PV output (descaled by dividing wo by A host-side).
S_X = 16.0
S_W = 512.0
A_SC = S_X * S_W
INV_A2 = 1.0 / (A_SC * A_SC)
INV_A = 1.0 / A_SC
INV_W = 1.0 / S_W
S_O = 32.0
INV_OW = 1.0 / (S_O * S_W)


def core_blocks(i):
    return [i, 7 - i, 8 + i, 15 - i]


# ---------------------------------------------------------------------------
# Bass kernel builder
# ---------------------------------------------------------------------------

def build_nc(sa_mode, ca_mode):
    """sa_mode: 'causal' | 'zeros' | 'generic'; ca_mode: 'zeros' | 'generic'."""
    nc = bacc.Bacc("TRN2", target_bir_lowering=False, debug=False, num_devices=8)

    def din(name, shape, dtype=f32):
        return nc.dram_tensor(name, list(shape), dtype, kind="ExternalInput").ap()

    env = {}
    env["sa_mode"], env["ca_mode"] = sa_mode, ca_mode
    env["exts"] = EXT_CAUSAL if sa_mode == "causal" else [TC_SA] * NL
    env["xT_d"] = din("xT", [P, KC, L], f8)
    env["xTown_d"] = din("xTown", [P, KC, LTOT])
    env["xTownb_d"] = din("xTownb", [P, KC, LTOT], f8)
    env["encT_d"] = din("encT", [P, KC, S], f8)
    wdt = {"wq_sa": f8, "wk_sa": f8, "wv_sa": f8, "wo_sa": f8,
           "wq_ca": f8, "wk_ca": f8, "wv_ca": f8, "wo_ca": f8}
    env["wd"] = {n: din(n, [P, KC, D], dt) for n, dt in wdt.items()}
    env["w1_d"] = din("w1", [P, KC, DFF], bf)
    env["w2_d"] = din("w2", [P, KC2, D], bf)
    env["biases_d"] = din("biases", [P, 10 * KC + KC2])
    env["gd"] = {n: din(n, [1, D], bf) for n in ["g1", "g2", "g3"]}
    env["expm_d"] = None
    if sa_mode == "causal":
        env["expm_d"] = din("expm", [P, NL, MREG, LW], bf)
    elif sa_mode == "generic":
        env["expm_d"] = din("expm", [P, TC_SA * NL, LW], bf)
    env["expmc_d"] = din("expmc", [P, TC_CA, LTOT], bf) if ca_mode == "generic" else None
    env["out_d"] = nc.dram_tensor("out", [P, KC, LTOT], f32, kind="ExternalOutput").ap()

    with tile.TileContext(nc) as tc:
        _build_body(nc, tc, env)
    nc.compile()
    return nc


def _build_body(nc, tc, env):
    from contextlib import ExitStack

    xT_d, xTown_d, encT_d = env["xT_d"], env["xTown_d"], env["encT_d"]
    xTownb_d = env["xTownb_d"]
    wd, w1_d, w2_d, biases_d, gd = env["wd"], env["w1_d"], env["w2_d"], env["biases_d"], env["gd"]
    expm_d, expmc_d, out_d = env["expm_d"], env["expmc_d"], env["out_d"]
    sa_mode, ca_mode, exts = env["sa_mode"], env["ca_mode"], env["exts"]

    with ExitStack() as ctx:
        consts = ctx.enter_context(tc.tile_pool(name="consts", bufs=1))

        # ---- constants (tiles allocated now; DMAs deferred via load_consts
        # so the startup DMA queue serves the k-proj inputs first) ----
        biases_sb = consts.tile([P, 10 * KC + KC2], f32, tag="c_bias")
        bias_names = ["bq_sa", "bk_sa", "bo_sa", "bq_ca", "bk_ca", "bo_ca",
                      "b2", "lb1", "lb2", "lb3"]
        bias_sb = {n: biases_sb[:, i * KC:(i + 1) * KC]
                   for i, n in enumerate(bias_names)}
        b1_sb = biases_sb[:, 10 * KC:10 * KC + KC2]
        ones128b = consts.tile([P, 1], bf, tag="ones128b")
        nc.vector.memset(ones128b, 1.0)
        ones64b = consts.tile([1, DH], bf, tag="ones64b")
        nc.vector.memset(ones64b, 1.0)
        eps_sb = consts.tile([1, 1], f32, tag="eps")
        nc.vector.memset(eps_sb, EPS)
        zero128 = consts.tile([P, 1], f32, tag="zero128")
        nc.vector.memset(zero128, 0.0)
        shift128 = consts.tile([P, 1], f32, tag="shift128")
        nc.vector.memset(shift128, EXP_SHIFT)
        expm_sb = None
        if expm_d is not None:
            shape = [P, NL, MREG, LW] if sa_mode == "causal" else [P, TC_SA * NL, LW]
            expm_sb = consts.tile(shape, bf, tag="expm")
        expmc_sb = None
        if expmc_d is not None:
            expmc_sb = consts.tile([P, TC_CA, LTOT], bf, tag="expmc")

        def load_consts():
            nc.sync.dma_start(biases_sb, biases_d)
            if expm_sb is not None:
                nc.sync.dma_start(expm_sb, expm_d)
            if expmc_sb is not None:
                nc.sync.dma_start(expmc_sb, expmc_d)

        # ------------- helpers -------------
        def proj_to(wpool, wtag, wdt, src_sb, w_dram, n_oc, evict, psum_proj, n_tt=1,
                    tt_width=LTOT, n_kc=KC, dr=False, wt0=None):
            for oc in range(n_oc):
                if oc == 0 and wt0 is not None:
                    wt = wt0
                else:
                    wt = wpool.tile([P, n_kc, P], wdt, tag=wtag)
                    nc.sync.dma_start(wt, w_dram[:, :, ts(oc, P)])
                for tt in range(n_tt):
                    ps = psum_proj.tile([P, tt_width], f32, tag="psproj")
                    if dr:
                        for k2 in range(n_kc // 2):
                            nc.tensor.matmul(
                                ps, wt[:, 2 * k2:2 * k2 + 2, :],
                                src_sb[:, 2 * k2:2 * k2 + 2, ts(tt, tt_width)],
                                start=(k2 == 0), stop=(k2 == n_kc // 2 - 1),
                                perf_mode=DR)
                    else:
                        for kc in range(n_kc):
                            nc.tensor.matmul(
                                ps, wt[:, kc, :], src_sb[:, kc, ts(tt, tt_width)],
                                start=(kc == 0), stop=(kc == n_kc - 1))
                    evict(oc, tt, ps)

        def layer_norm(tag, x_sb, g_dram, lb, out_sb, chunk_done=None, bf_out=None,
                       bf_scale=None):
            # Stats and per-token scale broadcasts run as bf16 matmuls (4x the
            # fp32 matmul rate); the residual/output path stays fp32.
            with ExitStack() as lctx:
                lnp = lctx.enter_context(tc.tile_pool(
                    name=f"lnp_{tag}", bufs=1 if sa_mode == "generic" else 2))
                lns = lctx.enter_context(tc.tile_pool(name=f"lns_{tag}", bufs=1))
                psum_st = lctx.enter_context(
                    tc.tile_pool(name=f"psum_lns_{tag}", bufs=1, space="PSUM"))
                psum_ln = lctx.enter_context(
                    tc.tile_pool(name=f"psum_ln_{tag}", bufs=3, space="PSUM"))
                g = lns.tile([1, D], bf, tag="ln_g")
                nc.sync.dma_start(g, g_dram)
                psx = psum_st.tile([1, LTOT], f32, tag="ln_sx")
                psx2 = psum_st.tile([1, LTOT], f32, tag="ln_sx2")
                for kc in range(KC):
                    xb = lnp.tile([P, LTOT], bf, tag="ln_xb")
                    nc.vector.tensor_copy(xb, x_sb[:, kc, :])
                    sq = lnp.tile([P, LTOT], bf, tag="ln_sq")
                    nc.scalar.activation(sq, x_sb[:, kc, :], AF.Square, bias=zero128)
                    nc.tensor.matmul(psx, ones128b, xb,
                                     start=(kc == 0), stop=(kc == KC - 1))
                    nc.tensor.matmul(psx2, ones128b, sq,
                                     start=(kc == 0), stop=(kc == KC - 1))
                mean = lns.tile([1, LTOT], f32, tag="ln_mean")
                nc.vector.tensor_scalar_mul(mean, psx, 1.0 / D)
                msq = lns.tile([1, LTOT], f32, tag="ln_msq")
                nc.vector.tensor_tensor(msq, mean, mean, OP.mult)
                var = lns.tile([1, LTOT], f32, tag="ln_var")
                nc.vector.scalar_tensor_tensor(var, psx2, 1.0 / D, msq, OP.mult, OP.subtract)
                std = lns.tile([1, LTOT], f32, tag="ln_std")
                nc.scalar.activation(std, var, AF.Sqrt, bias=eps_sb)
                rstd = lns.tile([1, LTOT], f32, tag="ln_rstd")
                nc.vector.reciprocal_approx_fast(rstd, std)
                rstd_b = lns.tile([1, LTOT], bf, tag="ln_rstdb")
                nc.vector.tensor_copy(rstd_b, rstd)
                mrstd_b = lns.tile([1, LTOT], bf, tag="ln_mrstdb")
                nc.vector.tensor_tensor(mrstd_b, mean, rstd, OP.mult)
                for kc in range(KC):
                    pg = psum_ln.tile([P, LTOT], f32, tag="ln_pg")
                    pm = psum_ln.tile([P, LTOT], f32, tag="ln_pm")
                    nc.tensor.matmul(pg, g[:, ts(kc, P)], rstd_b, start=True, stop=True)
                    nc.tensor.matmul(pm, g[:, ts(kc, P)], mrstd_b, start=True, stop=True)
                    t = lnp.tile([P, LTOT], f32, tag="ln_t")
                    nc.vector.tensor_tensor(t, x_sb[:, kc, :], pg, OP.mult)
                    nc.vector.scalar_tensor_tensor(out_sb[:, kc, :], t, lb[:, kc:kc + 1],
                                                   pm, OP.add, OP.subtract)
                    if bf_out is not None:
                        if bf_scale is not None:
                            nc.vector.tensor_scalar_mul(bf_out[:, kc, :],
                                                        out_sb[:, kc, :], bf_scale)
                        else:
                            nc.vector.tensor_copy(bf_out[:, kc, :], out_sb[:, kc, :])
                    if chunk_done is not None:
                        chunk_done(kc)

        def kv_proj_v(vpool_dst, src_sb, wv_dram, wrhs, wrhs_tag, n_tc, psum_proj):
            for half in range(2):
                wvh = wrhs.tile([P, KC, 512], f8, tag=wrhs_tag)
                nc.sync.dma_start(wvh, wv_dram[:, :, ts(half, 512)])
                for tci in range(n_tc):
                    ps = psum_proj.tile([P, 512], f32, tag="psproj")
                    for k2 in range(KC // 2):
                        nc.tensor.matmul(
                            ps, src_sb[:, 2 * k2:2 * k2 + 2, ts(tci, P)],
                            wvh[:, 2 * k2:2 * k2 + 2, :],
                            start=(k2 == 0), stop=(k2 == KC // 2 - 1),
                            perf_mode=DR)
                    nc.vector.tensor_copy(
                        vpool_dst[:, tci, half * 8:(half + 1) * 8, 0:DH],
                        ps.rearrange("p (h d) -> p h d", h=8))

        def den_recip(den_row, at_pool, inner):
            # den_row: [1, *inner] PSUM slice holding softmax denominators.
            # The PSUM->SBUF copy runs on ScalarE (Copy is in every table) so
            # only the reciprocal+cast sit on the vector queue.
            sums = at_pool.tile([1] + inner, f32, tag="at_sums")
            nc.vector.tensor_copy(sums, den_row)
            recip = at_pool.tile([1] + inner, f32, tag="at_recip")
            nc.vector.reciprocal_approx_fast(recip, sums)
            recip_b = at_pool.tile([1] + inner, bf, tag="at_recipb")
            nc.vector.tensor_scalar_mul(recip_b, recip, S_O / A_SC)
            return recip_b

        def apply_norm(dst, pv_rows, recip_b, at_pool, psum_bc, inner):
            # dst = pv_rows * broadcast(recip)
            pb = psum_bc.tile([DH] + inner, f32, tag="bc")
            nc.tensor.matmul(pb, ones64b, recip_b, start=True, stop=True)
            bc_sb = at_pool.tile([DH] + inner, f32, tag="at_bc")
            nc.vector.tensor_copy(bc_sb, pb)
            if len(inner) == 2:
                dst = dst.rearrange("p (j l) -> p j l", j=inner[0])
            nc.vector.scalar_tensor_tensor(dst, pv_rows, 0.0, bc_sb,
                                           OP.bypass, OP.mult)

        # =================== SA ===================
        with ExitStack() as sctx:
            sa_pool = sctx.enter_context(tc.tile_pool(name="sa", bufs=1))
            kT_sb = sa_pool.tile([P, KC, L], bf, tag="kT")
            v_sb = sa_pool.tile([P, TC_SA, H, DH + 1], bf, tag="v")
            qT_sb = sa_pool.tile([P, KC, LTOT], bf, tag="qT")
            nc.gpsimd.memset(v_sb[:, :, :, DH:DH + 1], 1.0)

            with ExitStack() as xctx:
                xpool = xctx.enter_context(tc.tile_pool(name="xpool", bufs=1))
                wrhs = xctx.enter_context(tc.tile_pool(name="wrhs", bufs=2))
                wkp = xctx.enter_context(tc.tile_pool(name="wk_sa_p", bufs=3))
                psum_kv = xctx.enter_context(tc.tile_pool(name="psum_kv", bufs=4, space="PSUM"))
                # first weight tile and the xT chunks lead the DMA queue so
                # the k-proj can start ~4us in; const loads queue behind them.
                wt0k = wkp.tile([P, KC, P], f8, tag="wtb")
                nc.sync.dma_start(wt0k, wd["wk_sa"][:, :, ts(0, P)])
                xT_sb = xpool.tile([P, KC, L], f8, tag="xT")
                for kc in range(KC):
                    nc.sync.dma_start(xT_sb[:, kc, :], xT_d[:, kc, :])
                qsrc = xpool.tile([P, KC, LTOT], f8, tag="qsrc")
                nc.sync.dma_start(qsrc, xTownb_d)
                load_consts()

                def evk(oc, tt, ps):
                    nc.vector.tensor_scalar_add(kT_sb[:, oc, ts(tt, 512)], ps,
                                                bias_sb["bk_sa"][:, oc:oc + 1])
                proj_to(wkp, "wtb", f8, xT_sb, wd["wk_sa"], KC, evk, psum_kv,
                        n_tt=L // 512, tt_width=512, dr=True, wt0=wt0k)

                def evq(oc, tt, ps):
                    nc.vector.tensor_scalar_add(qT_sb[:, oc, :], ps,
                                                bias_sb["bq_sa"][:, oc:oc + 1])
                proj_to(wkp, "wtb", f8, qsrc, wd["wq_sa"], KC, evq, psum_kv, dr=True)

                kv_proj_v(v_sb, xT_sb, wd["wv_sa"], wrhs, "wrhs", TC_SA, psum_kv)

            ca_pool = ctx.enter_context(tc.tile_pool(name="ca", bufs=1, side="right"))
            kcT_sb = ca_pool.tile([P, KC, S], bf, tag="kcT")
            vc_sb = ca_pool.tile([P, TC_CA, H, DH + 1], bf, tag="vc")
            encT_sb = ca_pool.tile([P, KC, S], f8, tag="encT")
            for kc in range(KC):
                nc.sync.dma_start(encT_sb[:, kc, :], encT_d[:, kc, :])
            nc.gpsimd.memset(vc_sb[:, :, :, DH:DH + 1], 1.0)
            oT_sb = sctx.enter_context(tc.tile_pool(name="oT_sa", bufs=1)).tile(
                [P, KC, LTOT], f8, tag="oT")
            pre_pool = ctx.enter_context(tc.tile_pool(name="prep", bufs=1, side="right"))
            h1pre = pre_pool.tile([P, KC, LTOT], f32, tag="pre")
            nc.sync.dma_start(h1pre, xTown_d)

            with ExitStack() as actx:
                e_pool = actx.enter_context(tc.tile_pool(name="e_sa", bufs=2))
                at_pool = actx.enter_context(tc.tile_pool(name="at_sa", bufs=3))
                psum_s = actx.enter_context(tc.tile_pool(name="psum_s", bufs=3, space="PSUM"))
                psum_pv = actx.enter_context(tc.tile_pool(name="psum_pv", bufs=2, space="PSUM"))
                psum_bc = actx.enter_context(tc.tile_pool(name="psum_bc", bufs=1, space="PSUM"))

                def flush_sa(st):
                    hc_p, pv_p, recs_p = st
                    for u in range(2):
                        apply_norm(oT_sb[u * DH:(u + 1) * DH, hc_p, :],
                                   pv_p[0:DH, u], recs_p[u], at_pool, psum_bc,
                                   [NL, LW])

                # compact slot layout over (tc, j>=jmin(tc)); causal skips j<tc//4
                jmin = [(tci // 4 if sa_mode == "causal" else 0) for tci in range(TC_SA)]
                bases = []
                nslot = 0
                for tci in range(TC_SA):
                    bases.append(nslot)
                    nslot += NL - jmin[tci]

                prev_sa = None
                for hc in range(KC):  # head pair (2*hc, 2*hc+1)
                    pv = psum_pv.tile([DH + 1, 2, NL, LW], f32, tag="pv")
                    e0 = e_pool.tile([P, nslot, LW], bf, tag="e_sa")
                    e1 = e_pool.tile([P, nslot, LW], bf, tag="e_sa")

                    def sa_scores(g0):
                        # one 1-bank PSUM tile per (tci, head) so three can
                        # rotate inside 3 banks
                        for u in range(2):
                            tci = g0 + u
                            jm = jmin[tci]
                            N = (NL - jm) * LW
                            loff = jm * LW
                            psa = psum_s.tile([P, NL * LW], f32, tag="ps_sa")
                            psb = psum_s.tile([P, NL * LW], f32, tag="ps_sa")
                            nc.tensor.matmul(
                                psa[:, :N], kT_sb[0:DH, hc, ts(tci, P)],
                                qT_sb[0:DH, hc, loff:loff + N], start=True, stop=True)
                            nc.tensor.matmul(
                                psb[:, :N], kT_sb[DH:P, hc, ts(tci, P)],
                                qT_sb[DH:P, hc, loff:loff + N], start=True, stop=True)
                            nsl = NL - jm
                            nc.scalar.activation(
                                e0[:, bases[tci]:bases[tci] + nsl, :],
                                psa[:, :N].rearrange("p (j l) -> p j l", l=LW),
                                AF.Exp, bias=shift128, scale=INV_A2)
                            nc.scalar.activation(
                                e1[:, bases[tci]:bases[tci] + nsl, :],
                                psb[:, :N].rearrange("p (j l) -> p j l", l=LW),
                                AF.Exp, bias=shift128, scale=INV_A2)

                    def sa_mask(j):
                        w = NL - j  # slot stride across the 4 diagonal chunks
                        for e_sb in (e0, e1):
                            view = e_sb[:, bases[4 * j]:bases[4 * j] + MREG * w, :]
                            view = view.rearrange("p (t w) l -> p t w l", w=w)[:, :, 0, :]
                            nc.vector.tensor_tensor(view, view,
                                                    expm_sb[:, j, :, :], OP.mult)

                    def sa_pv(tlo, thi):
                        for tci in range(tlo, thi):
                            jm = jmin[tci]
                            nc.tensor.matmul(
                                pv[:, 0, jm:, :], v_sb[:, tci, 2 * hc, :],
                                e0[:, bases[tci]:bases[tci] + NL - jm, :],
                                start=(tci == 0), stop=(tci == TC_SA - 1),
                                skip_group_check=True)
                            nc.tensor.matmul(
                                pv[:, 1, jm:, :], v_sb[:, tci, 2 * hc + 1, :],
                                e1[:, bases[tci]:bases[tci] + NL - jm, :],
                                start=(tci == 0), stop=(tci == TC_SA - 1),
                                skip_group_check=True)

                    if sa_mode == "causal":
                        # Interleave the PV chains one block-group behind the
                        # scores/exp so TensorE rides through the exp latency;
                        # the deferred normalize of hc-1 fills the first slot.
                        for j in range(NL):
                            sa_scores(4 * j)
                            if j == 0:
                                if prev_sa is not None:
                                    flush_sa(prev_sa)
                            else:
                                sa_pv(4 * (j - 1), 4 * j)
                            sa_scores(4 * j + 2)
                            sa_mask(j)
                        sa_pv(4 * (NL - 1), TC_SA)
                    else:
                        for g0 in range(0, TC_SA, 2):
                            sa_scores(g0)
                        if sa_mode == "generic":
                            nc.vector.tensor_tensor(e0, e0, expm_sb, OP.mult)
                            nc.vector.tensor_tensor(e1, e1, expm_sb, OP.mult)
                        if prev_sa is not None:
                            flush_sa(prev_sa)
                        sa_pv(0, TC_SA)
                    recs = [den_recip(pv[DH:DH + 1, u], at_pool, [NL, LW])
                            for u in range(2)]
                    prev_sa = (hc, pv, recs)
                flush_sa(prev_sa)

            # CA k/v projections as one dense fp8 block (same PE config as the
            # fp8 o-proj that follows -- avoids per-head-pair mode switches).
            with ExitStack() as kctx:
                wkvc = kctx.enter_context(tc.tile_pool(name="wkv_ca", bufs=3))
                wrhsc = kctx.enter_context(tc.tile_pool(name="wrhs_ca", bufs=2))
                psum_ckv = kctx.enter_context(
                    tc.tile_pool(name="psum_ckv", bufs=4, space="PSUM"))
                ca_state = {}

                def ca_kv_chunk(hc):
                    wt = ca_state.pop(f"wt{hc}")
                    for tt in range(S // 512):
                        ps = psum_ckv.tile([P, 512], f32, tag="ps_ckv")
                        for k2 in range(KC // 2):
                            nc.tensor.matmul(ps, wt[:, 2 * k2:2 * k2 + 2, :],
                                             encT_sb[:, 2 * k2:2 * k2 + 2, ts(tt, 512)],
                                             start=(k2 == 0), stop=(k2 == KC // 2 - 1),
                                             perf_mode=DR)
                        nc.vector.tensor_scalar_add(kcT_sb[:, hc, ts(tt, 512)], ps,
                                                    bias_sb["bk_ca"][:, hc:hc + 1])
                    half, tq = hc // 4, hc % 4
                    wvh = ca_state["wvh"]
                    for tci in (2 * tq, 2 * tq + 1):
                        ps = psum_ckv.tile([P, 512], f32, tag="ps_ckv")
                        for k2 in range(KC // 2):
                            nc.tensor.matmul(ps, encT_sb[:, 2 * k2:2 * k2 + 2, ts(tci, P)],
                                             wvh[:, 2 * k2:2 * k2 + 2, :],
                                             start=(k2 == 0), stop=(k2 == KC // 2 - 1),
                                             perf_mode=DR)
                        nc.vector.tensor_copy(
                            vc_sb[:, tci, half * 8:(half + 1) * 8, 0:DH],
                            ps.rearrange("p (h d) -> p h d", h=8))

                for hc in range(KC):
                    if hc % 4 == 0:
                        wvh_new = wrhsc.tile([P, KC, 512], f8, tag="wv_ca")
                        ca_state["wvh"] = wvh_new
                        nc.sync.dma_start(wvh_new,
                                          wd["wv_ca"][:, :, ts(hc // 4, 512)])
                    wt = wkvc.tile([P, KC, P], f8, tag="wt_ck")
                    nc.sync.dma_start(wt, wd["wk_ca"][:, :, ts(hc, P)])
                    ca_state[f"wt{hc}"] = wt
                    ca_kv_chunk(hc)

            with ExitStack() as octx:
                wop = octx.enter_context(tc.tile_pool(name="wo_sa_p", bufs=3))
                otp = octx.enter_context(tc.tile_pool(name="otmp_sa", bufs=2))
                psum_op = octx.enter_context(tc.tile_pool(name="psum_osa", bufs=4, space="PSUM"))

                def evo(oc, tt, ps):
                    # h1pre was pre-loaded with the residual (xTown); descale
                    # the fp8 o-proj on ScalarE, then accumulate in place
                    tmp = otp.tile([P, LTOT], f32, tag="otmp")
                    nc.scalar.activation(tmp, ps, AF.Identity,
                                         bias=bias_sb["bo_sa"][:, oc:oc + 1],
                                         scale=INV_OW)
                    nc.vector.tensor_tensor(h1pre[:, oc, :], tmp,
                                            h1pre[:, oc, :], OP.add)
                proj_to(wop, "wtb", f8, oT_sb, wd["wo_sa"], KC, evo, psum_op,
                        dr=True)

            h1_pool = ctx.enter_context(tc.tile_pool(name="h1p", bufs=1, side="right"))
            h1_sb = h1_pool.tile([P, KC, LTOT], f32, tag="h1")
            bfp = ctx.enter_context(tc.tile_pool(name="bfcast", bufs=1, side="right"))
            h1bf = bfp.tile([P, KC, LTOT], f8, tag="bfx")
            layer_norm("ln1", h1pre, gd["g1"], bias_sb["lb1"], h1_sb, bf_out=h1bf,
                       bf_scale=S_X)

        # =================== CA ===================
        with ExitStack() as cctx:
            qcT_sb = cctx.enter_context(tc.tile_pool(name="qc_ca", bufs=1)).tile(
                [P, KC, LTOT], bf, tag="qcT")

            with ExitStack() as xctx:
                wkp = xctx.enter_context(tc.tile_pool(name="wk_ca_p", bufs=3))
                psum_kv = xctx.enter_context(tc.tile_pool(name="psum_cq", bufs=4, space="PSUM"))

                def evqc(oc, tt, ps):
                    nc.vector.tensor_scalar_add(qcT_sb[:, oc, :], ps,
                                                bias_sb["bq_ca"][:, oc:oc + 1])
                proj_to(wkp, "wtb", f8, h1bf, wd["wq_ca"], KC, evqc, psum_kv,
                        dr=True)

            ocT_sb = cctx.enter_context(tc.tile_pool(name="oT_ca", bufs=1)).tile(
                [P, KC, LTOT], f8, tag="ocT")
            h2pre = pre_pool.tile([P, KC, LTOT], f32, tag="pre")

            with ExitStack() as actx:
                e_pool = actx.enter_context(tc.tile_pool(name="e_ca", bufs=4))
                at_pool = actx.enter_context(tc.tile_pool(name="at_ca", bufs=3))
                psum_s = actx.enter_context(tc.tile_pool(name="psum_cs", bufs=2, space="PSUM"))
                psum_pv = actx.enter_context(tc.tile_pool(name="psum_cpv", bufs=3, space="PSUM"))
                psum_bc = actx.enter_context(tc.tile_pool(name="psum_cbc", bufs=1, space="PSUM"))

                def flush_ca(st):
                    hc, pvu0, rec0, pvu1, rec1 = st
                    apply_norm(ocT_sb[0:DH, hc, :], pvu0[0:DH, :], rec0,
                               at_pool, psum_bc, [LTOT])
                    apply_norm(ocT_sb[DH:P, hc, :], pvu1[0:DH, :], rec1,
                               at_pool, psum_bc, [LTOT])

                # normalize for head pair hc-1 is deferred to between the two
                # PV chains of pair hc so its broadcast matmuls never stall
                # the tensor queue waiting on the DVE reciprocal.
                prev = None
                for hc in range(KC):  # head pair (2*hc, 2*hc+1)
                    ec0 = e_pool.tile([P, TC_CA, LTOT], bf, tag="ec")
                    ec1 = e_pool.tile([P, TC_CA, LTOT], bf, tag="ec")

                    def ca_scores(g0):
                        cs0 = psum_s.tile([P, 2, LTOT], f32, tag="cs")
                        cs1 = psum_s.tile([P, 2, LTOT], f32, tag="cs")
                        for u in range(2):
                            tci = g0 + u
                            nc.tensor.matmul(cs0[:, u, :],
                                             kcT_sb[0:DH, hc, ts(tci, P)],
                                             qcT_sb[0:DH, hc, :],
                                             start=True, stop=True)
                            nc.tensor.matmul(cs1[:, u, :],
                                             kcT_sb[DH:P, hc, ts(tci, P)],
                                             qcT_sb[DH:P, hc, :],
                                             start=True, stop=True)
                        nc.scalar.activation(ec0[:, g0:g0 + 2, :], cs0, AF.Exp,
                                             bias=shift128, scale=INV_A2)
                        nc.scalar.activation(ec1[:, g0:g0 + 2, :], cs1, AF.Exp,
                                             bias=shift128, scale=INV_A2)

                    def ca_pv(tlo, thi):
                        for tci in range(tlo, thi):
                            nc.tensor.matmul(pvc0, vc_sb[:, tci, 2 * hc, :],
                                             ec0[:, tci, :],
                                             start=(tci == 0), stop=(tci == TC_CA - 1),
                                             skip_group_check=True)
                            nc.tensor.matmul(pvc1, vc_sb[:, tci, 2 * hc + 1, :],
                                             ec1[:, tci, :],
                                             start=(tci == 0), stop=(tci == TC_CA - 1),
                                             skip_group_check=True)

                    if ca_mode == "generic":
                        # masks force a full-e barrier; keep the simple order
                        for g0 in range(0, TC_CA, 2):
                            ca_scores(g0)
                        nc.vector.tensor_tensor(ec0, ec0, expmc_sb, OP.mult)
                        nc.vector.tensor_tensor(ec1, ec1, expmc_sb, OP.mult)
                        pvc0 = psum_pv.tile([DH + 1, LTOT], f32, tag="pvc")
                        pvc1 = psum_pv.tile([DH + 1, LTOT], f32, tag="pvc")
                        if prev is not None:
                            flush_ca(prev)
                        ca_pv(0, TC_CA)
                    else:
                        # pipeline: pv lags scores by one pair-group; the
                        # deferred normalize of hc-1 fills the first slot.
                        ca_scores(0)
                        pvc0 = psum_pv.tile([DH + 1, LTOT], f32, tag="pvc")
                        pvc1 = psum_pv.tile([DH + 1, LTOT], f32, tag="pvc")
                        if prev is not None:
                            flush_ca(prev)
                        for g0 in range(2, TC_CA, 2):
                            ca_scores(g0)
                            ca_pv(g0 - 2, g0)
                        ca_pv(TC_CA - 2, TC_CA)
                    rec0 = den_recip(pvc0[DH:DH + 1, :], at_pool, [LTOT])
                    rec1 = den_recip(pvc1[DH:DH + 1, :], at_pool, [LTOT])
                    prev = (hc, pvc0, rec0, pvc1, rec1)
                flush_ca(prev)

            with ExitStack() as octx:
                wop = octx.enter_context(tc.tile_pool(name="wo_ca_p", bufs=3))
                otp = octx.enter_context(tc.tile_pool(name="otmp_ca", bufs=2))
                psum_op = octx.enter_context(tc.tile_pool(name="psum_oca", bufs=4, space="PSUM"))

                def evoc(oc, tt, ps):
                    tmp = otp.tile([P, LTOT], f32, tag="otmp")
                    nc.scalar.activation(tmp, ps, AF.Identity,
                                         bias=bias_sb["bo_ca"][:, oc:oc + 1],
                                         scale=INV_OW)
                    nc.vector.tensor_tensor(h2pre[:, oc, :], tmp,
                                            h1_sb[:, oc, :], OP.add)
                proj_to(wop, "wtb", f8, ocT_sb, wd["wo_ca"], KC, evoc, psum_op,
                        dr=True)

            h2_pool = ctx.enter_context(tc.tile_pool(name="h2p", bufs=1, side="right"))
            h2_sb = h2_pool.tile([P, KC, LTOT], f32, tag="h2")
            h2bf = bfp.tile([P, KC, LTOT], bf, tag="bfx")
            layer_norm("ln2", h2pre, gd["g2"], bias_sb["lb2"], h2_sb, bf_out=h2bf)

        # =================== FFN ===================
        with ExitStack() as fctx:
            ffn_pool = fctx.enter_context(tc.tile_pool(name="ffn", bufs=1))
            w2pool = fctx.enter_context(tc.tile_pool(name="wtile32", bufs=2))
            w1pool = fctx.enter_context(tc.tile_pool(name="w1p", bufs=3))
            psum_f = fctx.enter_context(tc.tile_pool(name="psum_f", bufs=4, space="PSUM"))
            f1_sb = ffn_pool.tile([P, KC2, LTOT], bf, tag="f1")
            h3pre = pre_pool.tile([P, KC, LTOT], f32, tag="pre")

            def evg(oc, tt, ps):
                nc.scalar.activation(f1_sb[:, oc, :], ps, AF.Gelu,
                                     bias=b1_sb[:, oc:oc + 1])
            proj_to(w1pool, "wtb", bf, h2bf, w1_d, KC2, evg, psum_f)

            for oc in range(KC):
                w2t = w2pool.tile([P, KC2, P], bf, tag="w2t")
                nc.sync.dma_start(w2t, w2_d[:, :, ts(oc, P)])
                ps = psum_f.tile([P, LTOT], f32, tag="psproj")
                for kc in range(KC2):
                    nc.tensor.matmul(ps, w2t[:, kc, :], f1_sb[:, kc, :],
                                     start=(kc == 0), stop=(kc == KC2 - 1))
                nc.vector.scalar_tensor_tensor(
                    h3pre[:, oc, :], ps, bias_sb["b2"][:, oc:oc + 1],
                    h2_sb[:, oc, :], OP.add, OP.add)

        out_sb = h1_pool.tile([P, KC, LTOT], f32, tag="h1")
        layer_norm("ln3", h3pre, gd["g3"], bias_sb["lb3"], out_sb,
                   chunk_done=lambda kc: nc.sync.dma_start(out_d[:, kc, :],
                                                           out_sb[:, kc, :]))


# ---------------------------------------------------------------------------
# Host-side packing
# ---------------------------------------------------------------------------

def _pack_wT(w, dtype=np.float32):
    # w: [dout, din] -> [P, din//P, dout] with wT[d, o] layout
    din = w.shape[1]
    return np.ascontiguousarray(
        w.T.reshape(din // P, P, w.shape[0]).transpose(1, 0, 2)).astype(dtype)


def _pack_xT(x, dtype=np.float32):
    # x: [T, D] -> [P, KC, T]
    t = x.shape[0]
    return np.ascontiguousarray(x.T.reshape(KC, P, t).transpose(1, 0, 2)).astype(dtype)


def _pack_bias(v):
    n = v.shape[0] // P
    return np.ascontiguousarray(v.reshape(n, P).T).astype(np.float32)


def detect_sa_mode(mask):
    if not np.isfinite(np.nan_to_num(mask, nan=np.inf)).all():
        return "generic"
    if (mask == 0).all():
        return "zeros"
    li, ti = np.tril_indices(L)
    if (mask[li, ti] == 0).all():
        ui, uj = np.triu_indices(L, k=1)
        if (mask[ui, uj] <= -1e8).all():
            return "causal"
    return "generic"


def make_in_maps(inputs):
    inputs = {k: np.asarray(v, dtype=np.float32) for k, v in inputs.items()}
    mask = inputs["attention_mask"]
    cmask = inputs["encoder_attention_mask"]
    sa_mode = detect_sa_mode(mask)
    ca_mode = "zeros" if (cmask == 0).all() else "generic"
    s = DH ** -0.5

    def fp8q(arr):
        return np.clip(arr, -240.0, 240.0).astype(FP8)

    A = S_X * S_W
    shared = {
        "wq_sa": fp8q(_pack_wT(inputs["sa_wq"] * (s * S_W))),
        "wk_sa": fp8q(_pack_wT(inputs["sa_wk"] * S_W)),
        "wv_sa": fp8q(_pack_wT(inputs["sa_wv"] * S_W)),
        "wo_sa": fp8q(_pack_wT(inputs["sa_wo"] * S_W)),
        "wq_ca": fp8q(_pack_wT(inputs["ca_wq"] * (s * S_W))),
        "wk_ca": fp8q(_pack_wT(inputs["ca_wk"] * S_W)),
        "wv_ca": fp8q(_pack_wT(inputs["ca_wv"] * S_W)),
        "wo_ca": fp8q(_pack_wT(inputs["ca_wo"] * S_W)),
        "w1": _pack_wT(inputs["ffn_w1"], BF16),
        "w2": _pack_wT(inputs["ffn_w2"], BF16),
        "biases": np.concatenate([
            _pack_bias(inputs["sa_bq"] * (s * A)),
            _pack_bias(inputs["sa_bk"] * A),
            _pack_bias(inputs["sa_bo"] + inputs["sa_wo"] @ inputs["sa_bv"]),
            _pack_bias(inputs["ca_bq"] * (s * A)),
            _pack_bias(inputs["ca_bk"] * A),
            _pack_bias(inputs["ca_bo"] + inputs["ca_wo"] @ inputs["ca_bv"]),
            _pack_bias(inputs["ffn_b2"]),
            _pack_bias(inputs["sa_ln_b"]),
            _pack_bias(inputs["ca_ln_b"]),
            _pack_bias(inputs["ffn_ln_b"]),
            _pack_bias(inputs["ffn_b1"]),
        ], axis=1),
        "g1": np.ascontiguousarray(inputs["sa_ln_g"].reshape(1, D)).astype(BF16),
        "g2": np.ascontiguousarray(inputs["ca_ln_g"].reshape(1, D)).astype(BF16),
        "g3": np.ascontiguousarray(inputs["ffn_ln_g"].reshape(1, D)).astype(BF16),
    }

    exts = EXT_CAUSAL if sa_mode == "causal" else [TC_SA] * NL
    in_maps = []
    for c in range(8):
        b, i = c // 4, c % 4
        blocks = core_blocks(i)
        own_rows = np.concatenate([np.arange(p * LW, (p + 1) * LW) for p in blocks])
        xTp32 = _pack_xT(inputs["hidden_states"][b])
        m = dict(shared)
        m["xT"] = fp8q(xTp32 * S_X)
        m["xTown"] = np.ascontiguousarray(xTp32[:, :, own_rows])
        m["xTownb"] = fp8q(m["xTown"] * S_X)
        m["encT"] = fp8q(_pack_xT(inputs["encoder_hidden_states"][b]) * S_X)
        if sa_mode == "causal":
            em = np.empty((P, NL, MREG, LW), dtype=BF16)
            for j, pblk in enumerate(blocks):
                rows = slice(pblk * LW, (pblk + 1) * LW)
                t0 = (exts[j] - MREG) * P
                blk = np.exp(np.minimum(mask[rows, t0:t0 + MREG * P], 60.0))
                em[:, j] = blk.reshape(LW, MREG, P).transpose(2, 1, 0)
            m["expm"] = em
        elif sa_mode == "generic":
            em = np.empty((P, TC_SA * NL, LW), dtype=BF16)
            for j, pblk in enumerate(blocks):
                rows = slice(pblk * LW, (pblk + 1) * LW)
                blk = np.exp(np.minimum(mask[rows, :], 60.0))
                em[:, j::NL, :] = blk.reshape(LW, TC_SA, P).transpose(2, 1, 0)
            m["expm"] = em
        if ca_mode == "generic":
            em = np.empty((P, TC_CA, LTOT), dtype=BF16)
            for j, pblk in enumerate(blocks):
                rows = slice(pblk * LW, (pblk + 1) * LW)
                blk = np.exp(np.minimum(cmask[rows, :], 60.0))
                em[:, :, j * LW:(j + 1) * LW] = blk.reshape(LW, TC_CA, P).transpose(2, 1, 0)
            m["expmc"] = em
        in_maps.append(m)
    return in_maps, sa_mode, ca_mode


def assemble_output(results):
    out = np.zeros((B, L, D), np.float32)
    for c in range(8):
        b, i = c // 4, c % 4
        arr = np.asarray(results[c]["out"])  # [P, KC, LTOT]
        for j, pblk in enumerate(core_blocks(i)):
            blk = arr[:, :, j * LW:(j + 1) * LW]          # [P, KC, LW]
            out[b, pblk * LW:(pblk + 1) * LW, :] = blk.transpose(2, 1, 0).reshape(LW, D)
    return out


# ---------------------------------------------------------------------------
# Entry point
# ---------------------------------------------------------------------------

_NC_CACHE = {}


def get_nc(sa_mode, ca_mode):
    key = (sa_mode, ca_mode)
    if key not in _NC_CACHE:
        _NC_CACHE[key] = build_nc(sa_mode, ca_mode)
    return _NC_CACHE[key]


def _install_ntff_hook():
    """bass_utils' trace path needs antenv.axon_hooks, absent in this image.
    Inject a shim and register the ctypes-based NTFF hook from trn_agent_boot."""
    import types
    if "antenv.axon_hooks" in sys.modules:
        return
    holder = {}
    mod = types.ModuleType("antenv.axon_hooks")
    mod.set_axon_ntff_profile_hook = lambda h: holder.__setitem__("h", h)
    mod.get_axon_ntff_profile_hook = lambda: holder.get("h")
    sys.modules["antenv.axon_hooks"] = mod
    try:
        import antenv
        antenv.axon_hooks = mod
    except ImportError:
        pass
    try:
        from trn_agent_boot.trn_boot import _ntff_profile_via_ctypes
        hook = _ntff_profile_via_ctypes("/opt/axon/libaxon_pjrt.so")
        if hook is not None:
            mod.set_axon_ntff_profile_hook(hook)
    except Exception as e:  # degrade to no tracing
        print(f"ntff hook install failed: {e}", file=sys.stderr)


def run(inputs, trace=False):
    _install_ntff_hook()
    from concourse.bass_utils import run_bass_kernel_spmd
    in_maps, sa_mode, ca_mode = make_in_maps(inputs)
    nc = get_nc(sa_mode, ca_mode)
    res = run_bass_kernel_spmd(nc, in_maps, core_ids=list(range(8)), trace=trace)
    return assemble_output(res.results), res


def kernel(**inputs):
    out, _ = run(inputs, trace=False)
    return out



# revision 48
# speedup vs baseline: 1.0447x; 1.0253x over previous
"""Trainium2 Bass kernel for nn_ConicaLayer (transformer decoder layer:
self-attn (causal) + cross-attn + FFN, post-LN residuals).

Sharding: rows (B x L) split across 8 cores; core c -> batch b=c//4, and 4
interleaved 128-row blocks {i, 7-i, 8+i, 15-i} of the 16 blocks of that batch
(balances causal attention work). Each core computes full K/V for its batch.

All transposes/packing are done host-side; on-device activations stay d-major
([D, tokens]) end-to-end. Softmax uses exp(s-20) without max-subtraction
(scores are bounded; masked entries handled by multiplying exp(mask), with
fully-masked blocks skipped in causal mode). The V matrix carries an appended
ones column so PV matmuls also produce softmax denominators. V-bias folds into
the out-proj bias host-side (bo_eff = bo + wo @ bv, exact since sum(p) = 1).

Dtypes: bf16 for kv-path (xT/encT, wk/wv, k/v/q/e), attention out + wo, f1 +
w2. f32 for residual stream, wq/w1, layernorms, all PSUM accumulation.
"""

import sys
import numpy as np

try:
    import concourse.bass as bass  # noqa: F401
except ImportError:
    sys.path.insert(0, "/opt/trn_rl_repo")

import ml_dtypes
import concourse.bass as bass
import concourse.bacc as bacc
import concourse.tile as tile
from concourse import mybir
from concourse.bass import ts

BF16 = ml_dtypes.bfloat16

P = 128
B, L, S, D, H, DFF = 2, 2048, 1024, 1024, 16, 4096
DH = D // H           # 64
KC = D // P           # 8
KC2 = DFF // P        # 32
NL = 4                # l-blocks per core
LW = 128              # l width per block
LTOT = NL * LW        # 512 rows per core
TC_SA = L // P        # 16 t-chunks (self attn)
TC_CA = S // P        # 8 t-chunks (cross attn)
MREG = 4              # masked tail chunks per block (causal mode)
EXT_CAUSAL = [4, 8, 12, 16]
EXP_SHIFT = -20.0
EPS = 1e-5

f32 = mybir.dt.float32
bf = mybir.dt.bfloat16
f8 = mybir.dt.float8e4
FP8 = mybir.dt.np(f8)
DR = mybir.MatmulPerfMode.DoubleRow
AF = mybir.ActivationFunctionType
OP = mybir.AluOpType

# fp8 scaling: activations x16, weights x512. q/k carry the combined factor
# A into SBUF (descaled for free in the exp's scale arg); v carries A into
# the PV output (descaled by dividing wo by A host-side).
S_X = 16.0
S_W = 512.0
A_SC = S_X * S_W
INV_A2 = 1.0 / (A_SC * A_SC)
INV_A = 1.0 / A_SC
INV_W = 1.0 / S_W
S_O = 32.0
INV_OW = 1.0 / (S_O * S_W)


def core_blocks(i):
    return [i, 7 - i, 8 + i, 15 - i]


# ---------------------------------------------------------------------------
# Bass kernel builder
# ---------------------------------------------------------------------------

def build_nc(sa_mode, ca_mode):
    """sa_mode: 'causal' | 'zeros' | 'generic'; ca_mode: 'zeros' | 'generic'."""
    nc = bacc.Bacc("TRN2", target_bir_lowering=False, debug=False, num_devices=8)

    def din(name, shape, dtype=f32):
        return nc.dram_tensor(name, list(shape), dtype, kind="ExternalInput").ap()

    env = {}
    env["sa_mode"], env["ca_mode"] = sa_mode, ca_mode
    env["exts"] = EXT_CAUSAL if sa_mode == "causal" else [TC_SA] * NL
    env["xT_d"] = din("xT", [P, KC, L], f8)
    env["xTown_d"] = din("xTown", [P, KC, LTOT])
    env["xTownb_d"] = din("xTownb", [P, KC, LTOT], f8)
    env["encT_d"] = din("encT", [P, KC, S], f8)
    wdt = {"wq_sa": f8, "wk_sa": f8, "wv_sa": f8, "wo_sa": f8,
           "wq_ca": f8, "wk_ca": f8, "wv_ca": f8, "wo_ca": f8}
    env["wd"] = {n: din(n, [P, KC, D], dt) for n, dt in wdt.items()}
    env["w1_d"] = din("w1", [P, KC, DFF], bf)
    env["w2_d"] = din("w2", [P, KC2, D], bf)
    env["biases_d"] = din("biases", [P, 10 * KC + KC2])
    env["gd"] = {n: din(n, [1, D], bf) for n in ["g1", "g2", "g3"]}
    env["expm_d"] = None
    if sa_mode == "causal":
        env["expm_d"] = din("expm", [P, NL, MREG, LW], bf)
    elif sa_mode == "generic":
        env["expm_d"] = din("expm", [P, TC_SA * NL, LW], bf)
    env["expmc_d"] = din("expmc", [P, TC_CA, LTOT], bf) if ca_mode == "generic" else None
    env["out_d"] = nc.dram_tensor("out", [P, KC, LTOT], f32, kind="ExternalOutput").ap()

    with tile.TileContext(nc) as tc:
        _build_body(nc, tc, env)
    nc.compile()
    return nc


def _build_body(nc, tc, env):
    from contextlib import ExitStack

    xT_d, xTown_d, encT_d = env["xT_d"], env["xTown_d"], env["encT_d"]
    xTownb_d = env["xTownb_d"]
    wd, w1_d, w2_d, biases_d, gd = env["wd"], env["w1_d"], env["w2_d"], env["biases_d"], env["gd"]
    expm_d, expmc_d, out_d = env["expm_d"], env["expmc_d"], env["out_d"]
    sa_mode, ca_mode, exts = env["sa_mode"], env["ca_mode"], env["exts"]

    with ExitStack() as ctx:
        consts = ctx.enter_context(tc.tile_pool(name="consts", bufs=1))

        # ---- constants (tiles allocated now; DMAs deferred via load_consts
        # so the startup DMA queue serves the k-proj inputs first) ----
        biases_sb = consts.tile([P, 10 * KC + KC2], f32, tag="c_bias")
        bias_names = ["bq_sa", "bk_sa", "bo_sa", "bq_ca", "bk_ca", "bo_ca",
                      "b2", "lb1", "lb2", "lb3"]
        bias_sb = {n: biases_sb[:, i * KC:(i + 1) * KC]
                   for i, n in enumerate(bias_names)}
        b1_sb = biases_sb[:, 10 * KC:10 * KC + KC2]
        ones128b = consts.tile([P, 1], bf, tag="ones128b")
        nc.vector.memset(ones128b, 1.0)
        ones64b = consts.tile([1, DH], bf, tag="ones64b")
        nc.vector.memset(ones64b, 1.0)
        eps_sb = consts.tile([1, 1], f32, tag="eps")
        nc.vector.memset(eps_sb, EPS)
        zero128 = consts.tile([P, 1], f32, tag="zero128")
        nc.vector.memset(zero128, 0.0)
        shift128 = consts.tile([P, 1], f32, tag="shift128")
        nc.vector.memset(shift128, EXP_SHIFT)
        expm_sb = None
        if expm_d is not None:
            shape = [P, NL, MREG, LW] if sa_mode == "causal" else [P, TC_SA * NL, LW]
            expm_sb = consts.tile(shape, bf, tag="expm")
        expmc_sb = None
        if expmc_d is not None:
            expmc_sb = consts.tile([P, TC_CA, LTOT], bf, tag="expmc")

        def load_consts():
            nc.sync.dma_start(biases_sb, biases_d)
            if expm_sb is not None:
                nc.sync.dma_start(expm_sb, expm_d)
            if expmc_sb is not None:
                nc.sync.dma_start(expmc_sb, expmc_d)

        # ------------- helpers -------------
        def proj_to(wpool, wtag, wdt, src_sb, w_dram, n_oc, evict, psum_proj, n_tt=1,
                    tt_width=LTOT, n_kc=KC, dr=False, wt0=None):
            for oc in range(n_oc):
                if oc == 0 and wt0 is not None:
                    wt = wt0
                else:
                    wt = wpool.tile([P, n_kc, P], wdt, tag=wtag)
                    nc.sync.dma_start(wt, w_dram[:, :, ts(oc, P)])
                for tt in range(n_tt):
                    ps = psum_proj.tile([P, tt_width], f32, tag="psproj")
                    if dr:
                        for k2 in range(n_kc // 2):
                            nc.tensor.matmul(
                                ps, wt[:, 2 * k2:2 * k2 + 2, :],
                                src_sb[:, 2 * k2:2 * k2 + 2, ts(tt, tt_width)],
                                start=(k2 == 0), stop=(k2 == n_kc // 2 - 1),
                                perf_mode=DR)
                    else:
                        for kc in range(n_kc):
                            nc.tensor.matmul(
                                ps, wt[:, kc, :], src_sb[:, kc, ts(tt, tt_width)],
                                start=(kc == 0), stop=(kc == n_kc - 1))
                    evict(oc, tt, ps)

        def layer_norm(tag, x_sb, g_dram, lb, out_sb, chunk_done=None, bf_out=None,
                       bf_scale=None):
            # Stats and per-token scale broadcasts run as bf16 matmuls (4x the
            # fp32 matmul rate); the residual/output path stays fp32.
            with ExitStack() as lctx:
                lnp = lctx.enter_context(tc.tile_pool(
                    name=f"lnp_{tag}", bufs=1 if sa_mode == "generic" else 2))
                lns = lctx.enter_context(tc.tile_pool(name=f"lns_{tag}", bufs=1))
                psum_st = lctx.enter_context(
                    tc.tile_pool(name=f"psum_lns_{tag}", bufs=1, space="PSUM"))
                psum_ln = lctx.enter_context(
                    tc.tile_pool(name=f"psum_ln_{tag}", bufs=3, space="PSUM"))
                g = lns.tile([1, D], bf, tag="ln_g")
                nc.sync.dma_start(g, g_dram)
                psx = psum_st.tile([1, LTOT], f32, tag="ln_sx")
                psx2 = psum_st.tile([1, LTOT], f32, tag="ln_sx2")
                for kc in range(KC):
                    xb = lnp.tile([P, LTOT], bf, tag="ln_xb")
                    nc.vector.tensor_copy(xb, x_sb[:, kc, :])
                    sq = lnp.tile([P, LTOT], bf, tag="ln_sq")
                    nc.scalar.activation(sq, x_sb[:, kc, :], AF.Square, bias=zero128)
                    nc.tensor.matmul(psx, ones128b, xb,
                                     start=(kc == 0), stop=(kc == KC - 1))
                    nc.tensor.matmul(psx2, ones128b, sq,
                                     start=(kc == 0), stop=(kc == KC - 1))
                mean = lns.tile([1, LTOT], f32, tag="ln_mean")
                nc.vector.tensor_scalar_mul(mean, psx, 1.0 / D)
                msq = lns.tile([1, LTOT], f32, tag="ln_msq")
                nc.vector.tensor_tensor(msq, mean, mean, OP.mult)
                var = lns.tile([1, LTOT], f32, tag="ln_var")
                nc.vector.scalar_tensor_tensor(var, psx2, 1.0 / D, msq, OP.mult, OP.subtract)
                std = lns.tile([1, LTOT], f32, tag="ln_std")
                nc.scalar.activation(std, var, AF.Sqrt, bias=eps_sb)
                rstd = lns.tile([1, LTOT], f32, tag="ln_rstd")
                nc.vector.reciprocal_approx_fast(rstd, std)
                rstd_b = lns.tile([1, LTOT], bf, tag="ln_rstdb")
                nc.vector.tensor_copy(rstd_b, rstd)
                mrstd_b = lns.tile([1, LTOT], bf, tag="ln_mrstdb")
                nc.vector.tensor_tensor(mrstd_b, mean, rstd, OP.mult)
                for kc in range(KC):
                    pg = psum_ln.tile([P, LTOT], f32, tag="ln_pg")
                    pm = psum_ln.tile([P, LTOT], f32, tag="ln_pm")
                    nc.tensor.matmul(pg, g[:, ts(kc, P)], rstd_b, start=True, stop=True)
                    nc.tensor.matmul(pm, g[:, ts(kc, P)], mrstd_b, start=True, stop=True)
                    t = lnp.tile([P, LTOT], f32, tag="ln_t")
                    nc.vector.tensor_tensor(t, x_sb[:, kc, :], pg, OP.mult)
                    nc.vector.scalar_tensor_tensor(out_sb[:, kc, :], t, lb[:, kc:kc + 1],
                                                   pm, OP.add, OP.subtract)
                    if bf_out is not None:
                        if bf_scale is not None:
                            nc.vector.tensor_scalar_mul(bf_out[:, kc, :],
                                                        out_sb[:, kc, :], bf_scale)
                        else:
                            nc.vector.tensor_copy(bf_out[:, kc, :], out_sb[:, kc, :])
                    if chunk_done is not None:
                        chunk_done(kc)

        def kv_proj_v(vpool_dst, src_sb, wv_dram, wrhs, wrhs_tag, n_tc, psum_proj):
            for half in range(2):
                wvh = wrhs.tile([P, KC, 512], f8, tag=wrhs_tag)
                nc.sync.dma_start(wvh, wv_dram[:, :, ts(half, 512)])
                for tci in range(n_tc):
                    ps = psum_proj.tile([P, 512], f32, tag="psproj")
                    for k2 in range(KC // 2):
                        nc.tensor.matmul(
                            ps, src_sb[:, 2 * k2:2 * k2 + 2, ts(tci, P)],
                            wvh[:, 2 * k2:2 * k2 + 2, :],
                            start=(k2 == 0), stop=(k2 == KC // 2 - 1),
                            perf_mode=DR)
                    nc.vector.tensor_copy(
                        vpool_dst[:, tci, half * 8:(half + 1) * 8, 0:DH],
                        ps.rearrange("p (h d) -> p h d", h=8))

        def den_recip(den_row, at_pool, inner):
            # den_row: [1, *inner] PSUM slice holding softmax denominators.
            # The PSUM->SBUF copy runs on ScalarE (Copy is in every table) so
            # only the reciprocal+cast sit on the vector queue.
            sums = at_pool.tile([1] + inner, f32, tag="at_sums")
            nc.vector.tensor_copy(sums, den_row)
            recip = at_pool.tile([1] + inner, f32, tag="at_recip")
            nc.vector.reciprocal_approx_fast(recip, sums)
            recip_b = at_pool.tile([1] + inner, bf, tag="at_recipb")
            nc.vector.tensor_scalar_mul(recip_b, recip, S_O / A_SC)
            return recip_b

        def apply_norm(dst, pv_rows, recip_b, at_pool, psum_bc, inner):
            # dst = pv_rows * broadcast(recip)
            pb = psum_bc.tile([DH] + inner, f32, tag="bc")
            nc.tensor.matmul(pb, ones64b, recip_b, start=True, stop=True)
            bc_sb = at_pool.tile([DH] + inner, f32, tag="at_bc")
            nc.vector.tensor_copy(bc_sb, pb)
            if len(inner) == 2:
                dst = dst.rearrange("p (j l) -> p j l", j=inner[0])
            nc.vector.scalar_tensor_tensor(dst, pv_rows, 0.0, bc_sb,
                                           OP.bypass, OP.mult)

        # =================== SA ===================
        with ExitStack() as sctx:
            sa_pool = sctx.enter_context(tc.tile_pool(name="sa", bufs=1))
            kT_sb = sa_pool.tile([P, KC, L], bf, tag="kT")
            v_sb = sa_pool.tile([P, TC_SA, H, DH + 1], bf, tag="v")
            qT_sb = sa_pool.tile([P, KC, LTOT], bf, tag="qT")
            nc.gpsimd.memset(v_sb[:, :, :, DH:DH + 1], 1.0)

            with ExitStack() as xctx:
                xpool = xctx.enter_context(tc.tile_pool(name="xpool", bufs=1))
                wrhs = xctx.enter_context(tc.tile_pool(name="wrhs", bufs=2))
                wkp = xctx.enter_context(tc.tile_pool(name="wk_sa_p", bufs=3))
                psum_kv = xctx.enter_context(tc.tile_pool(name="psum_kv", bufs=4, space="PSUM"))
                # first weight tile and the xT chunks lead the DMA queue so
                # the k-proj can start ~4us in; const loads queue behind them.
                wt0k = wkp.tile([P, KC, P], f8, tag="wtb")
                nc.sync.dma_start(wt0k, wd["wk_sa"][:, :, ts(0, P)])
                xT_sb = xpool.tile([P, KC, L], f8, tag="xT")
                for kc in range(KC):
                    nc.sync.dma_start(xT_sb[:, kc, :], xT_d[:, kc, :])
                qsrc = xpool.tile([P, KC, LTOT], f8, tag="qsrc")
                nc.sync.dma_start(qsrc, xTownb_d)
                load_consts()

                def evk(oc, tt, ps):
                    nc.vector.tensor_scalar_add(kT_sb[:, oc, ts(tt, 512)], ps,
                                                bias_sb["bk_sa"][:, oc:oc + 1])
                proj_to(wkp, "wtb", f8, xT_sb, wd["wk_sa"], KC, evk, psum_kv,
                        n_tt=L // 512, tt_width=512, dr=True, wt0=wt0k)

                def evq(oc, tt, ps):
                    nc.vector.tensor_scalar_add(qT_sb[:, oc, :], ps,
                                                bias_sb["bq_sa"][:, oc:oc + 1])
                proj_to(wkp, "wtb", f8, qsrc, wd["wq_sa"], KC, evq, psum_kv, dr=True)

                kv_proj_v(v_sb, xT_sb, wd["wv_sa"], wrhs, "wrhs", TC_SA, psum_kv)

            ca_pool = ctx.enter_context(tc.tile_pool(name="ca", bufs=1, side="right"))
            kcT_sb = ca_pool.tile([P, KC, S], bf, tag="kcT")
            vc_sb = ca_pool.tile([P, TC_CA, H, DH + 1], bf, tag="vc")
            encT_sb = ca_pool.tile([P, KC, S], f8, tag="encT")
            for kc in range(KC):
                nc.sync.dma_start(encT_sb[:, kc, :], encT_d[:, kc, :])
            nc.gpsimd.memset(vc_sb[:, :, :, DH:DH + 1], 1.0)
            oT_sb = sctx.enter_context(tc.tile_pool(name="oT_sa", bufs=1)).tile(
                [P, KC, LTOT], f8, tag="oT")
            pre_pool = ctx.enter_context(tc.tile_pool(name="prep", bufs=1, side="right"))
            h1pre = pre_pool.tile([P, KC, LTOT], f32, tag="pre")
            nc.sync.dma_start(h1pre, xTown_d)

            with ExitStack() as actx:
                e_pool = actx.enter_context(tc.tile_pool(name="e_sa", bufs=2))
                at_pool = actx.enter_context(tc.tile_pool(name="at_sa", bufs=3))
                psum_s = actx.enter_context(tc.tile_pool(name="psum_s", bufs=2, space="PSUM"))
                psum_pv = actx.enter_context(tc.tile_pool(name="psum_pv", bufs=3, space="PSUM"))
                psum_bc = actx.enter_context(tc.tile_pool(name="psum_bc", bufs=1, space="PSUM"))

                def flush_sa(st):
                    hc_p, pvs_p, recs_p = st
                    for u in range(2):
                        apply_norm(oT_sb[u * DH:(u + 1) * DH, hc_p, :],
                                   pvs_p[u][0:DH], recs_p[u], at_pool, psum_bc,
                                   [NL, LW])

                # compact slot layout over (tc, j>=jmin(tc)); causal skips j<tc//4
                jmin = [(tci // 4 if sa_mode == "causal" else 0) for tci in range(TC_SA)]
                bases = []
                nslot = 0
                for tci in range(TC_SA):
                    bases.append(nslot)
                    nslot += NL - jmin[tci]

                prev_sa = None
                for hc in range(KC):  # head pair (2*hc, 2*hc+1)
                    pv0 = psum_pv.tile([DH + 1, NL, LW], f32, tag="pv")
                    pv1 = psum_pv.tile([DH + 1, NL, LW], f32, tag="pv")
                    e0 = e_pool.tile([P, nslot, LW], bf, tag="e_sa")
                    e1 = e_pool.tile([P, nslot, LW], bf, tag="e_sa")

                    def sa_scores(g0):
                        jm = jmin[g0]
                        N = (NL - jm) * LW
                        ps0 = psum_s.tile([P, 2, NL * LW], f32, tag="ps_sa")
                        ps1 = psum_s.tile([P, 2, NL * LW], f32, tag="ps_sa")
                        for u in range(2):
                            tci = g0 + u
                            loff = jm * LW
                            nc.tensor.matmul(
                                ps0[:, u, :N], kT_sb[0:DH, hc, ts(tci, P)],
                                qT_sb[0:DH, hc, loff:loff + N], start=True, stop=True)
                            nc.tensor.matmul(
                                ps1[:, u, :N], kT_sb[DH:P, hc, ts(tci, P)],
                                qT_sb[DH:P, hc, loff:loff + N], start=True, stop=True)
                        nsl = 2 * (NL - jm)
                        eo0 = e0[:, bases[g0]:bases[g0] + nsl, :].rearrange(
                            "p (u j) l -> p u j l", u=2)
                        eo1 = e1[:, bases[g0]:bases[g0] + nsl, :].rearrange(
                            "p (u j) l -> p u j l", u=2)
                        nc.scalar.activation(
                            eo0, ps0[:, :, :N].rearrange("p u (j l) -> p u j l", l=LW),
                            AF.Exp, bias=shift128, scale=INV_A2)
                        nc.scalar.activation(
                            eo1, ps1[:, :, :N].rearrange("p u (j l) -> p u j l", l=LW),
                            AF.Exp, bias=shift128, scale=INV_A2)

                    def sa_mask(j):
                        w = NL - j  # slot stride across the 4 diagonal chunks
                        for e_sb in (e0, e1):
                            view = e_sb[:, bases[4 * j]:bases[4 * j] + MREG * w, :]
                            view = view.rearrange("p (t w) l -> p t w l", w=w)[:, :, 0, :]
                            nc.vector.tensor_tensor(view, view,
                                                    expm_sb[:, j, :, :], OP.mult)

                    def sa_pv(tlo, thi):
                        for tci in range(tlo, thi):
                            jm = jmin[tci]
                            nc.tensor.matmul(
                                pv0[:, jm:, :], v_sb[:, tci, 2 * hc, :],
                                e0[:, bases[tci]:bases[tci] + NL - jm, :],
                                start=(tci == 0), stop=(tci == TC_SA - 1),
                                skip_group_check=True)
                            nc.tensor.matmul(
                                pv1[:, jm:, :], v_sb[:, tci, 2 * hc + 1, :],
                                e1[:, bases[tci]:bases[tci] + NL - jm, :],
                                start=(tci == 0), stop=(tci == TC_SA - 1),
                                skip_group_check=True)

                    if sa_mode == "causal":
                        # Interleave the PV chains one block-group behind the
                        # scores/exp so TensorE rides through the exp latency;
                        # the deferred normalize of hc-1 fills the first slot.
                        for j in range(NL):
                            sa_scores(4 * j)
                            if j == 0:
                                if prev_sa is not None:
                                    flush_sa(prev_sa)
                            else:
                                sa_pv(4 * (j - 1), 4 * j)
                            sa_scores(4 * j + 2)
                            sa_mask(j)
                        sa_pv(4 * (NL - 1), TC_SA)
                    else:
                        for g0 in range(0, TC_SA, 2):
                            sa_scores(g0)
                        if sa_mode == "generic":
                            nc.vector.tensor_tensor(e0, e0, expm_sb, OP.mult)
                            nc.vector.tensor_tensor(e1, e1, expm_sb, OP.mult)
                        if prev_sa is not None:
                            flush_sa(prev_sa)
                        sa_pv(0, TC_SA)
                    recs = [den_recip(pvu[DH:DH + 1], at_pool, [NL, LW])
                            for pvu in (pv0, pv1)]
                    prev_sa = (hc, (pv0, pv1), recs)
                flush_sa(prev_sa)

            # CA k/v projections as one dense fp8 block (same PE config as the
            # fp8 o-proj that follows -- avoids per-head-pair mode switches).
            with ExitStack() as kctx:
                wkvc = kctx.enter_context(tc.tile_pool(name="wkv_ca", bufs=3))
                wrhsc = kctx.enter_context(tc.tile_pool(name="wrhs_ca", bufs=2))
                psum_ckv = kctx.enter_context(
                    tc.tile_pool(name="psum_ckv", bufs=4, space="PSUM"))
                ca_state = {}

                def ca_kv_chunk(hc):
                    wt = ca_state.pop(f"wt{hc}")
                    for tt in range(S // 512):
                        ps = psum_ckv.tile([P, 512], f32, tag="ps_ckv")
                        for k2 in range(KC // 2):
                            nc.tensor.matmul(ps, wt[:, 2 * k2:2 * k2 + 2, :],
                                             encT_sb[:, 2 * k2:2 * k2 + 2, ts(tt, 512)],
                                             start=(k2 == 0), stop=(k2 == KC // 2 - 1),
                                             perf_mode=DR)
                        nc.vector.tensor_scalar_add(kcT_sb[:, hc, ts(tt, 512)], ps,
                                                    bias_sb["bk_ca"][:, hc:hc + 1])
                    half, tq = hc // 4, hc % 4
                    wvh = ca_state["wvh"]
                    for tci in (2 * tq, 2 * tq + 1):
                        ps = psum_ckv.tile([P, 512], f32, tag="ps_ckv")
                        for k2 in range(KC // 2):
                            nc.tensor.matmul(ps, encT_sb[:, 2 * k2:2 * k2 + 2, ts(tci, P)],
                                             wvh[:, 2 * k2:2 * k2 + 2, :],
                                             start=(k2 == 0), stop=(k2 == KC // 2 - 1),
                                             perf_mode=DR)
                        nc.vector.tensor_copy(
                            vc_sb[:, tci, half * 8:(half + 1) * 8, 0:DH],
                            ps.rearrange("p (h d) -> p h d", h=8))

                for hc in range(KC):
                    if hc % 4 == 0:
                        wvh_new = wrhsc.tile([P, KC, 512], f8, tag="wv_ca")
                        ca_state["wvh"] = wvh_new
                        nc.sync.dma_start(wvh_new,
                                          wd["wv_ca"][:, :, ts(hc // 4, 512)])
                    wt = wkvc.tile([P, KC, P], f8, tag="wt_ck")
                    nc.sync.dma_start(wt, wd["wk_ca"][:, :, ts(hc, P)])
                    ca_state[f"wt{hc}"] = wt
                    ca_kv_chunk(hc)

            with ExitStack() as octx:
                wop = octx.enter_context(tc.tile_pool(name="wo_sa_p", bufs=3))
                otp = octx.enter_context(tc.tile_pool(name="otmp_sa", bufs=2))
                psum_op = octx.enter_context(tc.tile_pool(name="psum_osa", bufs=4, space="PSUM"))

                def evo(oc, tt, ps):
                    # h1pre was pre-loaded with the residual (xTown); descale
                    # the fp8 o-proj on ScalarE, then accumulate in place
                    tmp = otp.tile([P, LTOT], f32, tag="otmp")
                    nc.scalar.activation(tmp, ps, AF.Identity,
                                         bias=bias_sb["bo_sa"][:, oc:oc + 1],
                                         scale=INV_OW)
                    nc.vector.tensor_tensor(h1pre[:, oc, :], tmp,
                                            h1pre[:, oc, :], OP.add)
                proj_to(wop, "wtb", f8, oT_sb, wd["wo_sa"], KC, evo, psum_op,
                        dr=True)

            h1_pool = ctx.enter_context(tc.tile_pool(name="h1p", bufs=1, side="right"))
            h1_sb = h1_pool.tile([P, KC, LTOT], f32, tag="h1")
            bfp = ctx.enter_context(tc.tile_pool(name="bfcast", bufs=1, side="right"))
            h1bf = bfp.tile([P, KC, LTOT], f8, tag="bfx")
            layer_norm("ln1", h1pre, gd["g1"], bias_sb["lb1"], h1_sb, bf_out=h1bf,
                       bf_scale=S_X)

        # =================== CA ===================
        with ExitStack() as cctx:
            qcT_sb = cctx.enter_context(tc.tile_pool(name="qc_ca", bufs=1)).tile(
                [P, KC, LTOT], bf, tag="qcT")

            with ExitStack() as xctx:
                wkp = xctx.enter_context(tc.tile_pool(name="wk_ca_p", bufs=3))
                psum_kv = xctx.enter_context(tc.tile_pool(name="psum_cq", bufs=4, space="PSUM"))

                def evqc(oc, tt, ps):
                    nc.vector.tensor_scalar_add(qcT_sb[:, oc, :], ps,
                                                bias_sb["bq_ca"][:, oc:oc + 1])
                proj_to(wkp, "wtb", f8, h1bf, wd["wq_ca"], KC, evqc, psum_kv,
                        dr=True)

            ocT_sb = cctx.enter_context(tc.tile_pool(name="oT_ca", bufs=1)).tile(
                [P, KC, LTOT], f8, tag="ocT")
            h2pre = pre_pool.tile([P, KC, LTOT], f32, tag="pre")

            with ExitStack() as actx:
                e_pool = actx.enter_context(tc.tile_pool(name="e_ca", bufs=4))
                at_pool = actx.enter_context(tc.tile_pool(name="at_ca", bufs=3))
                psum_s = actx.enter_context(tc.tile_pool(name="psum_cs", bufs=2, space="PSUM"))
                psum_pv = actx.enter_context(tc.tile_pool(name="psum_cpv", bufs=3, space="PSUM"))
                psum_bc = actx.enter_context(tc.tile_pool(name="psum_cbc", bufs=1, space="PSUM"))

                def flush_ca(st):
                    hc, pvu0, rec0, pvu1, rec1 = st
                    apply_norm(ocT_sb[0:DH, hc, :], pvu0[0:DH, :], rec0,
                               at_pool, psum_bc, [LTOT])
                    apply_norm(ocT_sb[DH:P, hc, :], pvu1[0:DH, :], rec1,
                               at_pool, psum_bc, [LTOT])

                # normalize for head pair hc-1 is deferred to between the two
                # PV chains of pair hc so its broadcast matmuls never stall
                # the tensor queue waiting on the DVE reciprocal.
                prev = None
                for hc in range(KC):  # head pair (2*hc, 2*hc+1)
                    ec0 = e_pool.tile([P, TC_CA, LTOT], bf, tag="ec")
                    ec1 = e_pool.tile([P, TC_CA, LTOT], bf, tag="ec")

                    def ca_scores(g0):
                        cs0 = psum_s.tile([P, 2, LTOT], f32, tag="cs")
                        cs1 = psum_s.tile([P, 2, LTOT], f32, tag="cs")
                        for u in range(2):
                            tci = g0 + u
                            nc.tensor.matmul(cs0[:, u, :],
                                             kcT_sb[0:DH, hc, ts(tci, P)],
                                             qcT_sb[0:DH, hc, :],
                                             start=True, stop=True)
                            nc.tensor.matmul(cs1[:, u, :],
                                             kcT_sb[DH:P, hc, ts(tci, P)],
                                             qcT_sb[DH:P, hc, :],
                                             start=True, stop=True)
                        nc.scalar.activation(ec0[:, g0:g0 + 2, :], cs0, AF.Exp,
                                             bias=shift128, scale=INV_A2)
                        nc.scalar.activation(ec1[:, g0:g0 + 2, :], cs1, AF.Exp,
                                             bias=shift128, scale=INV_A2)

                    def ca_pv(tlo, thi):
                        for tci in range(tlo, thi):
                            nc.tensor.matmul(pvc0, vc_sb[:, tci, 2 * hc, :],
                                             ec0[:, tci, :],
                                             start=(tci == 0), stop=(tci == TC_CA - 1),
                                             skip_group_check=True)
                            nc.tensor.matmul(pvc1, vc_sb[:, tci, 2 * hc + 1, :],
                                             ec1[:, tci, :],
                                             start=(tci == 0), stop=(tci == TC_CA - 1),
                                             skip_group_check=True)

                    if ca_mode == "generic":
                        # masks force a full-e barrier; keep the simple order
                        for g0 in range(0, TC_CA, 2):
                            ca_scores(g0)
                        nc.vector.tensor_tensor(ec0, ec0, expmc_sb, OP.mult)
                        nc.vector.tensor_tensor(ec1, ec1, expmc_sb, OP.mult)
                        pvc0 = psum_pv.tile([DH + 1, LTOT], f32, tag="pvc")
                        pvc1 = psum_pv.tile([DH + 1, LTOT], f32, tag="pvc")
                        if prev is not None:
                            flush_ca(prev)
                        ca_pv(0, TC_CA)
                    else:
                        # pipeline: pv lags scores by one pair-group; the
                        # deferred normalize of hc-1 fills the first slot.
                        ca_scores(0)
                        pvc0 = psum_pv.tile([DH + 1, LTOT], f32, tag="pvc")
                        pvc1 = psum_pv.tile([DH + 1, LTOT], f32, tag="pvc")
                        if prev is not None:
                            flush_ca(prev)
                        for g0 in range(2, TC_CA, 2):
                            ca_scores(g0)
                            ca_pv(g0 - 2, g0)
                        ca_pv(TC_CA - 2, TC_CA)
                    rec0 = den_recip(pvc0[DH:DH + 1, :], at_pool, [LTOT])
                    rec1 = den_recip(pvc1[DH:DH + 1, :], at_pool, [LTOT])
                    prev = (hc, pvc0, rec0, pvc1, rec1)
                flush_ca(prev)

            with ExitStack() as octx:
                wop = octx.enter_context(tc.tile_pool(name="wo_ca_p", bufs=3))
                otp = octx.enter_context(tc.tile_pool(name="otmp_ca", bufs=2))
                psum_op = octx.enter_context(tc.tile_pool(name="psum_oca", bufs=4, space="PSUM"))

                def evoc(oc, tt, ps):
                    tmp = otp.tile([P, LTOT], f32, tag="otmp")
                    nc.scalar.activation(tmp, ps, AF.Identity,
                                         bias=bias_sb["bo_ca"][:, oc:oc + 1],
                                         scale=INV_OW)
                    nc.vector.tensor_tensor(h2pre[:, oc, :], tmp,
                                            h1_sb[:, oc, :], OP.add)
                proj_to(wop, "wtb", f8, ocT_sb, wd["wo_ca"], KC, evoc, psum_op,
                        dr=True)

            h2_pool = ctx.enter_context(tc.tile_pool(name="h2p", bufs=1, side="right"))
            h2_sb = h2_pool.tile([P, KC, LTOT], f32, tag="h2")
            h2bf = bfp.tile([P, KC, LTOT], bf, tag="bfx")
            layer_norm("ln2", h2pre, gd["g2"], bias_sb["lb2"], h2_sb, bf_out=h2bf)

        # =================== FFN ===================
        with ExitStack() as fctx:
            ffn_pool = fctx.enter_context(tc.tile_pool(name="ffn", bufs=1))
            w2pool = fctx.enter_context(tc.tile_pool(name="wtile32", bufs=2))
            w1pool = fctx.enter_context(tc.tile_pool(name="w1p", bufs=3))
            psum_f = fctx.enter_context(tc.tile_pool(name="psum_f", bufs=4, space="PSUM"))
            f1_sb = ffn_pool.tile([P, KC2, LTOT], bf, tag="f1")
            h3pre = pre_pool.tile([P, KC, LTOT], f32, tag="pre")

            def evg(oc, tt, ps):
                nc.scalar.activation(f1_sb[:, oc, :], ps, AF.Gelu,
                                     bias=b1_sb[:, oc:oc + 1])
            proj_to(w1pool, "wtb", bf, h2bf, w1_d, KC2, evg, psum_f)

            for oc in range(KC):
                w2t = w2pool.tile([P, KC2, P], bf, tag="w2t")
                nc.sync.dma_start(w2t, w2_d[:, :, ts(oc, P)])
                ps = psum_f.tile([P, LTOT], f32, tag="psproj")
                for kc in range(KC2):
                    nc.tensor.matmul(ps, w2t[:, kc, :], f1_sb[:, kc, :],
                                     start=(kc == 0), stop=(kc == KC2 - 1))
                nc.vector.scalar_tensor_tensor(
                    h3pre[:, oc, :], ps, bias_sb["b2"][:, oc:oc + 1],
                    h2_sb[:, oc, :], OP.add, OP.add)

        out_sb = h1_pool.tile([P, KC, LTOT], f32, tag="h1")
        layer_norm("ln3", h3pre, gd["g3"], bias_sb["lb3"], out_sb,
                   chunk_done=lambda kc: nc.sync.dma_start(out_d[:, kc, :],
                                                           out_sb[:, kc, :]))


# ---------------------------------------------------------------------------
# Host-side packing
# ---------------------------------------------------------------------------

def _pack_wT(w, dtype=np.float32):
    # w: [dout, din] -> [P, din//P, dout] with wT[d, o] layout
    din = w.shape[1]
    return np.ascontiguousarray(
        w.T.reshape(din // P, P, w.shape[0]).transpose(1, 0, 2)).astype(dtype)


def _pack_xT(x, dtype=np.float32):
    # x: [T, D] -> [P, KC, T]
    t = x.shape[0]
    return np.ascontiguousarray(x.T.reshape(KC, P, t).transpose(1, 0, 2)).astype(dtype)


def _pack_bias(v):
    n = v.shape[0] // P
    return np.ascontiguousarray(v.reshape(n, P).T).astype(np.float32)


def detect_sa_mode(mask):
    if not np.isfinite(np.nan_to_num(mask, nan=np.inf)).all():
        return "generic"
    if (mask == 0).all():
        return "zeros"
    li, ti = np.tril_indices(L)
    if (mask[li, ti] == 0).all():
        ui, uj = np.triu_indices(L, k=1)
        if (mask[ui, uj] <= -1e8).all():
            return "causal"
    return "generic"


def make_in_maps(inputs):
    inputs = {k: np.asarray(v, dtype=np.float32) for k, v in inputs.items()}
    mask = inputs["attention_mask"]
    cmask = inputs["encoder_attention_mask"]
    sa_mode = detect_sa_mode(mask)
    ca_mode = "zeros" if (cmask == 0).all() else "generic"
    s = DH ** -0.5

    def fp8q(arr):
        return np.clip(arr, -240.0, 240.0).astype(FP8)

    A = S_X * S_W
    shared = {
        "wq_sa": fp8q(_pack_wT(inputs["sa_wq"] * (s * S_W))),
        "wk_sa": fp8q(_pack_wT(inputs["sa_wk"] * S_W)),
        "wv_sa": fp8q(_pack_wT(inputs["sa_wv"] * S_W)),
        "wo_sa": fp8q(_pack_wT(inputs["sa_wo"] * S_W)),
        "wq_ca": fp8q(_pack_wT(inputs["ca_wq"] * (s * S_W))),
        "wk_ca": fp8q(_pack_wT(inputs["ca_wk"] * S_W)),
        "wv_ca": fp8q(_pack_wT(inputs["ca_wv"] * S_W)),
        "wo_ca": fp8q(_pack_wT(inputs["ca_wo"] * S_W)),
        "w1": _pack_wT(inputs["ffn_w1"], BF16),
        "w2": _pack_wT(inputs["ffn_w2"], BF16),
        "biases": np.concatenate([
            _pack_bias(inputs["sa_bq"] * (s * A)),
            _pack_bias(inputs["sa_bk"] * A),
            _pack_bias(inputs["sa_bo"] + inputs["sa_wo"] @ inputs["sa_bv"]),
            _pack_bias(inputs["ca_bq"] * (s * A)),
            _pack_bias(inputs["ca_bk"] * A),
            _pack_bias(inputs["ca_bo"] + inputs["ca_wo"] @ inputs["ca_bv"]),
            _pack_bias(inputs["ffn_b2"]),
            _pack_bias(inputs["sa_ln_b"]),
            _pack_bias(inputs["ca_ln_b"]),
            _pack_bias(inputs["ffn_ln_b"]),
            _pack_bias(inputs["ffn_b1"]),
        ], axis=1),
        "g1": np.ascontiguousarray(inputs["sa_ln_g"].reshape(1, D)).astype(BF16),
        "g2": np.ascontiguousarray(inputs["ca_ln_g"].reshape(1, D)).astype(BF16),
        "g3": np.ascontiguousarray(inputs["ffn_ln_g"].reshape(1, D)).astype(BF16),
    }

    exts = EXT_CAUSAL if sa_mode == "causal" else [TC_SA] * NL
    in_maps = []
    for c in range(8):
        b, i = c // 4, c % 4
        blocks = core_blocks(i)
        own_rows = np.concatenate([np.arange(p * LW, (p + 1) * LW) for p in blocks])
        xTp32 = _pack_xT(inputs["hidden_states"][b])
        m = dict(shared)
        m["xT"] = fp8q(xTp32 * S_X)
        m["xTown"] = np.ascontiguousarray(xTp32[:, :, own_rows])
        m["xTownb"] = fp8q(m["xTown"] * S_X)
        m["encT"] = fp8q(_pack_xT(inputs["encoder_hidden_states"][b]) * S_X)
        if sa_mode == "causal":
            em = np.empty((P, NL, MREG, LW), dtype=BF16)
            for j, pblk in enumerate(blocks):
                rows = slice(pblk * LW, (pblk + 1) * LW)
                t0 = (exts[j] - MREG) * P
                blk = np.exp(np.minimum(mask[rows, t0:t0 + MREG * P], 60.0))
                em[:, j] = blk.reshape(LW, MREG, P).transpose(2, 1, 0)
            m["expm"] = em
        elif sa_mode == "generic":
            em = np.empty((P, TC_SA * NL, LW), dtype=BF16)
            for j, pblk in enumerate(blocks):
                rows = slice(pblk * LW, (pblk + 1) * LW)
                blk = np.exp(np.minimum(mask[rows, :], 60.0))
                em[:, j::NL, :] = blk.reshape(LW, TC_SA, P).transpose(2, 1, 0)
            m["expm"] = em
        if ca_mode == "generic":
            em = np.empty((P, TC_CA, LTOT), dtype=BF16)
            for j, pblk in enumerate(blocks):
                rows = slice(pblk * LW, (pblk + 1) * LW)
                blk = np.exp(np.minimum(cmask[rows, :], 60.0))
                em[:, :, j * LW:(j + 1) * LW] = blk.reshape(LW, TC_CA, P).transpose(2, 1, 0)
            m["expmc"] = em
        in_maps.append(m)
    return in_maps, sa_mode, ca_mode


def assemble_output(results):
    out = np.zeros((B, L, D), np.float32)
    for c in range(8):
        b, i = c // 4, c % 4
        arr = np.asarray(results[c]["out"])  # [P, KC, LTOT]
        for j, pblk in enumerate(core_blocks(i)):
            blk = arr[:, :, j * LW:(j + 1) * LW]          # [P, KC, LW]
            out[b, pblk * LW:(pblk + 1) * LW, :] = blk.transpose(2, 1, 0).reshape(LW, D)
    return out


# ---------------------------------------------------------------------------
# Entry point
# ---------------------------------------------------------------------------

_NC_CACHE = {}


def get_nc(sa_mode, ca_mode):
    key = (sa_mode, ca_mode)
    if key not in _NC_CACHE:
        _NC_CACHE[key] = build_nc(sa_mode, ca_mode)
    return _NC_CACHE[key]


def _install_ntff_hook():
    """bass_utils' trace path needs antenv.axon_hooks, absent in this image.
    Inject a shim and register the ctypes-based NTFF hook from trn_agent_boot."""
    import types
    if "antenv.axon_hooks" in sys.modules:
        return
    holder = {}
    mod = types.ModuleType("antenv.axon_hooks")
    mod.set_axon_ntff_profile_hook = lambda h: holder.__setitem__("h", h)
    mod.get_axon_ntff_profile_hook = lambda: holder.get("h")
    sys.modules["antenv.axon_hooks"] = mod
    try:
        import antenv
        antenv.axon_hooks = mod
    except ImportError:
        pass
    try:
        from trn_agent_boot.trn_boot import _ntff_profile_via_ctypes
        hook = _ntff_profile_via_ctypes("/opt/axon/libaxon_pjrt.so")
        if hook is not None:
            mod.set_axon_ntff_profile_hook(hook)
    except Exception as e:  # degrade to no tracing
        print(f"ntff hook install failed: {e}", file=sys.stderr)


def run(inputs, trace=False):
    _install_ntff_hook()
    from concourse.bass_utils import run_bass_kernel_spmd
    in_maps, sa_mode, ca_mode = make_in_maps(inputs)
    nc = get_nc(sa_mode, ca_mode)
    res = run_bass_kernel_spmd(nc, in_maps, core_ids=list(range(8)), trace=trace)
    return assemble_output(res.results), res


def kernel(**inputs):
    out, _ = run(inputs, trace=False)
    return out



# revision 49
# speedup vs baseline: 1.0681x; 1.0224x over previous
"""Trainium2 Bass kernel for nn_ConicaLayer (transformer decoder layer:
self-attn (causal) + cross-attn + FFN, post-LN residuals).

Sharding: rows (B x L) split across 8 cores; core c -> batch b=c//4, and 4
interleaved 128-row blocks {i, 7-i, 8+i, 15-i} of the 16 blocks of that batch
(balances causal attention work). Each core computes full K/V for its batch.

All transposes/packing are done host-side; on-device activations stay d-major
([D, tokens]) end-to-end. Softmax uses exp(s-20) without max-subtraction
(scores are bounded; masked entries handled by multiplying exp(mask), with
fully-masked blocks skipped in causal mode). The V matrix carries an appended
ones column so PV matmuls also produce softmax denominators. V-bias folds into
the out-proj bias host-side (bo_eff = bo + wo @ bv, exact since sum(p) = 1).

Dtypes: bf16 for kv-path (xT/encT, wk/wv, k/v/q/e), attention out + wo, f1 +
w2. f32 for residual stream, wq/w1, layernorms, all PSUM accumulation.
"""

import sys
import numpy as np

try:
    import concourse.bass as bass  # noqa: F401
except ImportError:
    sys.path.insert(0, "/opt/trn_rl_repo")

import ml_dtypes
import concourse.bass as bass
import concourse.bacc as bacc
import concourse.tile as tile
from concourse import mybir
from concourse.bass import ts

BF16 = ml_dtypes.bfloat16

P = 128
B, L, S, D, H, DFF = 2, 2048, 1024, 1024, 16, 4096
DH = D // H           # 64
KC = D // P           # 8
KC2 = DFF // P        # 32
NL = 4                # l-blocks per core
LW = 128              # l width per block
LTOT = NL * LW        # 512 rows per core
TC_SA = L // P        # 16 t-chunks (self attn)
TC_CA = S // P        # 8 t-chunks (cross attn)
MREG = 4              # masked tail chunks per block (causal mode)
EXT_CAUSAL = [4, 8, 12, 16]
EXP_SHIFT = -20.0
EPS = 1e-5

f32 = mybir.dt.float32
bf = mybir.dt.bfloat16
f8 = mybir.dt.float8e4
FP8 = mybir.dt.np(f8)
DR = mybir.MatmulPerfMode.DoubleRow
AF = mybir.ActivationFunctionType
OP = mybir.AluOpType

# fp8 scaling: activations x16, weights x512. q/k carry the combined factor
# A into SBUF (descaled for free in the exp's scale arg); v carries A into
# the PV output (descaled by dividing wo by A host-side).
S_X = 16.0
S_W = 512.0
A_SC = S_X * S_W
INV_A2 = 1.0 / (A_SC * A_SC)
INV_A = 1.0 / A_SC
INV_W = 1.0 / S_W
S_O = 32.0
INV_OW = 1.0 / (S_O * S_W)


def core_blocks(i):
    return [i, 7 - i, 8 + i, 15 - i]


# ---------------------------------------------------------------------------
# Bass kernel builder
# ---------------------------------------------------------------------------

def build_nc(sa_mode, ca_mode):
    """sa_mode: 'causal' | 'zeros' | 'generic'; ca_mode: 'zeros' | 'generic'."""
    nc = bacc.Bacc("TRN2", target_bir_lowering=False, debug=False, num_devices=8)

    def din(name, shape, dtype=f32):
        return nc.dram_tensor(name, list(shape), dtype, kind="ExternalInput").ap()

    env = {}
    env["sa_mode"], env["ca_mode"] = sa_mode, ca_mode
    env["exts"] = EXT_CAUSAL if sa_mode == "causal" else [TC_SA] * NL
    env["xT_d"] = din("xT", [P, KC, L], f8)
    env["xTown_d"] = din("xTown", [P, KC, LTOT])
    env["xTownb_d"] = din("xTownb", [P, KC, LTOT], f8)
    env["encT_d"] = din("encT", [P, KC, S], f8)
    wdt = {"wq_sa": f8, "wk_sa": f8, "wv_sa": f8, "wo_sa": f8,
           "wq_ca": f8, "wk_ca": f8, "wv_ca": f8, "wo_ca": f8}
    env["wd"] = {n: din(n, [P, KC, D], dt) for n, dt in wdt.items()}
    env["w1_d"] = din("w1", [P, KC, DFF], bf)
    env["w2_d"] = din("w2", [P, KC2, D], bf)
    env["biases_d"] = din("biases", [P, 10 * KC + KC2])
    env["gd"] = {n: din(n, [1, D], bf) for n in ["g1", "g2", "g3"]}
    env["expm_d"] = None
    if sa_mode == "causal":
        env["expm_d"] = din("expm", [P, NL, MREG, LW], bf)
    elif sa_mode == "generic":
        env["expm_d"] = din("expm", [P, TC_SA * NL, LW], bf)
    env["expmc_d"] = din("expmc", [P, TC_CA, LTOT], bf) if ca_mode == "generic" else None
    env["out_d"] = nc.dram_tensor("out", [P, KC, LTOT], f32, kind="ExternalOutput").ap()

    with tile.TileContext(nc) as tc:
        _build_body(nc, tc, env)
    nc.compile()
    return nc


def _build_body(nc, tc, env):
    from contextlib import ExitStack

    xT_d, xTown_d, encT_d = env["xT_d"], env["xTown_d"], env["encT_d"]
    xTownb_d = env["xTownb_d"]
    wd, w1_d, w2_d, biases_d, gd = env["wd"], env["w1_d"], env["w2_d"], env["biases_d"], env["gd"]
    expm_d, expmc_d, out_d = env["expm_d"], env["expmc_d"], env["out_d"]
    sa_mode, ca_mode, exts = env["sa_mode"], env["ca_mode"], env["exts"]

    with ExitStack() as ctx:
        consts = ctx.enter_context(tc.tile_pool(name="consts", bufs=1))

        # ---- constants (tiles allocated now; DMAs deferred via load_consts
        # so the startup DMA queue serves the k-proj inputs first) ----
        biases_sb = consts.tile([P, 10 * KC + KC2], f32, tag="c_bias")
        bias_names = ["bq_sa", "bk_sa", "bo_sa", "bq_ca", "bk_ca", "bo_ca",
                      "b2", "lb1", "lb2", "lb3"]
        bias_sb = {n: biases_sb[:, i * KC:(i + 1) * KC]
                   for i, n in enumerate(bias_names)}
        b1_sb = biases_sb[:, 10 * KC:10 * KC + KC2]
        ones128b = consts.tile([P, 1], bf, tag="ones128b")
        nc.vector.memset(ones128b, 1.0)
        ones64b = consts.tile([1, DH], bf, tag="ones64b")
        nc.vector.memset(ones64b, 1.0)
        eps_sb = consts.tile([1, 1], f32, tag="eps")
        nc.vector.memset(eps_sb, EPS)
        zero128 = consts.tile([P, 1], f32, tag="zero128")
        nc.vector.memset(zero128, 0.0)
        shift128 = consts.tile([P, 1], f32, tag="shift128")
        nc.vector.memset(shift128, EXP_SHIFT)
        expm_sb = None
        if expm_d is not None:
            shape = [P, NL, MREG, LW] if sa_mode == "causal" else [P, TC_SA * NL, LW]
            expm_sb = consts.tile(shape, bf, tag="expm")
        expmc_sb = None
        if expmc_d is not None:
            expmc_sb = consts.tile([P, TC_CA, LTOT], bf, tag="expmc")

        def load_consts():
            nc.sync.dma_start(biases_sb, biases_d)
            if expm_sb is not None:
                nc.sync.dma_start(expm_sb, expm_d)
            if expmc_sb is not None:
                nc.sync.dma_start(expmc_sb, expmc_d)

        # ------------- helpers -------------
        def proj_to(wpool, wtag, wdt, src_sb, w_dram, n_oc, evict, psum_proj, n_tt=1,
                    tt_width=LTOT, n_kc=KC, dr=False, wt0=None):
            for oc in range(n_oc):
                if oc == 0 and wt0 is not None:
                    wt = wt0
                else:
                    wt = wpool.tile([P, n_kc, P], wdt, tag=wtag)
                    nc.sync.dma_start(wt, w_dram[:, :, ts(oc, P)])
                for tt in range(n_tt):
                    ps = psum_proj.tile([P, tt_width], f32, tag="psproj")
                    if dr:
                        for k2 in range(n_kc // 2):
                            nc.tensor.matmul(
                                ps, wt[:, 2 * k2:2 * k2 + 2, :],
                                src_sb[:, 2 * k2:2 * k2 + 2, ts(tt, tt_width)],
                                start=(k2 == 0), stop=(k2 == n_kc // 2 - 1),
                                perf_mode=DR)
                    else:
                        for kc in range(n_kc):
                            nc.tensor.matmul(
                                ps, wt[:, kc, :], src_sb[:, kc, ts(tt, tt_width)],
                                start=(kc == 0), stop=(kc == n_kc - 1))
                    evict(oc, tt, ps)

        def layer_norm(tag, x_sb, g_dram, lb, out_sb, chunk_done=None, bf_out=None,
                       bf_scale=None):
            # Stats and per-token scale broadcasts run as bf16 matmuls (4x the
            # fp32 matmul rate); the residual/output path stays fp32.
            with ExitStack() as lctx:
                lnp = lctx.enter_context(tc.tile_pool(
                    name=f"lnp_{tag}", bufs=1 if sa_mode == "generic" else 2))
                lns = lctx.enter_context(tc.tile_pool(name=f"lns_{tag}", bufs=1))
                psum_st = lctx.enter_context(
                    tc.tile_pool(name=f"psum_lns_{tag}", bufs=1, space="PSUM"))
                psum_ln = lctx.enter_context(
                    tc.tile_pool(name=f"psum_ln_{tag}", bufs=2, space="PSUM"))
                g = lns.tile([1, D], bf, tag="ln_g")
                nc.sync.dma_start(g, g_dram)
                psx = psum_st.tile([1, LTOT], f32, tag="ln_sx")
                psx2 = psum_st.tile([1, LTOT], f32, tag="ln_sx2")
                for kc in range(KC):
                    xb = lnp.tile([P, LTOT], bf, tag="ln_xb")
                    nc.vector.tensor_copy(xb, x_sb[:, kc, :])
                    sq = lnp.tile([P, LTOT], bf, tag="ln_sq")
                    nc.scalar.activation(sq, x_sb[:, kc, :], AF.Square, bias=zero128)
                    nc.tensor.matmul(psx, ones128b, xb,
                                     start=(kc == 0), stop=(kc == KC - 1))
                    nc.tensor.matmul(psx2, ones128b, sq,
                                     start=(kc == 0), stop=(kc == KC - 1))
                mean = lns.tile([1, LTOT], f32, tag="ln_mean")
                nc.vector.tensor_scalar_mul(mean, psx, 1.0 / D)
                msq = lns.tile([1, LTOT], f32, tag="ln_msq")
                nc.vector.tensor_tensor(msq, mean, mean, OP.mult)
                var = lns.tile([1, LTOT], f32, tag="ln_var")
                nc.vector.scalar_tensor_tensor(var, psx2, 1.0 / D, msq, OP.mult, OP.subtract)
                std = lns.tile([1, LTOT], f32, tag="ln_std")
                nc.scalar.activation(std, var, AF.Sqrt, bias=eps_sb)
                rstd = lns.tile([1, LTOT], f32, tag="ln_rstd")
                nc.vector.reciprocal_approx_fast(rstd, std)
                rstd_b = lns.tile([1, LTOT], bf, tag="ln_rstdb")
                nc.vector.tensor_copy(rstd_b, rstd)
                mrstd_b = lns.tile([1, LTOT], bf, tag="ln_mrstdb")
                nc.vector.tensor_tensor(mrstd_b, mean, rstd, OP.mult)
                for kc in range(KC):
                    pg = psum_ln.tile([P, LTOT], f32, tag="ln_pg")
                    pm = psum_ln.tile([P, LTOT], f32, tag="ln_pm")
                    nc.tensor.matmul(pg, g[:, ts(kc, P)], rstd_b, start=True, stop=True)
                    nc.tensor.matmul(pm, g[:, ts(kc, P)], mrstd_b, start=True, stop=True)
                    t = lnp.tile([P, LTOT], f32, tag="ln_t")
                    nc.vector.tensor_tensor(t, x_sb[:, kc, :], pg, OP.mult)
                    nc.vector.scalar_tensor_tensor(out_sb[:, kc, :], t, lb[:, kc:kc + 1],
                                                   pm, OP.add, OP.subtract)
                    if bf_out is not None:
                        if bf_scale is not None:
                            nc.vector.tensor_scalar_mul(bf_out[:, kc, :],
                                                        out_sb[:, kc, :], bf_scale)
                        else:
                            nc.vector.tensor_copy(bf_out[:, kc, :], out_sb[:, kc, :])
                    if chunk_done is not None:
                        chunk_done(kc)

        def kv_proj_v(vpool_dst, src_sb, wv_dram, wrhs, wrhs_tag, n_tc, psum_proj):
            for half in range(2):
                wvh = wrhs.tile([P, KC, 512], f8, tag=wrhs_tag)
                nc.sync.dma_start(wvh, wv_dram[:, :, ts(half, 512)])
                for tci in range(n_tc):
                    ps = psum_proj.tile([P, 512], f32, tag="psproj")
                    for k2 in range(KC // 2):
                        nc.tensor.matmul(
                            ps, src_sb[:, 2 * k2:2 * k2 + 2, ts(tci, P)],
                            wvh[:, 2 * k2:2 * k2 + 2, :],
                            start=(k2 == 0), stop=(k2 == KC // 2 - 1),
                            perf_mode=DR)
                    nc.vector.tensor_copy(
                        vpool_dst[:, tci, half * 8:(half + 1) * 8, 0:DH],
                        ps.rearrange("p (h d) -> p h d", h=8))

        def den_recip(den_row, at_pool, inner):
            # den_row: [1, *inner] PSUM slice holding softmax denominators
            sums = at_pool.tile([1] + inner, f32, tag="at_sums")
            nc.vector.tensor_copy(sums, den_row)
            recip = at_pool.tile([1] + inner, f32, tag="at_recip")
            nc.vector.reciprocal_approx_fast(recip, sums)
            recip_b = at_pool.tile([1] + inner, bf, tag="at_recipb")
            nc.vector.tensor_scalar_mul(recip_b, recip, S_O / A_SC)
            return recip_b

        def apply_norm(dst, pv_rows, recip_b, at_pool, psum_bc, inner):
            # dst = pv_rows * broadcast(recip)
            pb = psum_bc.tile([DH] + inner, f32, tag="bc")
            nc.tensor.matmul(pb, ones64b, recip_b, start=True, stop=True)
            bc_sb = at_pool.tile([DH] + inner, f32, tag="at_bc")
            nc.vector.tensor_copy(bc_sb, pb)
            if len(inner) == 2:
                dst = dst.rearrange("p (j l) -> p j l", j=inner[0])
            nc.vector.scalar_tensor_tensor(dst, pv_rows, 0.0, bc_sb,
                                           OP.bypass, OP.mult)

        # =================== SA ===================
        with ExitStack() as sctx:
            sa_pool = sctx.enter_context(tc.tile_pool(name="sa", bufs=1))
            kT_sb = sa_pool.tile([P, KC, L], bf, tag="kT")
            v_sb = sa_pool.tile([P, TC_SA, H, DH + 1], bf, tag="v")
            qT_sb = sa_pool.tile([P, KC, LTOT], bf, tag="qT")
            nc.gpsimd.memset(v_sb[:, :, :, DH:DH + 1], 1.0)

            with ExitStack() as xctx:
                xpool = xctx.enter_context(tc.tile_pool(name="xpool", bufs=1))
                wrhs = xctx.enter_context(tc.tile_pool(name="wrhs", bufs=2))
                wkp = xctx.enter_context(tc.tile_pool(name="wk_sa_p", bufs=3))
                psum_kv = xctx.enter_context(tc.tile_pool(name="psum_kv", bufs=4, space="PSUM"))
                # first weight tile and the xT chunks lead the DMA queue so
                # the k-proj can start ~4us in; const loads queue behind them.
                wt0k = wkp.tile([P, KC, P], f8, tag="wtb")
                nc.sync.dma_start(wt0k, wd["wk_sa"][:, :, ts(0, P)])
                xT_sb = xpool.tile([P, KC, L], f8, tag="xT")
                for kc in range(KC):
                    nc.sync.dma_start(xT_sb[:, kc, :], xT_d[:, kc, :])
                qsrc = xpool.tile([P, KC, LTOT], f8, tag="qsrc")
                nc.sync.dma_start(qsrc, xTownb_d)
                load_consts()

                def evk(oc, tt, ps):
                    nc.vector.tensor_scalar_add(kT_sb[:, oc, ts(tt, 512)], ps,
                                                bias_sb["bk_sa"][:, oc:oc + 1])
                proj_to(wkp, "wtb", f8, xT_sb, wd["wk_sa"], KC, evk, psum_kv,
                        n_tt=L // 512, tt_width=512, dr=True, wt0=wt0k)

                def evq(oc, tt, ps):
                    nc.vector.tensor_scalar_add(qT_sb[:, oc, :], ps,
                                                bias_sb["bq_sa"][:, oc:oc + 1])
                proj_to(wkp, "wtb", f8, qsrc, wd["wq_sa"], KC, evq, psum_kv, dr=True)

                kv_proj_v(v_sb, xT_sb, wd["wv_sa"], wrhs, "wrhs", TC_SA, psum_kv)

            ca_pool = ctx.enter_context(tc.tile_pool(name="ca", bufs=1, side="right"))
            kcT_sb = ca_pool.tile([P, KC, S], bf, tag="kcT")
            vc_sb = ca_pool.tile([P, TC_CA, H, DH + 1], bf, tag="vc")
            encT_sb = ca_pool.tile([P, KC, S], f8, tag="encT")
            for kc in range(KC):
                nc.sync.dma_start(encT_sb[:, kc, :], encT_d[:, kc, :])
            nc.gpsimd.memset(vc_sb[:, :, :, DH:DH + 1], 1.0)
            oT_sb = sctx.enter_context(tc.tile_pool(name="oT_sa", bufs=1)).tile(
                [P, KC, LTOT], f8, tag="oT")
            pre_pool = ctx.enter_context(tc.tile_pool(name="prep", bufs=1, side="right"))
            h1pre = pre_pool.tile([P, KC, LTOT], f32, tag="pre")
            nc.sync.dma_start(h1pre, xTown_d)

            with ExitStack() as actx:
                e_pool = actx.enter_context(tc.tile_pool(name="e_sa", bufs=2))
                at_pool = actx.enter_context(tc.tile_pool(name="at_sa", bufs=2))
                wkvc = actx.enter_context(tc.tile_pool(name="wkv_ca", bufs=2))
                wrhsc = actx.enter_context(tc.tile_pool(name="wrhs_ca", bufs=1))
                psum_s = actx.enter_context(tc.tile_pool(name="psum_s", bufs=2, space="PSUM"))
                psum_pv = actx.enter_context(tc.tile_pool(name="psum_pv", bufs=1, space="PSUM"))
                psum_bc = actx.enter_context(tc.tile_pool(name="psum_bc", bufs=1, space="PSUM"))
                psum_ckv = actx.enter_context(tc.tile_pool(name="psum_ckv", bufs=1, space="PSUM"))

                ca_state = {}

                def ca_kv_chunk(hc):
                    # 1/8 of CA k-proj and v-proj, emitted between SA head pairs
                    wt = wkvc.tile([P, KC, P], f8, tag="wt_ck")
                    nc.sync.dma_start(wt, wd["wk_ca"][:, :, ts(hc, P)])
                    for tt in range(S // 512):
                        ps = psum_ckv.tile([P, 512], f32, tag="ps_ckv")
                        for k2 in range(KC // 2):
                            nc.tensor.matmul(ps, wt[:, 2 * k2:2 * k2 + 2, :],
                                             encT_sb[:, 2 * k2:2 * k2 + 2, ts(tt, 512)],
                                             start=(k2 == 0), stop=(k2 == KC // 2 - 1),
                                             perf_mode=DR)
                        nc.vector.tensor_scalar_add(kcT_sb[:, hc, ts(tt, 512)], ps,
                                                    bias_sb["bk_ca"][:, hc:hc + 1])
                    half, tq = hc // 4, hc % 4
                    if tq == 0:
                        wvh_new = wrhsc.tile([P, KC, 512], f8, tag="wv_ca")
                        ca_state["wvh"] = wvh_new
                        nc.sync.dma_start(wvh_new, wd["wv_ca"][:, :, ts(half, 512)])
                    wvh = ca_state["wvh"]
                    for tci in (2 * tq, 2 * tq + 1):
                        ps = psum_ckv.tile([P, 512], f32, tag="ps_ckv")
                        for k2 in range(KC // 2):
                            nc.tensor.matmul(ps, encT_sb[:, 2 * k2:2 * k2 + 2, ts(tci, P)],
                                             wvh[:, 2 * k2:2 * k2 + 2, :],
                                             start=(k2 == 0), stop=(k2 == KC // 2 - 1),
                                             perf_mode=DR)
                        nc.vector.tensor_copy(
                            vc_sb[:, tci, half * 8:(half + 1) * 8, 0:DH],
                            ps.rearrange("p (h d) -> p h d", h=8))

                # compact slot layout over (tc, j>=jmin(tc)); causal skips j<tc//4
                jmin = [(tci // 4 if sa_mode == "causal" else 0) for tci in range(TC_SA)]
                bases = []
                nslot = 0
                for tci in range(TC_SA):
                    bases.append(nslot)
                    nslot += NL - jmin[tci]

                for hc in range(KC):  # head pair (2*hc, 2*hc+1)
                    pv = psum_pv.tile([DH + 1, 2, NL, LW], f32, tag="pv")
                    e0 = e_pool.tile([P, nslot, LW], bf, tag="e_sa")
                    e1 = e_pool.tile([P, nslot, LW], bf, tag="e_sa")

                    def sa_scores(g0):
                        jm = jmin[g0]
                        N = (NL - jm) * LW
                        ps0 = psum_s.tile([P, 2, NL * LW], f32, tag="ps_sa")
                        ps1 = psum_s.tile([P, 2, NL * LW], f32, tag="ps_sa")
                        for u in range(2):
                            tci = g0 + u
                            loff = jm * LW
                            nc.tensor.matmul(
                                ps0[:, u, :N], kT_sb[0:DH, hc, ts(tci, P)],
                                qT_sb[0:DH, hc, loff:loff + N], start=True, stop=True)
                            nc.tensor.matmul(
                                ps1[:, u, :N], kT_sb[DH:P, hc, ts(tci, P)],
                                qT_sb[DH:P, hc, loff:loff + N], start=True, stop=True)
                        nsl = 2 * (NL - jm)
                        eo0 = e0[:, bases[g0]:bases[g0] + nsl, :].rearrange(
                            "p (u j) l -> p u j l", u=2)
                        eo1 = e1[:, bases[g0]:bases[g0] + nsl, :].rearrange(
                            "p (u j) l -> p u j l", u=2)
                        nc.scalar.activation(
                            eo0, ps0[:, :, :N].rearrange("p u (j l) -> p u j l", l=LW),
                            AF.Exp, bias=shift128, scale=INV_A2)
                        nc.scalar.activation(
                            eo1, ps1[:, :, :N].rearrange("p u (j l) -> p u j l", l=LW),
                            AF.Exp, bias=shift128, scale=INV_A2)

                    def sa_mask(j):
                        w = NL - j  # slot stride across the 4 diagonal chunks
                        for e_sb in (e0, e1):
                            view = e_sb[:, bases[4 * j]:bases[4 * j] + MREG * w, :]
                            view = view.rearrange("p (t w) l -> p t w l", w=w)[:, :, 0, :]
                            nc.vector.tensor_tensor(view, view,
                                                    expm_sb[:, j, :, :], OP.mult)

                    def sa_pv(tlo, thi):
                        for tci in range(tlo, thi):
                            jm = jmin[tci]
                            nc.tensor.matmul(
                                pv[:, 0, jm:, :], v_sb[:, tci, 2 * hc, :],
                                e0[:, bases[tci]:bases[tci] + NL - jm, :],
                                start=(tci == 0), stop=(tci == TC_SA - 1),
                                skip_group_check=True)
                            nc.tensor.matmul(
                                pv[:, 1, jm:, :], v_sb[:, tci, 2 * hc + 1, :],
                                e1[:, bases[tci]:bases[tci] + NL - jm, :],
                                start=(tci == 0), stop=(tci == TC_SA - 1),
                                skip_group_check=True)

                    if sa_mode == "causal":
                        # Interleave the PV chains one block-group behind the
                        # scores/exp so TensorE rides through the exp latency.
                        for j in range(NL):
                            sa_scores(4 * j)
                            if j > 0:
                                sa_pv(4 * (j - 1), 4 * j)
                            sa_scores(4 * j + 2)
                            sa_mask(j)
                        sa_pv(4 * (NL - 1), TC_SA)
                    else:
                        for g0 in range(0, TC_SA, 2):
                            sa_scores(g0)
                        if sa_mode == "generic":
                            nc.vector.tensor_tensor(e0, e0, expm_sb, OP.mult)
                            nc.vector.tensor_tensor(e1, e1, expm_sb, OP.mult)
                        sa_pv(0, TC_SA)
                    # reciprocal of the denominators runs on DVE while the
                    # interleaved CA kv-projection chunk keeps TensorE busy;
                    # the broadcast matmuls then find it ready.
                    recs = [den_recip(pv[DH:DH + 1, u], at_pool, [NL, LW])
                            for u in range(2)]
                    ca_kv_chunk(hc)
                    for u in range(2):
                        apply_norm(oT_sb[u * DH:(u + 1) * DH, hc, :],
                                   pv[0:DH, u], recs[u], at_pool, psum_bc,
                                   [NL, LW])

            with ExitStack() as octx:
                wop = octx.enter_context(tc.tile_pool(name="wo_sa_p", bufs=3))
                otp = octx.enter_context(tc.tile_pool(name="otmp_sa", bufs=2))
                psum_op = octx.enter_context(tc.tile_pool(name="psum_osa", bufs=4, space="PSUM"))

                def evo(oc, tt, ps):
                    # h1pre was pre-loaded with the residual (xTown); descale
                    # the fp8 o-proj on ScalarE, then accumulate in place
                    tmp = otp.tile([P, LTOT], f32, tag="otmp")
                    nc.scalar.activation(tmp, ps, AF.Identity,
                                         bias=bias_sb["bo_sa"][:, oc:oc + 1],
                                         scale=INV_OW)
                    nc.vector.tensor_tensor(h1pre[:, oc, :], tmp,
                                            h1pre[:, oc, :], OP.add)
                proj_to(wop, "wtb", f8, oT_sb, wd["wo_sa"], KC, evo, psum_op,
                        dr=True)

            h1_pool = ctx.enter_context(tc.tile_pool(name="h1p", bufs=1, side="right"))
            h1_sb = h1_pool.tile([P, KC, LTOT], f32, tag="h1")
            bfp = ctx.enter_context(tc.tile_pool(name="bfcast", bufs=1, side="right"))
            h1bf = bfp.tile([P, KC, LTOT], f8, tag="bfx")
            layer_norm("ln1", h1pre, gd["g1"], bias_sb["lb1"], h1_sb, bf_out=h1bf,
                       bf_scale=S_X)

        # =================== CA ===================
        with ExitStack() as cctx:
            qcT_sb = cctx.enter_context(tc.tile_pool(name="qc_ca", bufs=1)).tile(
                [P, KC, LTOT], bf, tag="qcT")

            with ExitStack() as xctx:
                wkp = xctx.enter_context(tc.tile_pool(name="wk_ca_p", bufs=3))
                psum_kv = xctx.enter_context(tc.tile_pool(name="psum_cq", bufs=4, space="PSUM"))

                def evqc(oc, tt, ps):
                    nc.vector.tensor_scalar_add(qcT_sb[:, oc, :], ps,
                                                bias_sb["bq_ca"][:, oc:oc + 1])
                proj_to(wkp, "wtb", f8, h1bf, wd["wq_ca"], KC, evqc, psum_kv,
                        dr=True)

            ocT_sb = cctx.enter_context(tc.tile_pool(name="oT_ca", bufs=1)).tile(
                [P, KC, LTOT], f8, tag="ocT")
            h2pre = pre_pool.tile([P, KC, LTOT], f32, tag="pre")

            with ExitStack() as actx:
                e_pool = actx.enter_context(tc.tile_pool(name="e_ca", bufs=4))
                at_pool = actx.enter_context(tc.tile_pool(name="at_ca", bufs=3))
                psum_s = actx.enter_context(tc.tile_pool(name="psum_cs", bufs=2, space="PSUM"))
                psum_pv = actx.enter_context(tc.tile_pool(name="psum_cpv", bufs=3, space="PSUM"))
                psum_bc = actx.enter_context(tc.tile_pool(name="psum_cbc", bufs=1, space="PSUM"))

                def flush_ca(st):
                    hc, pvu0, rec0, pvu1, rec1 = st
                    apply_norm(ocT_sb[0:DH, hc, :], pvu0[0:DH, :], rec0,
                               at_pool, psum_bc, [LTOT])
                    apply_norm(ocT_sb[DH:P, hc, :], pvu1[0:DH, :], rec1,
                               at_pool, psum_bc, [LTOT])

                # normalize for head pair hc-1 is deferred to between the two
                # PV chains of pair hc so its broadcast matmuls never stall
                # the tensor queue waiting on the DVE reciprocal.
                prev = None
                for hc in range(KC):  # head pair (2*hc, 2*hc+1)
                    ec0 = e_pool.tile([P, TC_CA, LTOT], bf, tag="ec")
                    ec1 = e_pool.tile([P, TC_CA, LTOT], bf, tag="ec")

                    def ca_scores(g0):
                        cs0 = psum_s.tile([P, 2, LTOT], f32, tag="cs")
                        cs1 = psum_s.tile([P, 2, LTOT], f32, tag="cs")
                        for u in range(2):
                            tci = g0 + u
                            nc.tensor.matmul(cs0[:, u, :],
                                             kcT_sb[0:DH, hc, ts(tci, P)],
                                             qcT_sb[0:DH, hc, :],
                                             start=True, stop=True)
                            nc.tensor.matmul(cs1[:, u, :],
                                             kcT_sb[DH:P, hc, ts(tci, P)],
                                             qcT_sb[DH:P, hc, :],
                                             start=True, stop=True)
                        nc.scalar.activation(ec0[:, g0:g0 + 2, :], cs0, AF.Exp,
                                             bias=shift128, scale=INV_A2)
                        nc.scalar.activation(ec1[:, g0:g0 + 2, :], cs1, AF.Exp,
                                             bias=shift128, scale=INV_A2)

                    def ca_pv(tlo, thi):
                        for tci in range(tlo, thi):
                            nc.tensor.matmul(pvc0, vc_sb[:, tci, 2 * hc, :],
                                             ec0[:, tci, :],
                                             start=(tci == 0), stop=(tci == TC_CA - 1),
                                             skip_group_check=True)
                            nc.tensor.matmul(pvc1, vc_sb[:, tci, 2 * hc + 1, :],
                                             ec1[:, tci, :],
                                             start=(tci == 0), stop=(tci == TC_CA - 1),
                                             skip_group_check=True)

                    if ca_mode == "generic":
                        # masks force a full-e barrier; keep the simple order
                        for g0 in range(0, TC_CA, 2):
                            ca_scores(g0)
                        nc.vector.tensor_tensor(ec0, ec0, expmc_sb, OP.mult)
                        nc.vector.tensor_tensor(ec1, ec1, expmc_sb, OP.mult)
                        pvc0 = psum_pv.tile([DH + 1, LTOT], f32, tag="pvc")
                        pvc1 = psum_pv.tile([DH + 1, LTOT], f32, tag="pvc")
                        if prev is not None:
                            flush_ca(prev)
                        ca_pv(0, TC_CA)
                    else:
                        # pipeline: pv lags scores by one pair-group; the
                        # deferred normalize of hc-1 fills the first slot.
                        ca_scores(0)
                        pvc0 = psum_pv.tile([DH + 1, LTOT], f32, tag="pvc")
                        pvc1 = psum_pv.tile([DH + 1, LTOT], f32, tag="pvc")
                        if prev is not None:
                            flush_ca(prev)
                        for g0 in range(2, TC_CA, 2):
                            ca_scores(g0)
                            ca_pv(g0 - 2, g0)
                        ca_pv(TC_CA - 2, TC_CA)
                    rec0 = den_recip(pvc0[DH:DH + 1, :], at_pool, [LTOT])
                    rec1 = den_recip(pvc1[DH:DH + 1, :], at_pool, [LTOT])
                    prev = (hc, pvc0, rec0, pvc1, rec1)
                flush_ca(prev)

            with ExitStack() as octx:
                wop = octx.enter_context(tc.tile_pool(name="wo_ca_p", bufs=3))
                otp = octx.enter_context(tc.tile_pool(name="otmp_ca", bufs=2))
                psum_op = octx.enter_context(tc.tile_pool(name="psum_oca", bufs=4, space="PSUM"))

                def evoc(oc, tt, ps):
                    tmp = otp.tile([P, LTOT], f32, tag="otmp")
                    nc.scalar.activation(tmp, ps, AF.Identity,
                                         bias=bias_sb["bo_ca"][:, oc:oc + 1],
                                         scale=INV_OW)
                    nc.vector.tensor_tensor(h2pre[:, oc, :], tmp,
                                            h1_sb[:, oc, :], OP.add)
                proj_to(wop, "wtb", f8, ocT_sb, wd["wo_ca"], KC, evoc, psum_op,
                        dr=True)

            h2_pool = ctx.enter_context(tc.tile_pool(name="h2p", bufs=1, side="right"))
            h2_sb = h2_pool.tile([P, KC, LTOT], f32, tag="h2")
            h2bf = bfp.tile([P, KC, LTOT], bf, tag="bfx")
            layer_norm("ln2", h2pre, gd["g2"], bias_sb["lb2"], h2_sb, bf_out=h2bf)

        # =================== FFN ===================
        with ExitStack() as fctx:
            ffn_pool = fctx.enter_context(tc.tile_pool(name="ffn", bufs=1))
            w2pool = fctx.enter_context(tc.tile_pool(name="wtile32", bufs=2))
            w1pool = fctx.enter_context(tc.tile_pool(name="w1p", bufs=3))
            psum_f = fctx.enter_context(tc.tile_pool(name="psum_f", bufs=4, space="PSUM"))
            f1_sb = ffn_pool.tile([P, KC2, LTOT], bf, tag="f1")
            h3pre = pre_pool.tile([P, KC, LTOT], f32, tag="pre")

            def evg(oc, tt, ps):
                nc.scalar.activation(f1_sb[:, oc, :], ps, AF.Gelu,
                                     bias=b1_sb[:, oc:oc + 1])
            proj_to(w1pool, "wtb", bf, h2bf, w1_d, KC2, evg, psum_f)

            for oc in range(KC):
                w2t = w2pool.tile([P, KC2, P], bf, tag="w2t")
                nc.sync.dma_start(w2t, w2_d[:, :, ts(oc, P)])
                ps = psum_f.tile([P, LTOT], f32, tag="psproj")
                for kc in range(KC2):
                    nc.tensor.matmul(ps, w2t[:, kc, :], f1_sb[:, kc, :],
                                     start=(kc == 0), stop=(kc == KC2 - 1))
                nc.vector.scalar_tensor_tensor(
                    h3pre[:, oc, :], ps, bias_sb["b2"][:, oc:oc + 1],
                    h2_sb[:, oc, :], OP.add, OP.add)

        out_sb = h1_pool.tile([P, KC, LTOT], f32, tag="h1")
        layer_norm("ln3", h3pre, gd["g3"], bias_sb["lb3"], out_sb,
                   chunk_done=lambda kc: nc.sync.dma_start(out_d[:, kc, :],
                                                           out_sb[:, kc, :]))


# ---------------------------------------------------------------------------
# Host-side packing
# ---------------------------------------------------------------------------

def _pack_wT(w, dtype=np.float32):
    # w: [dout, din] -> [P, din//P, dout] with wT[d, o] layout
    din = w.shape[1]
    return np.ascontiguousarray(
        w.T.reshape(din // P, P, w.shape[0]).transpose(1, 0, 2)).astype(dtype)


def _pack_xT(x, dtype=np.float32):
    # x: [T, D] -> [P, KC, T]
    t = x.shape[0]
    return np.ascontiguousarray(x.T.reshape(KC, P, t).transpose(1, 0, 2)).astype(dtype)


def _pack_bias(v):
    n = v.shape[0] // P
    return np.ascontiguousarray(v.reshape(n, P).T).astype(np.float32)


def detect_sa_mode(mask):
    if not np.isfinite(np.nan_to_num(mask, nan=np.inf)).all():
        return "generic"
    if (mask == 0).all():
        return "zeros"
    li, ti = np.tril_indices(L)
    if (mask[li, ti] == 0).all():
        ui, uj = np.triu_indices(L, k=1)
        if (mask[ui, uj] <= -1e8).all():
            return "causal"
    return "generic"


def make_in_maps(inputs):
    inputs = {k: np.asarray(v, dtype=np.float32) for k, v in inputs.items()}
    mask = inputs["attention_mask"]
    cmask = inputs["encoder_attention_mask"]
    sa_mode = detect_sa_mode(mask)
    ca_mode = "zeros" if (cmask == 0).all() else "generic"
    s = DH ** -0.5

    def fp8q(arr):
        return np.clip(arr, -240.0, 240.0).astype(FP8)

    A = S_X * S_W
    shared = {
        "wq_sa": fp8q(_pack_wT(inputs["sa_wq"] * (s * S_W))),
        "wk_sa": fp8q(_pack_wT(inputs["sa_wk"] * S_W)),
        "wv_sa": fp8q(_pack_wT(inputs["sa_wv"] * S_W)),
        "wo_sa": fp8q(_pack_wT(inputs["sa_wo"] * S_W)),
        "wq_ca": fp8q(_pack_wT(inputs["ca_wq"] * (s * S_W))),
        "wk_ca": fp8q(_pack_wT(inputs["ca_wk"] * S_W)),
        "wv_ca": fp8q(_pack_wT(inputs["ca_wv"] * S_W)),
        "wo_ca": fp8q(_pack_wT(inputs["ca_wo"] * S_W)),
        "w1": _pack_wT(inputs["ffn_w1"], BF16),
        "w2": _pack_wT(inputs["ffn_w2"], BF16),
        "biases": np.concatenate([
            _pack_bias(inputs["sa_bq"] * (s * A)),
            _pack_bias(inputs["sa_bk"] * A),
            _pack_bias(inputs["sa_bo"] + inputs["sa_wo"] @ inputs["sa_bv"]),
            _pack_bias(inputs["ca_bq"] * (s * A)),
            _pack_bias(inputs["ca_bk"] * A),
            _pack_bias(inputs["ca_bo"] + inputs["ca_wo"] @ inputs["ca_bv"]),
            _pack_bias(inputs["ffn_b2"]),
            _pack_bias(inputs["sa_ln_b"]),
            _pack_bias(inputs["ca_ln_b"]),
            _pack_bias(inputs["ffn_ln_b"]),
            _pack_bias(inputs["ffn_b1"]),
        ], axis=1),
        "g1": np.ascontiguousarray(inputs["sa_ln_g"].reshape(1, D)).astype(BF16),
        "g2": np.ascontiguousarray(inputs["ca_ln_g"].reshape(1, D)).astype(BF16),
        "g3": np.ascontiguousarray(inputs["ffn_ln_g"].reshape(1, D)).astype(BF16),
    }

    exts = EXT_CAUSAL if sa_mode == "causal" else [TC_SA] * NL
    in_maps = []
    for c in range(8):
        b, i = c // 4, c % 4
        blocks = core_blocks(i)
        own_rows = np.concatenate([np.arange(p * LW, (p + 1) * LW) for p in blocks])
        xTp32 = _pack_xT(inputs["hidden_states"][b])
        m = dict(shared)
        m["xT"] = fp8q(xTp32 * S_X)
        m["xTown"] = np.ascontiguousarray(xTp32[:, :, own_rows])
        m["xTownb"] = fp8q(m["xTown"] * S_X)
        m["encT"] = fp8q(_pack_xT(inputs["encoder_hidden_states"][b]) * S_X)
        if sa_mode == "causal":
            em = np.empty((P, NL, MREG, LW), dtype=BF16)
            for j, pblk in enumerate(blocks):
                rows = slice(pblk * LW, (pblk + 1) * LW)
                t0 = (exts[j] - MREG) * P
                blk = np.exp(np.minimum(mask[rows, t0:t0 + MREG * P], 60.0))
                em[:, j] = blk.reshape(LW, MREG, P).transpose(2, 1, 0)
            m["expm"] = em
        elif sa_mode == "generic":
            em = np.empty((P, TC_SA * NL, LW), dtype=BF16)
            for j, pblk in enumerate(blocks):
                rows = slice(pblk * LW, (pblk + 1) * LW)
                blk = np.exp(np.minimum(mask[rows, :], 60.0))
                em[:, j::NL, :] = blk.reshape(LW, TC_SA, P).transpose(2, 1, 0)
            m["expm"] = em
        if ca_mode == "generic":
            em = np.empty((P, TC_CA, LTOT), dtype=BF16)
            for j, pblk in enumerate(blocks):
                rows = slice(pblk * LW, (pblk + 1) * LW)
                blk = np.exp(np.minimum(cmask[rows, :], 60.0))
                em[:, :, j * LW:(j + 1) * LW] = blk.reshape(LW, TC_CA, P).transpose(2, 1, 0)
            m["expmc"] = em
        in_maps.append(m)
    return in_maps, sa_mode, ca_mode


def assemble_output(results):
    out = np.zeros((B, L, D), np.float32)
    for c in range(8):
        b, i = c // 4, c % 4
        arr = np.asarray(results[c]["out"])  # [P, KC, LTOT]
        for j, pblk in enumerate(core_blocks(i)):
            blk = arr[:, :, j * LW:(j + 1) * LW]          # [P, KC, LW]
            out[b, pblk * LW:(pblk + 1) * LW, :] = blk.transpose(2, 1, 0).reshape(LW, D)
    return out


# ---------------------------------------------------------------------------
# Entry point
# ---------------------------------------------------------------------------

_NC_CACHE = {}


def get_nc(sa_mode, ca_mode):
    key = (sa_mode, ca_mode)
    if key not in _NC_CACHE:
        _NC_CACHE[key] = build_nc(sa_mode, ca_mode)
    return _NC_CACHE[key]


def _install_ntff_hook():
    """bass_utils' trace path needs antenv.axon_hooks, absent in this image.
    Inject a shim and register the ctypes-based NTFF hook from trn_agent_boot."""
    import types
    if "antenv.axon_hooks" in sys.modules:
        return
    holder = {}
    mod = types.ModuleType("antenv.axon_hooks")
    mod.set_axon_ntff_profile_hook = lambda h: holder.__setitem__("h", h)
    mod.get_axon_ntff_profile_hook = lambda: holder.get("h")
    sys.modules["antenv.axon_hooks"] = mod
    try:
        import antenv
        antenv.axon_hooks = mod
    except ImportError:
        pass
    try:
        from trn_agent_boot.trn_boot import _ntff_profile_via_ctypes
        hook = _ntff_profile_via_ctypes("/opt/axon/libaxon_pjrt.so")
        if hook is not None:
            mod.set_axon_ntff_profile_hook(hook)
    except Exception as e:  # degrade to no tracing
        print(f"ntff hook install failed: {e}", file=sys.stderr)


def run(inputs, trace=False):
    _install_ntff_hook()
    from concourse.bass_utils import run_bass_kernel_spmd
    in_maps, sa_mode, ca_mode = make_in_maps(inputs)
    nc = get_nc(sa_mode, ca_mode)
    res = run_bass_kernel_spmd(nc, in_maps, core_ids=list(range(8)), trace=trace)
    return assemble_output(res.results), res


def kernel(**inputs):
    out, _ = run(inputs, trace=False)
    return out



# revision 50
# speedup vs baseline: 1.0816x; 1.0127x over previous
"""Trainium2 Bass kernel for nn_ConicaLayer (transformer decoder layer:
self-attn (causal) + cross-attn + FFN, post-LN residuals).

Sharding: rows (B x L) split across 8 cores; core c -> batch b=c//4, and 4
interleaved 128-row blocks {i, 7-i, 8+i, 15-i} of the 16 blocks of that batch
(balances causal attention work). Each core computes full K/V for its batch.

All transposes/packing are done host-side; on-device activations stay d-major
([D, tokens]) end-to-end. Softmax uses exp(s-20) without max-subtraction
(scores are bounded; masked entries handled by multiplying exp(mask), with
fully-masked blocks skipped in causal mode). The V matrix carries an appended
ones column so PV matmuls also produce softmax denominators. V-bias folds into
the out-proj bias host-side (bo_eff = bo + wo @ bv, exact since sum(p) = 1).

Dtypes: bf16 for kv-path (xT/encT, wk/wv, k/v/q/e), attention out + wo, f1 +
w2. f32 for residual stream, wq/w1, layernorms, all PSUM accumulation.
"""

import sys
import numpy as np

try:
    import concourse.bass as bass  # noqa: F401
except ImportError:
    sys.path.insert(0, "/opt/trn_rl_repo")

import ml_dtypes
import concourse.bass as bass
import concourse.bacc as bacc
import concourse.tile as tile
from concourse import mybir
from concourse.bass import ts

BF16 = ml_dtypes.bfloat16

P = 128
B, L, S, D, H, DFF = 2, 2048, 1024, 1024, 16, 4096
DH = D // H           # 64
KC = D // P           # 8
KC2 = DFF // P        # 32
NL = 4                # l-blocks per core
LW = 128              # l width per block
LTOT = NL * LW        # 512 rows per core
TC_SA = L // P        # 16 t-chunks (self attn)
TC_CA = S // P        # 8 t-chunks (cross attn)
MREG = 4              # masked tail chunks per block (causal mode)
EXT_CAUSAL = [4, 8, 12, 16]
EXP_SHIFT = -20.0
EPS = 1e-5

f32 = mybir.dt.float32
bf = mybir.dt.bfloat16
f8 = mybir.dt.float8e4
FP8 = mybir.dt.np(f8)
DR = mybir.MatmulPerfMode.DoubleRow
AF = mybir.ActivationFunctionType
OP = mybir.AluOpType

# fp8 scaling: activations x16, weights x512. q/k carry the combined factor
# A into SBUF (descaled for free in the exp's scale arg); v carries A into
# the PV output (descaled by dividing wo by A host-side).
S_X = 16.0
S_W = 512.0
A_SC = S_X * S_W
INV_A2 = 1.0 / (A_SC * A_SC)
INV_A = 1.0 / A_SC
INV_W = 1.0 / S_W
S_O = 32.0
INV_OW = 1.0 / (S_O * S_W)


def core_blocks(i):
    return [i, 7 - i, 8 + i, 15 - i]


# ---------------------------------------------------------------------------
# Bass kernel builder
# ---------------------------------------------------------------------------

def build_nc(sa_mode, ca_mode):
    """sa_mode: 'causal' | 'zeros' | 'generic'; ca_mode: 'zeros' | 'generic'."""
    nc = bacc.Bacc("TRN2", target_bir_lowering=False, debug=False, num_devices=8)

    def din(name, shape, dtype=f32):
        return nc.dram_tensor(name, list(shape), dtype, kind="ExternalInput").ap()

    env = {}
    env["sa_mode"], env["ca_mode"] = sa_mode, ca_mode
    env["exts"] = EXT_CAUSAL if sa_mode == "causal" else [TC_SA] * NL
    env["xT_d"] = din("xT", [P, KC, L], f8)
    env["xTown_d"] = din("xTown", [P, KC, LTOT])
    env["xTownb_d"] = din("xTownb", [P, KC, LTOT], f8)
    env["encT_d"] = din("encT", [P, KC, S], f8)
    wdt = {"wq_sa": f8, "wk_sa": f8, "wv_sa": f8, "wo_sa": f8,
           "wq_ca": f8, "wk_ca": f8, "wv_ca": f8, "wo_ca": f8}
    env["wd"] = {n: din(n, [P, KC, D], dt) for n, dt in wdt.items()}
    env["w1_d"] = din("w1", [P, KC, DFF], bf)
    env["w2_d"] = din("w2", [P, KC2, D], bf)
    env["biases_d"] = din("biases", [P, 10 * KC + KC2])
    env["gd"] = {n: din(n, [1, D], bf) for n in ["g1", "g2", "g3"]}
    env["expm_d"] = None
    if sa_mode == "causal":
        env["expm_d"] = din("expm", [P, NL, MREG, LW], bf)
    elif sa_mode == "generic":
        env["expm_d"] = din("expm", [P, TC_SA * NL, LW], bf)
    env["expmc_d"] = din("expmc", [P, TC_CA, LTOT], bf) if ca_mode == "generic" else None
    env["out_d"] = nc.dram_tensor("out", [P, KC, LTOT], f32, kind="ExternalOutput").ap()

    with tile.TileContext(nc) as tc:
        _build_body(nc, tc, env)
    nc.compile()
    return nc


def _build_body(nc, tc, env):
    from contextlib import ExitStack

    xT_d, xTown_d, encT_d = env["xT_d"], env["xTown_d"], env["encT_d"]
    xTownb_d = env["xTownb_d"]
    wd, w1_d, w2_d, biases_d, gd = env["wd"], env["w1_d"], env["w2_d"], env["biases_d"], env["gd"]
    expm_d, expmc_d, out_d = env["expm_d"], env["expmc_d"], env["out_d"]
    sa_mode, ca_mode, exts = env["sa_mode"], env["ca_mode"], env["exts"]

    with ExitStack() as ctx:
        consts = ctx.enter_context(tc.tile_pool(name="consts", bufs=1))

        # ---- constants (tiles allocated now; DMAs deferred via load_consts
        # so the startup DMA queue serves the k-proj inputs first) ----
        biases_sb = consts.tile([P, 10 * KC + KC2], f32, tag="c_bias")
        bias_names = ["bq_sa", "bk_sa", "bo_sa", "bq_ca", "bk_ca", "bo_ca",
                      "b2", "lb1", "lb2", "lb3"]
        bias_sb = {n: biases_sb[:, i * KC:(i + 1) * KC]
                   for i, n in enumerate(bias_names)}
        b1_sb = biases_sb[:, 10 * KC:10 * KC + KC2]
        ones128b = consts.tile([P, 1], bf, tag="ones128b")
        nc.vector.memset(ones128b, 1.0)
        ones64b = consts.tile([1, DH], bf, tag="ones64b")
        nc.vector.memset(ones64b, 1.0)
        eps_sb = consts.tile([1, 1], f32, tag="eps")
        nc.vector.memset(eps_sb, EPS)
        zero128 = consts.tile([P, 1], f32, tag="zero128")
        nc.vector.memset(zero128, 0.0)
        shift128 = consts.tile([P, 1], f32, tag="shift128")
        nc.vector.memset(shift128, EXP_SHIFT)
        expm_sb = None
        if expm_d is not None:
            shape = [P, NL, MREG, LW] if sa_mode == "causal" else [P, TC_SA * NL, LW]
            expm_sb = consts.tile(shape, bf, tag="expm")
        expmc_sb = None
        if expmc_d is not None:
            expmc_sb = consts.tile([P, TC_CA, LTOT], bf, tag="expmc")

        def load_consts():
            nc.sync.dma_start(biases_sb, biases_d)
            if expm_sb is not None:
                nc.sync.dma_start(expm_sb, expm_d)
            if expmc_sb is not None:
                nc.sync.dma_start(expmc_sb, expmc_d)

        # ------------- helpers -------------
        def proj_to(wpool, wtag, wdt, src_sb, w_dram, n_oc, evict, psum_proj, n_tt=1,
                    tt_width=LTOT, n_kc=KC, dr=False, wt0=None):
            for oc in range(n_oc):
                if oc == 0 and wt0 is not None:
                    wt = wt0
                else:
                    wt = wpool.tile([P, n_kc, P], wdt, tag=wtag)
                    nc.sync.dma_start(wt, w_dram[:, :, ts(oc, P)])
                for tt in range(n_tt):
                    ps = psum_proj.tile([P, tt_width], f32, tag="psproj")
                    if dr:
                        for k2 in range(n_kc // 2):
                            nc.tensor.matmul(
                                ps, wt[:, 2 * k2:2 * k2 + 2, :],
                                src_sb[:, 2 * k2:2 * k2 + 2, ts(tt, tt_width)],
                                start=(k2 == 0), stop=(k2 == n_kc // 2 - 1),
                                perf_mode=DR)
                    else:
                        for kc in range(n_kc):
                            nc.tensor.matmul(
                                ps, wt[:, kc, :], src_sb[:, kc, ts(tt, tt_width)],
                                start=(kc == 0), stop=(kc == n_kc - 1))
                    evict(oc, tt, ps)

        def layer_norm(tag, x_sb, g_dram, lb, out_sb, chunk_done=None, bf_out=None,
                       bf_scale=None):
            # Stats and per-token scale broadcasts run as bf16 matmuls (4x the
            # fp32 matmul rate); the residual/output path stays fp32.
            with ExitStack() as lctx:
                lnp = lctx.enter_context(tc.tile_pool(
                    name=f"lnp_{tag}", bufs=1 if sa_mode == "generic" else 2))
                lns = lctx.enter_context(tc.tile_pool(name=f"lns_{tag}", bufs=1))
                psum_st = lctx.enter_context(
                    tc.tile_pool(name=f"psum_lns_{tag}", bufs=1, space="PSUM"))
                psum_ln = lctx.enter_context(
                    tc.tile_pool(name=f"psum_ln_{tag}", bufs=3, space="PSUM"))
                g = lns.tile([1, D], bf, tag="ln_g")
                nc.sync.dma_start(g, g_dram)
                psx = psum_st.tile([1, LTOT], f32, tag="ln_sx")
                psx2 = psum_st.tile([1, LTOT], f32, tag="ln_sx2")
                for kc in range(KC):
                    xb = lnp.tile([P, LTOT], bf, tag="ln_xb")
                    nc.vector.tensor_copy(xb, x_sb[:, kc, :])
                    sq = lnp.tile([P, LTOT], bf, tag="ln_sq")
                    nc.scalar.activation(sq, x_sb[:, kc, :], AF.Square, bias=zero128)
                    nc.tensor.matmul(psx, ones128b, xb,
                                     start=(kc == 0), stop=(kc == KC - 1))
                    nc.tensor.matmul(psx2, ones128b, sq,
                                     start=(kc == 0), stop=(kc == KC - 1))
                mean = lns.tile([1, LTOT], f32, tag="ln_mean")
                nc.vector.tensor_scalar_mul(mean, psx, 1.0 / D)
                msq = lns.tile([1, LTOT], f32, tag="ln_msq")
                nc.vector.tensor_tensor(msq, mean, mean, OP.mult)
                var = lns.tile([1, LTOT], f32, tag="ln_var")
                nc.vector.scalar_tensor_tensor(var, psx2, 1.0 / D, msq, OP.mult, OP.subtract)
                std = lns.tile([1, LTOT], f32, tag="ln_std")
                nc.scalar.activation(std, var, AF.Sqrt, bias=eps_sb)
                rstd = lns.tile([1, LTOT], f32, tag="ln_rstd")
                nc.vector.reciprocal_approx_fast(rstd, std)
                rstd_b = lns.tile([1, LTOT], bf, tag="ln_rstdb")
                nc.vector.tensor_copy(rstd_b, rstd)
                mrstd_b = lns.tile([1, LTOT], bf, tag="ln_mrstdb")
                nc.vector.tensor_tensor(mrstd_b, mean, rstd, OP.mult)
                for kc in range(KC):
                    pg = psum_ln.tile([P, LTOT], f32, tag="ln_pg")
                    pm = psum_ln.tile([P, LTOT], f32, tag="ln_pm")
                    nc.tensor.matmul(pg, g[:, ts(kc, P)], rstd_b, start=True, stop=True)
                    nc.tensor.matmul(pm, g[:, ts(kc, P)], mrstd_b, start=True, stop=True)
                    t = lnp.tile([P, LTOT], f32, tag="ln_t")
                    nc.vector.tensor_tensor(t, x_sb[:, kc, :], pg, OP.mult)
                    nc.vector.scalar_tensor_tensor(out_sb[:, kc, :], t, lb[:, kc:kc + 1],
                                                   pm, OP.add, OP.subtract)
                    if bf_out is not None:
                        if bf_scale is not None:
                            nc.vector.tensor_scalar_mul(bf_out[:, kc, :],
                                                        out_sb[:, kc, :], bf_scale)
                        else:
                            nc.vector.tensor_copy(bf_out[:, kc, :], out_sb[:, kc, :])
                    if chunk_done is not None:
                        chunk_done(kc)

        def kv_proj_v(vpool_dst, src_sb, wv_dram, wrhs, wrhs_tag, n_tc, psum_proj):
            for half in range(2):
                wvh = wrhs.tile([P, KC, 512], f8, tag=wrhs_tag)
                nc.sync.dma_start(wvh, wv_dram[:, :, ts(half, 512)])
                for tci in range(n_tc):
                    ps = psum_proj.tile([P, 512], f32, tag="psproj")
                    for k2 in range(KC // 2):
                        nc.tensor.matmul(
                            ps, src_sb[:, 2 * k2:2 * k2 + 2, ts(tci, P)],
                            wvh[:, 2 * k2:2 * k2 + 2, :],
                            start=(k2 == 0), stop=(k2 == KC // 2 - 1),
                            perf_mode=DR)
                    nc.vector.tensor_copy(
                        vpool_dst[:, tci, half * 8:(half + 1) * 8, 0:DH],
                        ps.rearrange("p (h d) -> p h d", h=8))

        def den_recip(den_row, at_pool, inner):
            # den_row: [1, *inner] PSUM slice holding softmax denominators
            sums = at_pool.tile([1] + inner, f32, tag="at_sums")
            nc.vector.tensor_copy(sums, den_row)
            recip = at_pool.tile([1] + inner, f32, tag="at_recip")
            nc.vector.reciprocal_approx_fast(recip, sums)
            recip_b = at_pool.tile([1] + inner, bf, tag="at_recipb")
            nc.vector.tensor_scalar_mul(recip_b, recip, S_O / A_SC)
            return recip_b

        def apply_norm(dst, pv_rows, recip_b, at_pool, psum_bc, inner):
            # dst = pv_rows * broadcast(recip)
            pb = psum_bc.tile([DH] + inner, f32, tag="bc")
            nc.tensor.matmul(pb, ones64b, recip_b, start=True, stop=True)
            bc_sb = at_pool.tile([DH] + inner, f32, tag="at_bc")
            nc.vector.tensor_copy(bc_sb, pb)
            if len(inner) == 2:
                dst = dst.rearrange("p (j l) -> p j l", j=inner[0])
            nc.vector.scalar_tensor_tensor(dst, pv_rows, 0.0, bc_sb,
                                           OP.bypass, OP.mult)

        # =================== SA ===================
        with ExitStack() as sctx:
            sa_pool = sctx.enter_context(tc.tile_pool(name="sa", bufs=1))
            kT_sb = sa_pool.tile([P, KC, L], bf, tag="kT")
            v_sb = sa_pool.tile([P, TC_SA, H, DH + 1], bf, tag="v")
            qT_sb = sa_pool.tile([P, KC, LTOT], bf, tag="qT")
            nc.gpsimd.memset(v_sb[:, :, :, DH:DH + 1], 1.0)

            with ExitStack() as xctx:
                xpool = xctx.enter_context(tc.tile_pool(name="xpool", bufs=1))
                wrhs = xctx.enter_context(tc.tile_pool(name="wrhs", bufs=2))
                wkp = xctx.enter_context(tc.tile_pool(name="wk_sa_p", bufs=3))
                psum_kv = xctx.enter_context(tc.tile_pool(name="psum_kv", bufs=4, space="PSUM"))
                # first weight tile and the xT chunks lead the DMA queue so
                # the k-proj can start ~4us in; const loads queue behind them.
                wt0k = wkp.tile([P, KC, P], f8, tag="wtb")
                nc.sync.dma_start(wt0k, wd["wk_sa"][:, :, ts(0, P)])
                xT_sb = xpool.tile([P, KC, L], f8, tag="xT")
                for kc in range(KC):
                    nc.sync.dma_start(xT_sb[:, kc, :], xT_d[:, kc, :])
                qsrc = xpool.tile([P, KC, LTOT], f8, tag="qsrc")
                nc.sync.dma_start(qsrc, xTownb_d)
                load_consts()

                def evk(oc, tt, ps):
                    nc.vector.tensor_scalar_add(kT_sb[:, oc, ts(tt, 512)], ps,
                                                bias_sb["bk_sa"][:, oc:oc + 1])
                proj_to(wkp, "wtb", f8, xT_sb, wd["wk_sa"], KC, evk, psum_kv,
                        n_tt=L // 512, tt_width=512, dr=True, wt0=wt0k)

                def evq(oc, tt, ps):
                    nc.vector.tensor_scalar_add(qT_sb[:, oc, :], ps,
                                                bias_sb["bq_sa"][:, oc:oc + 1])
                proj_to(wkp, "wtb", f8, qsrc, wd["wq_sa"], KC, evq, psum_kv, dr=True)

                kv_proj_v(v_sb, xT_sb, wd["wv_sa"], wrhs, "wrhs", TC_SA, psum_kv)

            ca_pool = ctx.enter_context(tc.tile_pool(name="ca", bufs=1, side="right"))
            kcT_sb = ca_pool.tile([P, KC, S], bf, tag="kcT")
            vc_sb = ca_pool.tile([P, TC_CA, H, DH + 1], bf, tag="vc")
            encT_sb = ca_pool.tile([P, KC, S], f8, tag="encT")
            for kc in range(KC):
                nc.sync.dma_start(encT_sb[:, kc, :], encT_d[:, kc, :])
            nc.gpsimd.memset(vc_sb[:, :, :, DH:DH + 1], 1.0)
            oT_sb = sctx.enter_context(tc.tile_pool(name="oT_sa", bufs=1)).tile(
                [P, KC, LTOT], f8, tag="oT")
            pre_pool = ctx.enter_context(tc.tile_pool(name="prep", bufs=1, side="right"))
            h1pre = pre_pool.tile([P, KC, LTOT], f32, tag="pre")
            nc.sync.dma_start(h1pre, xTown_d)

            with ExitStack() as actx:
                e_pool = actx.enter_context(tc.tile_pool(name="e_sa", bufs=2))
                at_pool = actx.enter_context(tc.tile_pool(name="at_sa", bufs=2))
                wkvc = actx.enter_context(tc.tile_pool(name="wkv_ca", bufs=2))
                wrhsc = actx.enter_context(tc.tile_pool(name="wrhs_ca", bufs=2))
                psum_s = actx.enter_context(tc.tile_pool(name="psum_s", bufs=2, space="PSUM"))
                psum_pv = actx.enter_context(tc.tile_pool(name="psum_pv", bufs=1, space="PSUM"))
                psum_bc = actx.enter_context(tc.tile_pool(name="psum_bc", bufs=1, space="PSUM"))
                psum_ckv = actx.enter_context(tc.tile_pool(name="psum_ckv", bufs=1, space="PSUM"))

                ca_state = {}

                def ca_kv_prefetch(hc):
                    wt = wkvc.tile([P, KC, P], f8, tag="wt_ck")
                    nc.sync.dma_start(wt, wd["wk_ca"][:, :, ts(hc, P)])
                    ca_state[f"wt{hc}"] = wt
                    if hc % 4 == 0:
                        wvh_new = wrhsc.tile([P, KC, 512], f8, tag="wv_ca")
                        ca_state["wvh"] = wvh_new
                        nc.sync.dma_start(wvh_new,
                                          wd["wv_ca"][:, :, ts(hc // 4, 512)])

                def ca_kv_chunk(hc):
                    # 1/8 of CA k-proj and v-proj, emitted between SA head pairs
                    wt = ca_state.pop(f"wt{hc}")
                    for tt in range(S // 512):
                        ps = psum_ckv.tile([P, 512], f32, tag="ps_ckv")
                        for k2 in range(KC // 2):
                            nc.tensor.matmul(ps, wt[:, 2 * k2:2 * k2 + 2, :],
                                             encT_sb[:, 2 * k2:2 * k2 + 2, ts(tt, 512)],
                                             start=(k2 == 0), stop=(k2 == KC // 2 - 1),
                                             perf_mode=DR)
                        nc.vector.tensor_scalar_add(kcT_sb[:, hc, ts(tt, 512)], ps,
                                                    bias_sb["bk_ca"][:, hc:hc + 1])
                    half, tq = hc // 4, hc % 4
                    wvh = ca_state["wvh"]
                    for tci in (2 * tq, 2 * tq + 1):
                        ps = psum_ckv.tile([P, 512], f32, tag="ps_ckv")
                        for k2 in range(KC // 2):
                            nc.tensor.matmul(ps, encT_sb[:, 2 * k2:2 * k2 + 2, ts(tci, P)],
                                             wvh[:, 2 * k2:2 * k2 + 2, :],
                                             start=(k2 == 0), stop=(k2 == KC // 2 - 1),
                                             perf_mode=DR)
                        nc.vector.tensor_copy(
                            vc_sb[:, tci, half * 8:(half + 1) * 8, 0:DH],
                            ps.rearrange("p (h d) -> p h d", h=8))

                # compact slot layout over (tc, j>=jmin(tc)); causal skips j<tc//4
                jmin = [(tci // 4 if sa_mode == "causal" else 0) for tci in range(TC_SA)]
                bases = []
                nslot = 0
                for tci in range(TC_SA):
                    bases.append(nslot)
                    nslot += NL - jmin[tci]

                for hc in range(KC):  # head pair (2*hc, 2*hc+1)
                    ca_kv_prefetch(hc)
                    pv = psum_pv.tile([DH + 1, 2, NL, LW], f32, tag="pv")
                    e0 = e_pool.tile([P, nslot, LW], bf, tag="e_sa")
                    e1 = e_pool.tile([P, nslot, LW], bf, tag="e_sa")

                    def sa_scores(g0):
                        jm = jmin[g0]
                        N = (NL - jm) * LW
                        ps0 = psum_s.tile([P, 2, NL * LW], f32, tag="ps_sa")
                        ps1 = psum_s.tile([P, 2, NL * LW], f32, tag="ps_sa")
                        for u in range(2):
                            tci = g0 + u
                            loff = jm * LW
                            nc.tensor.matmul(
                                ps0[:, u, :N], kT_sb[0:DH, hc, ts(tci, P)],
                                qT_sb[0:DH, hc, loff:loff + N], start=True, stop=True)
                            nc.tensor.matmul(
                                ps1[:, u, :N], kT_sb[DH:P, hc, ts(tci, P)],
                                qT_sb[DH:P, hc, loff:loff + N], start=True, stop=True)
                        nsl = 2 * (NL - jm)
                        eo0 = e0[:, bases[g0]:bases[g0] + nsl, :].rearrange(
                            "p (u j) l -> p u j l", u=2)
                        eo1 = e1[:, bases[g0]:bases[g0] + nsl, :].rearrange(
                            "p (u j) l -> p u j l", u=2)
                        nc.scalar.activation(
                            eo0, ps0[:, :, :N].rearrange("p u (j l) -> p u j l", l=LW),
                            AF.Exp, bias=shift128, scale=INV_A2)
                        nc.scalar.activation(
                            eo1, ps1[:, :, :N].rearrange("p u (j l) -> p u j l", l=LW),
                            AF.Exp, bias=shift128, scale=INV_A2)

                    def sa_mask(j):
                        w = NL - j  # slot stride across the 4 diagonal chunks
                        for e_sb in (e0, e1):
                            view = e_sb[:, bases[4 * j]:bases[4 * j] + MREG * w, :]
                            view = view.rearrange("p (t w) l -> p t w l", w=w)[:, :, 0, :]
                            nc.vector.tensor_tensor(view, view,
                                                    expm_sb[:, j, :, :], OP.mult)

                    def sa_pv(tlo, thi):
                        for tci in range(tlo, thi):
                            jm = jmin[tci]
                            nc.tensor.matmul(
                                pv[:, 0, jm:, :], v_sb[:, tci, 2 * hc, :],
                                e0[:, bases[tci]:bases[tci] + NL - jm, :],
                                start=(tci == 0), stop=(tci == TC_SA - 1),
                                skip_group_check=True)
                            nc.tensor.matmul(
                                pv[:, 1, jm:, :], v_sb[:, tci, 2 * hc + 1, :],
                                e1[:, bases[tci]:bases[tci] + NL - jm, :],
                                start=(tci == 0), stop=(tci == TC_SA - 1),
                                skip_group_check=True)

                    if sa_mode == "causal":
                        # Interleave the PV chains one block-group behind the
                        # scores/exp so TensorE rides through the exp latency.
                        for j in range(NL):
                            sa_scores(4 * j)
                            if j > 0:
                                sa_pv(4 * (j - 1), 4 * j)
                            sa_scores(4 * j + 2)
                            sa_mask(j)
                        sa_pv(4 * (NL - 1), TC_SA)
                    else:
                        for g0 in range(0, TC_SA, 2):
                            sa_scores(g0)
                        if sa_mode == "generic":
                            nc.vector.tensor_tensor(e0, e0, expm_sb, OP.mult)
                            nc.vector.tensor_tensor(e1, e1, expm_sb, OP.mult)
                        sa_pv(0, TC_SA)
                    # reciprocal of the denominators runs on DVE while the
                    # interleaved CA kv-projection chunk keeps TensorE busy;
                    # the broadcast matmuls then find it ready.
                    recs = [den_recip(pv[DH:DH + 1, u], at_pool, [NL, LW])
                            for u in range(2)]
                    ca_kv_chunk(hc)
                    for u in range(2):
                        apply_norm(oT_sb[u * DH:(u + 1) * DH, hc, :],
                                   pv[0:DH, u], recs[u], at_pool, psum_bc,
                                   [NL, LW])

            with ExitStack() as octx:
                wop = octx.enter_context(tc.tile_pool(name="wo_sa_p", bufs=3))
                otp = octx.enter_context(tc.tile_pool(name="otmp_sa", bufs=2))
                psum_op = octx.enter_context(tc.tile_pool(name="psum_osa", bufs=4, space="PSUM"))

                def evo(oc, tt, ps):
                    # h1pre was pre-loaded with the residual (xTown); descale
                    # the fp8 o-proj on ScalarE, then accumulate in place
                    tmp = otp.tile([P, LTOT], f32, tag="otmp")
                    nc.scalar.activation(tmp, ps, AF.Identity,
                                         bias=bias_sb["bo_sa"][:, oc:oc + 1],
                                         scale=INV_OW)
                    nc.vector.tensor_tensor(h1pre[:, oc, :], tmp,
                                            h1pre[:, oc, :], OP.add)
                proj_to(wop, "wtb", f8, oT_sb, wd["wo_sa"], KC, evo, psum_op,
                        dr=True)

            h1_pool = ctx.enter_context(tc.tile_pool(name="h1p", bufs=1, side="right"))
            h1_sb = h1_pool.tile([P, KC, LTOT], f32, tag="h1")
            bfp = ctx.enter_context(tc.tile_pool(name="bfcast", bufs=1, side="right"))
            h1bf = bfp.tile([P, KC, LTOT], f8, tag="bfx")
            layer_norm("ln1", h1pre, gd["g1"], bias_sb["lb1"], h1_sb, bf_out=h1bf,
                       bf_scale=S_X)

        # =================== CA ===================
        with ExitStack() as cctx:
            qcT_sb = cctx.enter_context(tc.tile_pool(name="qc_ca", bufs=1)).tile(
                [P, KC, LTOT], bf, tag="qcT")

            with ExitStack() as xctx:
                wkp = xctx.enter_context(tc.tile_pool(name="wk_ca_p", bufs=3))
                psum_kv = xctx.enter_context(tc.tile_pool(name="psum_cq", bufs=4, space="PSUM"))

                def evqc(oc, tt, ps):
                    nc.vector.tensor_scalar_add(qcT_sb[:, oc, :], ps,
                                                bias_sb["bq_ca"][:, oc:oc + 1])
                proj_to(wkp, "wtb", f8, h1bf, wd["wq_ca"], KC, evqc, psum_kv,
                        dr=True)

            ocT_sb = cctx.enter_context(tc.tile_pool(name="oT_ca", bufs=1)).tile(
                [P, KC, LTOT], f8, tag="ocT")
            h2pre = pre_pool.tile([P, KC, LTOT], f32, tag="pre")

            with ExitStack() as actx:
                e_pool = actx.enter_context(tc.tile_pool(name="e_ca", bufs=4))
                at_pool = actx.enter_context(tc.tile_pool(name="at_ca", bufs=3))
                psum_s = actx.enter_context(tc.tile_pool(name="psum_cs", bufs=2, space="PSUM"))
                psum_pv = actx.enter_context(tc.tile_pool(name="psum_cpv", bufs=3, space="PSUM"))
                psum_bc = actx.enter_context(tc.tile_pool(name="psum_cbc", bufs=1, space="PSUM"))

                def flush_ca(st):
                    hc, pvu0, rec0, pvu1, rec1 = st
                    apply_norm(ocT_sb[0:DH, hc, :], pvu0[0:DH, :], rec0,
                               at_pool, psum_bc, [LTOT])
                    apply_norm(ocT_sb[DH:P, hc, :], pvu1[0:DH, :], rec1,
                               at_pool, psum_bc, [LTOT])

                # normalize for head pair hc-1 is deferred to between the two
                # PV chains of pair hc so its broadcast matmuls never stall
                # the tensor queue waiting on the DVE reciprocal.
                prev = None
                for hc in range(KC):  # head pair (2*hc, 2*hc+1)
                    ec0 = e_pool.tile([P, TC_CA, LTOT], bf, tag="ec")
                    ec1 = e_pool.tile([P, TC_CA, LTOT], bf, tag="ec")

                    def ca_scores(g0):
                        cs0 = psum_s.tile([P, 2, LTOT], f32, tag="cs")
                        cs1 = psum_s.tile([P, 2, LTOT], f32, tag="cs")
                        for u in range(2):
                            tci = g0 + u
                            nc.tensor.matmul(cs0[:, u, :],
                                             kcT_sb[0:DH, hc, ts(tci, P)],
                                             qcT_sb[0:DH, hc, :],
                                             start=True, stop=True)
                            nc.tensor.matmul(cs1[:, u, :],
                                             kcT_sb[DH:P, hc, ts(tci, P)],
                                             qcT_sb[DH:P, hc, :],
                                             start=True, stop=True)
                        nc.scalar.activation(ec0[:, g0:g0 + 2, :], cs0, AF.Exp,
                                             bias=shift128, scale=INV_A2)
                        nc.scalar.activation(ec1[:, g0:g0 + 2, :], cs1, AF.Exp,
                                             bias=shift128, scale=INV_A2)

                    def ca_pv(tlo, thi):
                        for tci in range(tlo, thi):
                            nc.tensor.matmul(pvc0, vc_sb[:, tci, 2 * hc, :],
                                             ec0[:, tci, :],
                                             start=(tci == 0), stop=(tci == TC_CA - 1),
                                             skip_group_check=True)
                            nc.tensor.matmul(pvc1, vc_sb[:, tci, 2 * hc + 1, :],
                                             ec1[:, tci, :],
                                             start=(tci == 0), stop=(tci == TC_CA - 1),
                                             skip_group_check=True)

                    if ca_mode == "generic":
                        # masks force a full-e barrier; keep the simple order
                        for g0 in range(0, TC_CA, 2):
                            ca_scores(g0)
                        nc.vector.tensor_tensor(ec0, ec0, expmc_sb, OP.mult)
                        nc.vector.tensor_tensor(ec1, ec1, expmc_sb, OP.mult)
                        pvc0 = psum_pv.tile([DH + 1, LTOT], f32, tag="pvc")
                        pvc1 = psum_pv.tile([DH + 1, LTOT], f32, tag="pvc")
                        if prev is not None:
                            flush_ca(prev)
                        ca_pv(0, TC_CA)
                    else:
                        # pipeline: pv lags scores by one pair-group; the
                        # deferred normalize of hc-1 fills the first slot.
                        ca_scores(0)
                        pvc0 = psum_pv.tile([DH + 1, LTOT], f32, tag="pvc")
                        pvc1 = psum_pv.tile([DH + 1, LTOT], f32, tag="pvc")
                        if prev is not None:
                            flush_ca(prev)
                        for g0 in range(2, TC_CA, 2):
                            ca_scores(g0)
                            ca_pv(g0 - 2, g0)
                        ca_pv(TC_CA - 2, TC_CA)
                    rec0 = den_recip(pvc0[DH:DH + 1, :], at_pool, [LTOT])
                    rec1 = den_recip(pvc1[DH:DH + 1, :], at_pool, [LTOT])
                    prev = (hc, pvc0, rec0, pvc1, rec1)
                flush_ca(prev)

            with ExitStack() as octx:
                wop = octx.enter_context(tc.tile_pool(name="wo_ca_p", bufs=3))
                otp = octx.enter_context(tc.tile_pool(name="otmp_ca", bufs=2))
                psum_op = octx.enter_context(tc.tile_pool(name="psum_oca", bufs=4, space="PSUM"))

                def evoc(oc, tt, ps):
                    tmp = otp.tile([P, LTOT], f32, tag="otmp")
                    nc.scalar.activation(tmp, ps, AF.Identity,
                                         bias=bias_sb["bo_ca"][:, oc:oc + 1],
                                         scale=INV_OW)
                    nc.vector.tensor_tensor(h2pre[:, oc, :], tmp,
                                            h1_sb[:, oc, :], OP.add)
                proj_to(wop, "wtb", f8, ocT_sb, wd["wo_ca"], KC, evoc, psum_op,
                        dr=True)

            h2_pool = ctx.enter_context(tc.tile_pool(name="h2p", bufs=1, side="right"))
            h2_sb = h2_pool.tile([P, KC, LTOT], f32, tag="h2")
            h2bf = bfp.tile([P, KC, LTOT], bf, tag="bfx")
            layer_norm("ln2", h2pre, gd["g2"], bias_sb["lb2"], h2_sb, bf_out=h2bf)

        # =================== FFN ===================
        with ExitStack() as fctx:
            ffn_pool = fctx.enter_context(tc.tile_pool(name="ffn", bufs=1))
            w2pool = fctx.enter_context(tc.tile_pool(name="wtile32", bufs=2))
            w1pool = fctx.enter_context(tc.tile_pool(name="w1p", bufs=3))
            psum_f = fctx.enter_context(tc.tile_pool(name="psum_f", bufs=4, space="PSUM"))
            f1_sb = ffn_pool.tile([P, KC2, LTOT], bf, tag="f1")
            h3pre = pre_pool.tile([P, KC, LTOT], f32, tag="pre")

            def evg(oc, tt, ps):
                nc.scalar.activation(f1_sb[:, oc, :], ps, AF.Gelu,
                                     bias=b1_sb[:, oc:oc + 1])
            proj_to(w1pool, "wtb", bf, h2bf, w1_d, KC2, evg, psum_f)

            for oc in range(KC):
                w2t = w2pool.tile([P, KC2, P], bf, tag="w2t")
                nc.sync.dma_start(w2t, w2_d[:, :, ts(oc, P)])
                ps = psum_f.tile([P, LTOT], f32, tag="psproj")
                for kc in range(KC2):
                    nc.tensor.matmul(ps, w2t[:, kc, :], f1_sb[:, kc, :],
                                     start=(kc == 0), stop=(kc == KC2 - 1))
                nc.vector.scalar_tensor_tensor(
                    h3pre[:, oc, :], ps, bias_sb["b2"][:, oc:oc + 1],
                    h2_sb[:, oc, :], OP.add, OP.add)

        out_sb = h1_pool.tile([P, KC, LTOT], f32, tag="h1")
        layer_norm("ln3", h3pre, gd["g3"], bias_sb["lb3"], out_sb,
                   chunk_done=lambda kc: nc.sync.dma_start(out_d[:, kc, :],
                                                           out_sb[:, kc, :]))


# ---------------------------------------------------------------------------
# Host-side packing
# ---------------------------------------------------------------------------

def _pack_wT(w, dtype=np.float32):
    # w: [dout, din] -> [P, din//P, dout] with wT[d, o] layout
    din = w.shape[1]
    return np.ascontiguousarray(
        w.T.reshape(din // P, P, w.shape[0]).transpose(1, 0, 2)).astype(dtype)


def _pack_xT(x, dtype=np.float32):
    # x: [T, D] -> [P, KC, T]
    t = x.shape[0]
    return np.ascontiguousarray(x.T.reshape(KC, P, t).transpose(1, 0, 2)).astype(dtype)


def _pack_bias(v):
    n = v.shape[0] // P
    return np.ascontiguousarray(v.reshape(n, P).T).astype(np.float32)


def detect_sa_mode(mask):
    if not np.isfinite(np.nan_to_num(mask, nan=np.inf)).all():
        return "generic"
    if (mask == 0).all():
        return "zeros"
    li, ti = np.tril_indices(L)
    if (mask[li, ti] == 0).all():
        ui, uj = np.triu_indices(L, k=1)
        if (mask[ui, uj] <= -1e8).all():
            return "causal"
    return "generic"


def make_in_maps(inputs):
    inputs = {k: np.asarray(v, dtype=np.float32) for k, v in inputs.items()}
    mask = inputs["attention_mask"]
    cmask = inputs["encoder_attention_mask"]
    sa_mode = detect_sa_mode(mask)
    ca_mode = "zeros" if (cmask == 0).all() else "generic"
    s = DH ** -0.5

    def fp8q(arr):
        return np.clip(arr, -240.0, 240.0).astype(FP8)

    A = S_X * S_W
    shared = {
        "wq_sa": fp8q(_pack_wT(inputs["sa_wq"] * (s * S_W))),
        "wk_sa": fp8q(_pack_wT(inputs["sa_wk"] * S_W)),
        "wv_sa": fp8q(_pack_wT(inputs["sa_wv"] * S_W)),
        "wo_sa": fp8q(_pack_wT(inputs["sa_wo"] * S_W)),
        "wq_ca": fp8q(_pack_wT(inputs["ca_wq"] * (s * S_W))),
        "wk_ca": fp8q(_pack_wT(inputs["ca_wk"] * S_W)),
        "wv_ca": fp8q(_pack_wT(inputs["ca_wv"] * S_W)),
        "wo_ca": fp8q(_pack_wT(inputs["ca_wo"] * S_W)),
        "w1": _pack_wT(inputs["ffn_w1"], BF16),
        "w2": _pack_wT(inputs["ffn_w2"], BF16),
        "biases": np.concatenate([
            _pack_bias(inputs["sa_bq"] * (s * A)),
            _pack_bias(inputs["sa_bk"] * A),
            _pack_bias(inputs["sa_bo"] + inputs["sa_wo"] @ inputs["sa_bv"]),
            _pack_bias(inputs["ca_bq"] * (s * A)),
            _pack_bias(inputs["ca_bk"] * A),
            _pack_bias(inputs["ca_bo"] + inputs["ca_wo"] @ inputs["ca_bv"]),
            _pack_bias(inputs["ffn_b2"]),
            _pack_bias(inputs["sa_ln_b"]),
            _pack_bias(inputs["ca_ln_b"]),
            _pack_bias(inputs["ffn_ln_b"]),
            _pack_bias(inputs["ffn_b1"]),
        ], axis=1),
        "g1": np.ascontiguousarray(inputs["sa_ln_g"].reshape(1, D)).astype(BF16),
        "g2": np.ascontiguousarray(inputs["ca_ln_g"].reshape(1, D)).astype(BF16),
        "g3": np.ascontiguousarray(inputs["ffn_ln_g"].reshape(1, D)).astype(BF16),
    }

    exts = EXT_CAUSAL if sa_mode == "causal" else [TC_SA] * NL
    in_maps = []
    for c in range(8):
        b, i = c // 4, c % 4
        blocks = core_blocks(i)
        own_rows = np.concatenate([np.arange(p * LW, (p + 1) * LW) for p in blocks])
        xTp32 = _pack_xT(inputs["hidden_states"][b])
        m = dict(shared)
        m["xT"] = fp8q(xTp32 * S_X)
        m["xTown"] = np.ascontiguousarray(xTp32[:, :, own_rows])
        m["xTownb"] = fp8q(m["xTown"] * S_X)
        m["encT"] = fp8q(_pack_xT(inputs["encoder_hidden_states"][b]) * S_X)
        if sa_mode == "causal":
            em = np.empty((P, NL, MREG, LW), dtype=BF16)
            for j, pblk in enumerate(blocks):
                rows = slice(pblk * LW, (pblk + 1) * LW)
                t0 = (exts[j] - MREG) * P
                blk = np.exp(np.minimum(mask[rows, t0:t0 + MREG * P], 60.0))
                em[:, j] = blk.reshape(LW, MREG, P).transpose(2, 1, 0)
            m["expm"] = em
        elif sa_mode == "generic":
            em = np.empty((P, TC_SA * NL, LW), dtype=BF16)
            for j, pblk in enumerate(blocks):
                rows = slice(pblk * LW, (pblk + 1) * LW)
                blk = np.exp(np.minimum(mask[rows, :], 60.0))
                em[:, j::NL, :] = blk.reshape(LW, TC_SA, P).transpose(2, 1, 0)
            m["expm"] = em
        if ca_mode == "generic":
            em = np.empty((P, TC_CA, LTOT), dtype=BF16)
            for j, pblk in enumerate(blocks):
                rows = slice(pblk * LW, (pblk + 1) * LW)
                blk = np.exp(np.minimum(cmask[rows, :], 60.0))
                em[:, :, j * LW:(j + 1) * LW] = blk.reshape(LW, TC_CA, P).transpose(2, 1, 0)
            m["expmc"] = em
        in_maps.append(m)
    return in_maps, sa_mode, ca_mode


def assemble_output(results):
    out = np.zeros((B, L, D), np.float32)
    for c in range(8):
        b, i = c // 4, c % 4
        arr = np.asarray(results[c]["out"])  # [P, KC, LTOT]
        for j, pblk in enumerate(core_blocks(i)):
            blk = arr[:, :, j * LW:(j + 1) * LW]          # [P, KC, LW]
            out[b, pblk * LW:(pblk + 1) * LW, :] = blk.transpose(2, 1, 0).reshape(LW, D)
    return out


# ---------------------------------------------------------------------------
# Entry point
# ---------------------------------------------------------------------------

_NC_CACHE = {}


def get_nc(sa_mode, ca_mode):
    key = (sa_mode, ca_mode)
    if key not in _NC_CACHE:
        _NC_CACHE[key] = build_nc(sa_mode, ca_mode)
    return _NC_CACHE[key]


def _install_ntff_hook():
    """bass_utils' trace path needs antenv.axon_hooks, absent in this image.
    Inject a shim and register the ctypes-based NTFF hook from trn_agent_boot."""
    import types
    if "antenv.axon_hooks" in sys.modules:
        return
    holder = {}
    mod = types.ModuleType("antenv.axon_hooks")
    mod.set_axon_ntff_profile_hook = lambda h: holder.__setitem__("h", h)
    mod.get_axon_ntff_profile_hook = lambda: holder.get("h")
    sys.modules["antenv.axon_hooks"] = mod
    try:
        import antenv
        antenv.axon_hooks = mod
    except ImportError:
        pass
    try:
        from trn_agent_boot.trn_boot import _ntff_profile_via_ctypes
        hook = _ntff_profile_via_ctypes("/opt/axon/libaxon_pjrt.so")
        if hook is not None:
            mod.set_axon_ntff_profile_hook(hook)
    except Exception as e:  # degrade to no tracing
        print(f"ntff hook install failed: {e}", file=sys.stderr)


def run(inputs, trace=False):
    _install_ntff_hook()
    from concourse.bass_utils import run_bass_kernel_spmd
    in_maps, sa_mode, ca_mode = make_in_maps(inputs)
    nc = get_nc(sa_mode, ca_mode)
    res = run_bass_kernel_spmd(nc, in_maps, core_ids=list(range(8)), trace=trace)
    return assemble_output(res.results), res


def kernel(**inputs):
    out, _ = run(inputs, trace=False)
    return out

